# revision 1
# baseline (speedup 1.0000x reference)
"""Fused RoBERTa layer (attention + FFN, LoRA merged) on 8 Trainium2 cores.

Sharding: pure data-parallel over batch (16 batches -> 2 per core), no
collectives. LoRA low-rank updates are merged into the base weight matrices
on the host (x@W + (x@A)@B == x@(W + A@B) exactly in linear algebra), and the
1/sqrt(head_dim) score scale is folded into W_q/b_q.

Device dataflow per core (T=1024 tokens = 2 batches):
  - x^T (feature-major, bf16) resident in SBUF
  - q^T,k^T feature-major; V token-major with a ones-column interleaved per
    head ([k, 65] per head) so the attention-value matmul also produces the
    softmax denominator row for free
  - scores computed TRANSPOSED [k, q] so (a) the attention mask is a
    per-partition bias on the ScalarE Exp activation, and (b) exp'd scores
    are directly the rhs of the AV matmul -- no transposes in attention
  - o comes out feature-major -> O-proj directly; +x residual; LN1
  - FFN up produces F-major tiles -> Gelu evict is directly the down-proj
    operand; down + residual; LN2 -> out
All matmul operands bf16, accumulation fp32 (PSUM), softmax/LN math fp32.

SBUF is managed as a single "slab" pool of [128, 1040]-bf16 slots; later
phases allocate tiles with the tags of dead earlier tensors, so the Tile
framework recycles addresses with automatic WAR dependencies (pool closes
must be LIFO, which phase lifetimes here are not).
"""

import sys

sys.path.insert(0, "/opt/trn_rl_repo")

import numpy as np
import ml_dtypes

import concourse.bacc as bacc
import concourse.bass as bass
import concourse.tile as tile
from concourse import mybir
from concourse.bass_utils import run_bass_kernel_spmd
from concourse.masks import make_identity

BF16 = mybir.dt.bfloat16
F32 = mybir.dt.float32
NP_BF16 = np.dtype(ml_dtypes.bfloat16)

# Problem shape (hardcoded per spec)
B, S, D, H, HD, F = 16, 512, 1024, 16, 64, 4096
N_CORES = 8
TB = B // N_CORES          # batches per core
T = TB * S                 # tokens per core

MM_N = 512                 # matmul moving free dim / PSUM bank width (fp32)
P = 128


def _ceil_div(a, b):
    return (a + b - 1) // b


def build_program(cfg):
    """Build the SPMD bass program. cfg is a dict of gate flags + sizes."""
    D_, F_, T_, TB_, H_, HD_ = (cfg["D"], cfg["F"], cfg["T"], cfg["TB"],
                                cfg["H"], cfg["HD"])
    S_ = T_ // TB_
    KD = D_ // P               # D partition chunks
    KF = F_ // P               # F partition chunks
    TCH = T_ // P              # token chunks
    NT = _ceil_div(T_, MM_N)   # projection N tiles over tokens
    NTW = min(MM_N, T_)        # projection N tile width
    ND = _ceil_div(D_, MM_N)   # N tiles over D
    NDW = min(MM_N, D_)
    SKC = S_ // P              # key chunks per batch
    HPC = P // HD_             # heads per 128-partition chunk
    VW = HD_ + 1               # V' per-head width (ones column appended)
    UPW = min(1024, F_)        # wup column tile width
    UPT = F_ // UPW            # wup column tiles per D-chunk

    nc = bacc.Bacc("TRN2", target_bir_lowering=False, debug=False,
                   num_devices=N_CORES)

    # ---- DRAM I/O ----
    xT_d = nc.dram_tensor("xT", [D_, T_], BF16, kind="ExternalInput")
    x32_d = nc.dram_tensor("x32", [T_, D_], F32, kind="ExternalInput")
    wq_d = nc.dram_tensor("wq", [D_, D_], BF16, kind="ExternalInput")
    wk_d = nc.dram_tensor("wk", [D_, D_], BF16, kind="ExternalInput")
    wv_d = nc.dram_tensor("wv", [D_, D_], BF16, kind="ExternalInput")
    wo_d = nc.dram_tensor("wo", [D_, D_], BF16, kind="ExternalInput")
    wup_d = nc.dram_tensor("wup", [D_, F_], BF16, kind="ExternalInput")
    wdn_d = nc.dram_tensor("wdn", [F_, D_], BF16, kind="ExternalInput")
    bq_d = nc.dram_tensor("bq", [D_], F32, kind="ExternalInput")
    bk_d = nc.dram_tensor("bk", [D_], F32, kind="ExternalInput")
    bup_d = nc.dram_tensor("bup", [F_], F32, kind="ExternalInput")
    mask_d = nc.dram_tensor("maskT", [TB_, S_], F32, kind="ExternalInput")
    # gated (only read when the host says they are non-trivial)
    bv_d = nc.dram_tensor("bv", [D_], F32, kind="ExternalInput")
    bo_d = nc.dram_tensor("bo", [D_], F32, kind="ExternalInput")
    bdn_d = nc.dram_tensor("bdn", [D_], F32, kind="ExternalInput")
    g1_d = nc.dram_tensor("g1", [D_], F32, kind="ExternalInput")
    b1_d = nc.dram_tensor("b1", [D_], F32, kind="ExternalInput")
    g2_d = nc.dram_tensor("g2", [D_], F32, kind="ExternalInput")
    b2_d = nc.dram_tensor("b2", [D_], F32, kind="ExternalInput")
    out_d = nc.dram_tensor("out", [T_, D_], F32, kind="ExternalOutput")

    with tile.TileContext(nc) as tc, \
         tc.tile_pool(name="consts", bufs=1) as consts, \
         tc.tile_pool(name="slab", bufs=1) as slab, \
         tc.tile_pool(name="ps", bufs=2, space="PSUM") as ps, \
         tc.tile_pool(name="psS", bufs=2, space="PSUM") as psS, \
         tc.tile_pool(name="psV", bufs=2, space="PSUM") as psV, \
         tc.tile_pool(name="psT", bufs=2, space="PSUM") as psT, \
         tc.tile_pool(name="work", bufs=2) as work, \
         tc.tile_pool(name="x32p", bufs=2) as x32p, \
         tc.tile_pool(name="attnp", bufs=10) as attnp, \
         tc.tile_pool(name="attn2", bufs=3) as attn2, \
         tc.tile_pool(name="statp", bufs=4) as statp, \
         tc.tile_pool(name="outp", bufs=2) as outp, \
         tc.tile_pool(name="dramp", bufs=6, space="DRAM") as dramp:

        dma = nc.sync

        def slot(tag, width=None, dtype=BF16):
            # one [128, VW*HPC*... ] slab slot; width defaults to max layout
            w = (D_ // HD_) * VW if width is None else width
            return slab.tile([P, w], dtype, tag=tag, name=f"t_{tag}")

        # ---- constants ----
        eps_t = consts.tile([P, 1], F32)
        nc.vector.memset(eps_t, 1e-5)
        ident = consts.tile([P, P], BF16)
        make_identity(nc, ident)
        # per-partition bias layouts: col m <- bias[m*128:(m+1)*128]
        bq_sb = consts.tile([P, KD], F32)
        dma.dma_start(out=bq_sb, in_=bq_d.ap().rearrange("(m p) -> p m", p=P))
        bk_sb = consts.tile([P, KD], F32)
        dma.dma_start(out=bk_sb, in_=bk_d.ap().rearrange("(m p) -> p m", p=P))
        bup_sb = consts.tile([P, KF], F32)
        dma.dma_start(out=bup_sb, in_=bup_d.ap().rearrange("(m p) -> p m", p=P))
        # mask as per-partition bias: col (b*SKC+kc) <- mask[b, kc*128:+128]
        mask_sb = consts.tile([P, TB_ * SKC], F32)
        dma.dma_start(out=mask_sb,
                      in_=mask_d.ap().rearrange("b (kc p) -> p (b kc)", p=P))

        def bcast_row(dram_vec, n):
            # [n] fp32 -> [P, n] tile broadcast across partitions
            t = consts.tile([P, n], F32, name=f"bc_{dram_vec.name}")
            dma.dma_start(out=t,
                          in_=dram_vec.ap().unsqueeze(0).to_broadcast([P, n]))
            return t

        bv_bc = bcast_row(bv_d, D_) if cfg["has_bv"] else None
        bo_bc = bcast_row(bo_d, D_) if cfg["has_bo"] else None
        bdn_bc = bcast_row(bdn_d, D_) if cfg["has_bdn"] else None
        g1_bc = bcast_row(g1_d, D_) if cfg["has_n1"] else None
        b1_bc = bcast_row(b1_d, D_) if cfg["has_n1"] else None
        g2_bc = bcast_row(g2_d, D_) if cfg["has_n2"] else None
        b2_bc = bcast_row(b2_d, D_) if cfg["has_n2"] else None

        # ---- load x^T and QKV weights ----
        xT_sb = [slot(f"xT{c}", T_) for c in range(KD)]
        w_sb = {nm: [slot(f"w{nm}{c}", D_) for c in range(KD)]
                for nm in ("q", "k", "v")}
        for c in range(KD):
            dma.dma_start(out=xT_sb[c], in_=xT_d[c * P:(c + 1) * P, :])
            for nm, wd in (("q", wq_d), ("k", wk_d), ("v", wv_d)):
                dma.dma_start(out=w_sb[nm][c], in_=wd[c * P:(c + 1) * P, :])

        qT_sb = [slot(f"qT{c}", T_) for c in range(KD)]
        # k^T is stored twice with the other head-half zeroed, so attention
        # scores can contract over the full 128 partitions (K=64 matmuls
        # de-rate the whole PE stream on this part)
        kTe_sb = [slot(f"kTe{c}", T_) for c in range(KD)]
        kTo_sb = [slot(f"kTo{c}", T_) for c in range(KD)]
        for c in range(KD):
            nc.vector.memset(kTe_sb[c][P // 2:P, :], 0.0)
            nc.vector.memset(kTo_sb[c][0:P // 2, :], 0.0)
        Vp_sb = [slot(f"Vp{c}") for c in range(TCH)]

        # ---- QKV projections ----
        HB = P // 2
        for nm, bias in (("q", bq_sb), ("k", bk_sb)):
            for m in range(KD):
                for t2 in range(NT):
                    pt = ps.tile([P, MM_N], F32, tag="ps", name="ps_qk")
                    for kc in range(KD):
                        nc.tensor.matmul(
                            pt[:, :NTW],
                            lhsT=w_sb[nm][kc][:, m * P:(m + 1) * P],
                            rhs=xT_sb[kc][:, t2 * MM_N:t2 * MM_N + NTW],
                            start=(kc == 0), stop=(kc == KD - 1))
                    sl = slice(t2 * MM_N, t2 * MM_N + NTW)
                    if nm == "q":
                        nc.vector.tensor_scalar_add(
                            out=qT_sb[m][:, sl],
                            in0=pt[:, :NTW], scalar1=bias[:, m:m + 1])
                    else:
                        nc.vector.tensor_scalar_add(
                            out=kTe_sb[m][0:HB, sl],
                            in0=pt[0:HB, :NTW], scalar1=bias[0:HB, m:m + 1])
                        nc.vector.tensor_scalar_add(
                            out=kTo_sb[m][HB:P, sl],
                            in0=pt[HB:P, :NTW], scalar1=bias[HB:P, m:m + 1])
        # V token-major, scattered into V' layout (ones col per head)
        for tr in range(TCH):
            for n2 in range(ND):
                pt = ps.tile([P, MM_N], F32, tag="ps", name="ps_v")
                for kc in range(KD):
                    nc.tensor.matmul(
                        pt[:, :NDW],
                        lhsT=xT_sb[kc][:, tr * P:(tr + 1) * P],
                        rhs=w_sb["v"][kc][:, n2 * MM_N:n2 * MM_N + NDW],
                        start=(kc == 0), stop=(kc == KD - 1))
                hpn = NDW // HD_  # heads per N tile
                dst = Vp_sb[tr].rearrange("p (h c) -> p h c", c=VW)
                src = pt[:, :NDW].rearrange("p (h c) -> p h c", c=HD_)
                if cfg["has_bv"]:
                    tmp = work.tile([P, NDW], F32, tag="vtmp", name="vtmp")
                    nc.vector.tensor_add(
                        out=tmp, in0=pt[:, :NDW],
                        in1=bv_bc[:, n2 * MM_N:n2 * MM_N + NDW])
                    src = tmp.rearrange("p (h c) -> p h c", c=HD_)
                nc.vector.tensor_copy(
                    out=dst[:, n2 * hpn:(n2 + 1) * hpn, 0:HD_], in_=src)
            nc.vector.memset(
                Vp_sb[tr].rearrange("p (h c) -> p h c", c=VW)[:, :, HD_:VW],
                1.0)

        skip = cfg.get("skip", set())
        # ---- attention -> o^T feature-major ----
        wo_sb = []
        for c in range(KD):
            t = slot(f"xT{c}", D_)   # reuse xT slots
            dma.dma_start(out=t, in_=wo_d[c * P:(c + 1) * P, :])
            wo_sb.append(t)
        oT_sb = [slot(f"wq{c}", T_) for c in range(KD)]  # reuse wq slots

        if "attn" in skip:
            for c in range(KD):
                nc.vector.tensor_copy(out=oT_sb[c], in_=qT_sb[c])
        attn_iter = [] if "attn" in skip else [(b, h) for b in range(TB_)
                                               for h in range(H_)]
        for b, h in attn_iter:
                hc, ho = h // HPC, (h % HPC) * HD_
                # scores^T [k, q] per key chunk; exp via ScalarE (mask = bias)
                at_tiles = []
                for kc in range(SKC):
                    at = attnp.tile([P, S_], BF16, tag="attnT", name="attnT")
                    if True:
                        kTm = kTe_sb if (h % HPC) == 0 else kTo_sb
                        pt = psS.tile([P, MM_N], F32, tag="psS", name="ps_s")
                        nc.tensor.matmul(
                            pt[:, :S_],
                            lhsT=kTm[hc][:, b * S_ + kc * P:
                                         b * S_ + (kc + 1) * P],
                            rhs=qT_sb[hc][:, b * S_:(b + 1) * S_],
                            start=True, stop=True)
                        nc.scalar.activation(
                            out=at, in_=pt[:, :S_],
                            func=mybir.ActivationFunctionType.Exp,
                            bias=mask_sb[:, b * SKC + kc:b * SKC + kc + 1],
                            scale=1.0)
                    at_tiles.append(at)
                # o^T rows [HD] + denominator row, accumulated over key chunks
                pv = psV.tile([P, MM_N], F32, tag="psV", name="ps_v2")
                for kc in range(SKC):
                    nc.tensor.matmul(
                        pv[0:VW, :S_],
                        lhsT=Vp_sb[(b * S_) // P + kc][:,
                                                       h * VW:(h + 1) * VW],
                        rhs=at_tiles[kc],
                        start=(kc == 0), stop=(kc == SKC - 1))
                rs = attn2.tile([1, S_], F32, tag="rsum", name="rsum")
                nc.scalar.copy(out=rs, in_=pv[HD_:VW, :S_])
                rs_d = dramp.tile([1, S_], F32, tag="rs_d", name="rs_d")
                dma.dma_start(out=rs_d, in_=rs)
                rb = attn2.tile([HD_, S_], F32, tag="rsbc", name="rsbc")
                dma.dma_start(out=rb, in_=rs_d.to_broadcast([HD_, S_]))
                nc.vector.reciprocal_approx_fast(out=rb, in_=rb)
                nc.vector.tensor_mul(
                    out=oT_sb[hc][ho:ho + HD_, b * S_:(b + 1) * S_],
                    in0=pv[0:HD_, :S_], in1=rb)

        # ---- O projection + residual + LN1 -> x_medium (bf16) + transpose ----
        xm_bf = [slot(f"wk{c}", D_) for c in range(TCH)]   # reuse wk slots
        xmT_sb = [slot(f"wv{c}", T_) for c in range(KD)]   # reuse wv slots
        up_tags = ([f"qT{c}" for c in range(KD)]
                   + [f"kTe{c}" for c in range(KD)]
                   + [f"kTo{c}" for c in range(KD)]
                   + [f"Vp{c}" for c in range(TCH)]
                   + [f"u{c}" for c in range(max(0, KD * UPT - 3 * KD - TCH))])
        wup_sb = []
        for i in range(KD * UPT):
            t = slot(up_tags[i], UPW)
            dk, j = i // UPT, i % UPT
            dma.dma_start(out=t, in_=wup_d[dk * P:(dk + 1) * P,
                                           j * UPW:(j + 1) * UPW])
            wup_sb.append(t)

        def wup_lhsT(dk, fm):
            i = dk * UPT + (fm * P) // UPW
            o = (fm * P) % UPW
            return wup_sb[i][:, o:o + P]

        def layer_norm(src, dst, g_bc, b_bc):
            # src fp32 [P, D_]; dst [P, D_] (any dtype); per-partition stats
            bw = min(512, D_)
            nsub = _ceil_div(D_, bw)
            st = statp.tile([P, nsub, 6], F32, tag="bnst", name="bnst")
            for i in range(nsub):
                nc.vector.bn_stats(out=st[:, i, :],
                                   in_=src[:, i * bw:(i + 1) * bw])
            mv = statp.tile([P, 2], F32, tag="bnmv", name="bnmv")
            nc.vector.bn_aggr(out=mv, in_=st)
            rstd = statp.tile([P, 1], F32, tag="rstd", name="rstd")
            nc.scalar.activation(out=rstd, in_=mv[:, 1:2],
                                 func=mybir.ActivationFunctionType.Sqrt,
                                 bias=eps_t, scale=1.0)
            nc.vector.reciprocal(out=rstd, in_=rstd)
            if g_bc is None:
                nc.vector.tensor_scalar(
                    out=dst, in0=src, scalar1=mv[:, 0:1], scalar2=rstd,
                    op0=mybir.AluOpType.subtract, op1=mybir.AluOpType.mult)
            else:
                tmp = statp.tile([P, D_], F32, tag="lntmp", name="lntmp")
                nc.vector.tensor_scalar(
                    out=tmp, in0=src, scalar1=mv[:, 0:1], scalar2=rstd,
                    op0=mybir.AluOpType.subtract, op1=mybir.AluOpType.mult)
                nc.vector.tensor_mul(out=tmp, in0=tmp, in1=g_bc)
                nc.vector.tensor_add(out=dst, in0=tmp, in1=b_bc)

        for tr in range(TCH):
            xt = x32p.tile([P, D_], F32, tag="x32t", name="x32t")
            dma.dma_start(out=xt, in_=x32_d[tr * P:(tr + 1) * P, :])
            of = work.tile([P, D_], F32, tag="acc", name="of")
            for n2 in range(ND):
                pt = ps.tile([P, MM_N], F32, tag="ps", name="ps_o")
                for kc in range(KD):
                    nc.tensor.matmul(
                        pt[:, :NDW],
                        lhsT=oT_sb[kc][:, tr * P:(tr + 1) * P],
                        rhs=wo_sb[kc][:, n2 * MM_N:n2 * MM_N + NDW],
                        start=(kc == 0), stop=(kc == KD - 1))
                nc.vector.tensor_add(out=of[:, n2 * MM_N:n2 * MM_N + NDW],
                                     in0=pt[:, :NDW],
                                     in1=xt[:, n2 * MM_N:n2 * MM_N + NDW])
                if cfg["has_bo"]:
                    nc.vector.tensor_add(
                        out=of[:, n2 * MM_N:n2 * MM_N + NDW],
                        in0=of[:, n2 * MM_N:n2 * MM_N + NDW],
                        in1=bo_bc[:, n2 * MM_N:n2 * MM_N + NDW])
            if "ln" in skip:
                nc.vector.tensor_copy(out=xm_bf[tr], in_=of)
            else:
                layer_norm(of, xm_bf[tr],
                           g1_bc if cfg["has_n1"] else None,
                           b1_bc if cfg["has_n1"] else None)
            # transpose x_medium chunk -> xmT (PE transpose via identity)
            for c in range(KD):
                if "tr" in skip:
                    nc.vector.tensor_copy(
                        out=xmT_sb[c][:, tr * P:(tr + 1) * P],
                        in_=xm_bf[tr][:, c * P:(c + 1) * P])
                else:
                    pt = psT.tile([P, P], BF16, tag="psT", name="ps_t")
                    nc.tensor.transpose(pt, xm_bf[tr][:, c * P:(c + 1) * P],
                                        ident)
                    nc.scalar.copy(
                        out=xmT_sb[c][:, tr * P:(tr + 1) * P], in_=pt)

        # ---- FFN up (F-major out) + Gelu -> gT ----
        g_tags = ([f"xT{c}" for c in range(KD)] + [f"wq{c}" for c in range(KD)]
                  + [f"g{c}" for c in range(KF - 2 * KD)])
        gT_sb = [slot(g_tags[c], T_) for c in range(KF)]
        for fm in range(KF):
            for t2 in range(NT):
                pt = ps.tile([P, MM_N], F32, tag="ps", name="ps_up")
                for kc in range(KD):
                    nc.tensor.matmul(
                        pt[:, :NTW],
                        lhsT=wup_lhsT(kc, fm),
                        rhs=xmT_sb[kc][:, t2 * MM_N:t2 * MM_N + NTW],
                        start=(kc == 0), stop=(kc == KD - 1))
                nc.scalar.activation(
                    out=gT_sb[fm][:, t2 * MM_N:t2 * MM_N + NTW],
                    in_=pt[:, :NTW],
                    func=mybir.ActivationFunctionType.Gelu,
                    bias=bup_sb[:, fm:fm + 1], scale=1.0)

        # ---- FFN down + residual + LN2 -> out ----
        wdn_sb = []
        for fc in range(KF):
            t = slot(up_tags[fc], D_)   # reuse wup slots
            dma.dma_start(out=t, in_=wdn_d[fc * P:(fc + 1) * P, :])
            wdn_sb.append(t)
        for tr in range(TCH):
            dsb = work.tile([P, D_], F32, tag="acc", name="dsb")
            for n2 in range(ND):
                pt = ps.tile([P, MM_N], F32, tag="ps", name="ps_dn")
                for fc in range(KF):
                    nc.tensor.matmul(
                        pt[:, :NDW],
                        lhsT=gT_sb[fc][:, tr * P:(tr + 1) * P],
                        rhs=wdn_sb[fc][:, n2 * MM_N:n2 * MM_N + NDW],
                        start=(fc == 0), stop=(fc == KF - 1))
                nc.vector.tensor_add(
                    out=dsb[:, n2 * MM_N:n2 * MM_N + NDW],
                    in0=pt[:, :NDW],
                    in1=xm_bf[tr][:, n2 * MM_N:n2 * MM_N + NDW])
                if cfg["has_bdn"]:
                    nc.vector.tensor_add(
                        out=dsb[:, n2 * MM_N:n2 * MM_N + NDW],
                        in0=dsb[:, n2 * MM_N:n2 * MM_N + NDW],
                        in1=bdn_bc[:, n2 * MM_N:n2 * MM_N + NDW])
            ot = outp.tile([P, D_], F32, tag="ot", name="ot")
            if "ln" in skip:
                nc.vector.tensor_copy(out=ot, in_=dsb)
            else:
                layer_norm(dsb, ot,
                           g2_bc if cfg["has_n2"] else None,
                           b2_bc if cfg["has_n2"] else None)
            dma.dma_start(out=out_d[tr * P:(tr + 1) * P, :], in_=ot)

    nc.finalize()
    return nc


_PROGRAM_CACHE = {}


def _get_program(cfg_key, cfg):
    if cfg_key not in _PROGRAM_CACHE:
        _PROGRAM_CACHE[cfg_key] = build_program(cfg)
    return _PROGRAM_CACHE[cfg_key]


def make_in_maps(inputs):
    """Host-side prep: LoRA merge, scale fold, dtype conversion, sharding."""
    f32 = np.float32
    x = np.asarray(inputs["x"], f32)
    scale = 1.0 / np.sqrt(float(inputs["head_dim"]))

    def merged(w, a, b):
        return (np.asarray(w, f32)
                + np.asarray(a, f32) @ np.asarray(b, f32))

    wq = (merged(inputs["w_q"], inputs["w_q_lora_a"], inputs["w_q_lora_b"])
          * scale).astype(NP_BF16)
    wk = merged(inputs["w_k"], inputs["w_k_lora_a"],
                inputs["w_k_lora_b"]).astype(NP_BF16)
    wv = merged(inputs["w_v"], inputs["w_v_lora_a"],
                inputs["w_v_lora_b"]).astype(NP_BF16)
    wo = merged(inputs["w_o"], inputs["w_o_lora_a"],
                inputs["w_o_lora_b"]).astype(NP_BF16)
    wup = merged(inputs["w_up"], inputs["w_up_lora_a"],
                 inputs["w_up_lora_b"]).astype(NP_BF16)
    wdn = merged(inputs["w_down"], inputs["w_down_lora_a"],
                 inputs["w_down_lora_b"]).astype(NP_BF16)
    mask = np.asarray(inputs["attention_mask"], f32)  # [B,1,1,S]

    common = {
        "wq": wq, "wk": wk, "wv": wv, "wo": wo, "wup": wup, "wdn": wdn,
        "bq": (np.asarray(inputs["b_q"], f32) * scale).astype(f32),
        "bk": np.asarray(inputs["b_k"], f32),
        "bup": np.asarray(inputs["b_up"], f32),
        "bv": np.asarray(inputs["b_v"], f32),
        "bo": np.asarray(inputs["b_o"], f32),
        "bdn": np.asarray(inputs["b_down"], f32),
        "g1": np.asarray(inputs["norm_weight_1"], f32),
        "b1": np.asarray(inputs["norm_bias_1"], f32),
        "g2": np.asarray(inputs["norm_weight_2"], f32),
        "b2": np.asarray(inputs["norm_bias_2"], f32),
    }
    in_maps = []
    for i in range(N_CORES):
        xc = x[i * TB:(i + 1) * TB].reshape(T, D)
        m = dict(common)
        m["xT"] = np.ascontiguousarray(xc.T).astype(NP_BF16)
        m["x32"] = np.ascontiguousarray(xc)
        m["maskT"] = np.ascontiguousarray(mask[i * TB:(i + 1) * TB, 0, 0, :])
        in_maps.append(m)
    return in_maps


def full_cfg(inputs):
    f32 = np.float32
    return {
        "D": D, "F": F, "T": T, "TB": TB, "H": H, "HD": HD,
        "has_bv": bool(np.any(np.asarray(inputs["b_v"], f32))),
        "has_bo": bool(np.any(np.asarray(inputs["b_o"], f32))),
        "has_bdn": bool(np.any(np.asarray(inputs["b_down"], f32))),
        "has_n1": bool(np.any(np.asarray(inputs["norm_weight_1"], f32) != 1.0)
                       or np.any(np.asarray(inputs["norm_bias_1"], f32))),
        "has_n2": bool(np.any(np.asarray(inputs["norm_weight_2"], f32) != 1.0)
                       or np.any(np.asarray(inputs["norm_bias_2"], f32))),
    }


def run_on_hw(inputs, trace=False, tmpdir=None):
    cfg = full_cfg(inputs)
    cfg_key = tuple(sorted((k, v) for k, v in cfg.items()))
    nc = _get_program(cfg_key, cfg)
    in_maps = make_in_maps(inputs)
    kw = {}
    if trace:
        kw = {"trace": True, "tmpdir": tmpdir}
    res = run_bass_kernel_spmd(nc, in_maps, core_ids=list(range(N_CORES)),
                               **kw)
    out = np.empty((B, S, D), np.float32)
    for i in range(N_CORES):
        out[i * TB:(i + 1) * TB] = res.results[i]["out"].reshape(TB, S, D)
    return out, res


def kernel(**inputs):
    out, _ = run_on_hw(inputs)
    return out



# revision 8
# speedup vs baseline: 1.2322x; 1.2322x over previous
"""Fused RoBERTa layer (attention + FFN, LoRA merged) on 8 Trainium2 cores.

Sharding: pure data-parallel over batch (16 batches -> 2 per core), no
collectives. LoRA low-rank updates are merged into the base weight matrices
on the host (x@W + (x@A)@B == x@(W + A@B) exactly in linear algebra), and the
1/sqrt(head_dim) score scale is folded into W_q/b_q.

fp8 strategy (2x PE throughput via DoubleRow double-pumping):
  - QKV / AV / O-proj / FFN-up matmuls run fp8e4m3 with
    MatmulPerfMode.DoubleRow: both operands hold TWO 128-deep K-chunks side
    by side in the free dim ([128, 2, M]), contracting 256 per pass.
  - FFN-down stays bf16: fp8 there pushes the output past the accuracy gate.
  - Weights are pre-scaled by 2^7 on the host (2^10 for w_q, which also
    folds 1/sqrt(hd)) so sigma~0.02 entries sit in fp8's normal range; the
    inverse scales are folded for free into downstream activation input
    scales (exp: 2^-17, gelu: 2^-7) and into a host-side 2^7 pre-scale of
    the residual x (LayerNorm is scale-invariant, so LN1(2^7*(o+x)) ==
    LN1(o+x) up to a negligible eps shift).
  - The exp that produces attention weights also folds in a 2^-9 output
    scale (via its additive bias: exp(x - 9ln2)) so the unnormalized fp8
    attention weights and their V-products stay in fp8/bf16 range; the
    softmax denominator comes from a ones-column in V' and is the sum of
    the SAME fp8 weights, so the scale cancels exactly.

Device dataflow per core (T=1024 tokens = 2 batches):
  - x^T (feature-major, fp8, K-paired) resident in SBUF
  - q^T,k^T feature-major bf16 (scores stay bf16); V' token-major fp8 with
    a ones-column per head ([64+1] per head), K-chunk-paired for DoubleRow
  - scores computed TRANSPOSED [k, q] so the attention mask is a
    per-partition bias on the ScalarE Exp activation and exp'd scores are
    directly the rhs of the AV matmul -- no transposes in attention
  - o comes out feature-major fp8 -> O-proj directly; +x residual; LN1
  - FFN up produces F-major tiles -> Gelu evict is directly the down-proj
    operand; down (bf16) + residual; LN2 -> out
Accumulation fp32 (PSUM), softmax/LN math fp32.

SBUF is managed as a single "slab" pool of ~2080B-per-partition slots; later
phases allocate tiles with the tags of dead earlier tensors, so the Tile
framework recycles addresses with automatic WAR dependencies.
"""

import math
import sys

sys.path.insert(0, "/opt/trn_rl_repo")

import numpy as np
import ml_dtypes

import concourse.bacc as bacc
import concourse.bass as bass
import concourse.tile as tile
from concourse import mybir
from concourse.bass_utils import run_bass_kernel_spmd
from concourse.masks import make_identity

BF16 = mybir.dt.bfloat16
FP8 = mybir.dt.float8e4
F32 = mybir.dt.float32
NP_BF16 = np.dtype(ml_dtypes.bfloat16)
NP_FP8 = np.dtype(ml_dtypes.float8_e4m3)

# Problem shape (hardcoded per spec)
B, S, D, H, HD, F = 16, 512, 1024, 16, 64, 4096,
N_CORES = 8
TB = B // N_CORES          # batches per core
T = TB * S                 # tokens per core

MM_N = 512                 # matmul moving free dim / PSUM bank width (fp32)
P = 128

WSHIFT = 7                 # weight pre-scale exponent (fp8 normal range)
WS = float(2.0 ** WSHIFT)
QSHIFT = 10                # w_q pre-scale (includes folded 1/sqrt(hd))
QS = float(2.0 ** QSHIFT)
EXP_SCALE = float(2.0 ** (-(WSHIFT + QSHIFT)))   # undo q*k weight scales
ATT_LN2 = 9                # attention weights scaled by 2^-9 via exp bias
ATT_BIAS = -ATT_LN2 * math.log(2.0)


def _ceil_div(a, b):
    return (a + b - 1) // b


def build_program(cfg):
    """Build the SPMD bass program. cfg is a dict of gate flags + sizes."""
    D_, F_, T_, TB_, H_, HD_ = (cfg["D"], cfg["F"], cfg["T"], cfg["TB"],
                                cfg["H"], cfg["HD"])
    S_ = T_ // TB_
    KD = D_ // P               # D partition chunks
    KD2 = KD // 2              # D chunk pairs (fp8 DoubleRow)
    KF = F_ // P               # F partition chunks
    TCH = T_ // P              # token chunks
    NT = _ceil_div(T_, MM_N)   # projection N tiles over tokens
    NTW = min(MM_N, T_)        # projection N tile width
    ND = _ceil_div(D_, MM_N)   # N tiles over D
    NDW = min(MM_N, D_)
    SKC = S_ // P              # key chunks per batch
    SKC2 = SKC // 2            # key chunk pairs
    HPC = P // HD_             # heads per 128-partition chunk
    VW = HD_ + 1               # V' per-head width (ones column appended)
    VROW = (D_ // HD_) * VW    # V' row width for one key chunk
    UPW = 1024                 # wup column tile width (per K-pair tile)
    UPT = F_ // UPW            # wup column tiles per D-pair chunk

    nc = bacc.Bacc("TRN2", target_bir_lowering=False, debug=False,
                   num_devices=N_CORES)

    # ---- DRAM I/O ----
    xT8_d = nc.dram_tensor("xT8", [D_, T_], FP8, kind="ExternalInput")
    x32_d = nc.dram_tensor("x32", [T_, D_], F32, kind="ExternalInput")
    wq_d = nc.dram_tensor("wq", [D_, D_], FP8, kind="ExternalInput")
    wk_d = nc.dram_tensor("wk", [D_, D_], FP8, kind="ExternalInput")
    wv_d = nc.dram_tensor("wv", [D_, D_], FP8, kind="ExternalInput")
    wo_d = nc.dram_tensor("wo", [D_, D_], FP8, kind="ExternalInput")
    wup_d = nc.dram_tensor("wup", [D_, F_], FP8, kind="ExternalInput")
    wdn_d = nc.dram_tensor("wdn", [F_, D_], BF16, kind="ExternalInput")
    # gated (only read when the host says they are non-trivial)
    bq_d = nc.dram_tensor("bq", [D_], F32, kind="ExternalInput")
    bk_d = nc.dram_tensor("bk", [D_], F32, kind="ExternalInput")
    bup_d = nc.dram_tensor("bup", [F_], F32, kind="ExternalInput")
    mask_d = nc.dram_tensor("maskT", [TB_, S_], F32, kind="ExternalInput")
    bv_d = nc.dram_tensor("bv", [D_], F32, kind="ExternalInput")
    bo_d = nc.dram_tensor("bo", [D_], F32, kind="ExternalInput")
    bdn_d = nc.dram_tensor("bdn", [D_], F32, kind="ExternalInput")
    g1_d = nc.dram_tensor("g1", [D_], F32, kind="ExternalInput")
    b1_d = nc.dram_tensor("b1", [D_], F32, kind="ExternalInput")
    g2_d = nc.dram_tensor("g2", [D_], F32, kind="ExternalInput")
    b2_d = nc.dram_tensor("b2", [D_], F32, kind="ExternalInput")
    out_d = nc.dram_tensor("out", [T_, D_], F32, kind="ExternalOutput")

    DR = mybir.MatmulPerfMode.DoubleRow

    with tile.TileContext(nc) as tc, \
         tc.tile_pool(name="consts", bufs=1) as consts, \
         tc.tile_pool(name="slab", bufs=1) as slab, \
         tc.tile_pool(name="ps", bufs=2, space="PSUM") as ps, \
         tc.tile_pool(name="psS", bufs=2, space="PSUM") as psS, \
         tc.tile_pool(name="psV", bufs=1, space="PSUM") as psV, \
         tc.tile_pool(name="psT", bufs=1, space="PSUM") as psT, \
         tc.tile_pool(name="work", bufs=2) as work, \
         tc.tile_pool(name="x32p", bufs=2) as x32p, \
         tc.tile_pool(name="attnp", bufs=10) as attnp, \
         tc.tile_pool(name="attn2", bufs=3) as attn2, \
         tc.tile_pool(name="statp", bufs=4) as statp, \
         tc.tile_pool(name="outp", bufs=2) as outp, \
         tc.tile_pool(name="dramp", bufs=6, space="DRAM") as dramp:

        dma = nc.sync
        dma2 = nc.gpsimd       # separate queue for small latency-bound DMAs

        def slot(tag, width, dtype):
            return slab.tile([P, width], dtype, tag=tag, name=f"t_{tag}")

        def pair(ap_2d, i2):
            # [P, 2*W] tile -> [P, 2, W] K-paired view
            return ap_2d.rearrange("p (i w) -> p i w", i=2) if i2 is None \
                else ap_2d.rearrange("p (i w) -> p i w", i=2)[:, :, i2]

        # ---- constants ----
        eps_t = consts.tile([P, 1], F32)
        nc.vector.memset(eps_t, 1e-5)
        attb_t = consts.tile([P, 1], F32)
        nc.vector.memset(attb_t, ATT_BIAS)
        ident = consts.tile([P, P], BF16)
        make_identity(nc, ident)
        # per-partition bias layouts: col m <- bias[m*128:(m+1)*128]
        if cfg["has_bq"]:
            bq_sb = consts.tile([P, KD], F32)
            dma.dma_start(out=bq_sb,
                          in_=bq_d.ap().rearrange("(m p) -> p m", p=P))
        if cfg["has_bk"]:
            bk_sb = consts.tile([P, KD], F32)
            dma.dma_start(out=bk_sb,
                          in_=bk_d.ap().rearrange("(m p) -> p m", p=P))
        if cfg["has_bup"]:
            bup_sb = consts.tile([P, KF], F32)
            dma.dma_start(out=bup_sb,
                          in_=bup_d.ap().rearrange("(m p) -> p m", p=P))
        # mask as per-partition bias: col (b*SKC+kc) <- mask[b, kc*128:+128]
        if cfg["has_mask"]:
            mask_sb = consts.tile([P, TB_ * SKC], F32)
            dma.dma_start(out=mask_sb,
                          in_=mask_d.ap().rearrange("b (kc p) -> p (b kc)",
                                                    p=P))
            mask2_sb = consts.tile([P, TB_ * SKC], F32)
            nc.vector.tensor_scalar_add(out=mask2_sb, in0=mask_sb,
                                        scalar1=ATT_BIAS)

        def bcast_row(dram_vec, n):
            # [n] fp32 -> [P, n] tile broadcast across partitions
            t = consts.tile([P, n], F32, name=f"bc_{dram_vec.name}")
            dma.dma_start(out=t,
                          in_=dram_vec.ap().unsqueeze(0).to_broadcast([P, n]))
            return t

        bv_bc = bcast_row(bv_d, D_) if cfg["has_bv"] else None
        bo_bc = bcast_row(bo_d, D_) if cfg["has_bo"] else None
        bdn_bc = bcast_row(bdn_d, D_) if cfg["has_bdn"] else None
        g1_bc = bcast_row(g1_d, D_) if cfg["has_n1"] else None
        b1_bc = bcast_row(b1_d, D_) if cfg["has_n1"] else None
        g2_bc = bcast_row(g2_d, D_) if cfg["has_n2"] else None
        b2_bc = bcast_row(b2_d, D_) if cfg["has_n2"] else None

        # ---- load x^T (fp8, K-paired) and QKV weights (fp8, K-paired) ----
        # paired layout: tile [128, 2*W]; cols [0:W) = chunk 2c, [W:2W) = 2c+1
        def paired_dma(dst, dram, c2, w):
            dma.dma_start(out=dst[:, 0:w],
                          in_=dram[2 * c2 * P:(2 * c2 + 1) * P, :])
            dma.dma_start(out=dst[:, w:2 * w],
                          in_=dram[(2 * c2 + 1) * P:(2 * c2 + 2) * P, :])

        xT8_sb = [slot(f"xT8{c2}", 2 * T_, FP8) for c2 in range(KD2)]
        w_sb = {nm: [slot(f"w{nm}{c2}", 2 * D_, FP8) for c2 in range(KD2)]
                for nm in ("q", "k", "v")}
        for c2 in range(KD2):
            paired_dma(w_sb["q"][c2], wq_d, c2, D_)
            dma.dma_start(out=xT8_sb[c2][:, 0:T_],
                          in_=xT8_d[2 * c2 * P:(2 * c2 + 1) * P, :])
            dma.dma_start(out=xT8_sb[c2][:, T_:2 * T_],
                          in_=xT8_d[(2 * c2 + 1) * P:(2 * c2 + 2) * P, :])
        for c2 in range(KD2):
            paired_dma(w_sb["k"][c2], wk_d, c2, D_)
        for c2 in range(KD2):
            paired_dma(w_sb["v"][c2], wv_d, c2, D_)

        qT_sb = [slot(f"qT{c}", T_, BF16) for c in range(KD)]
        # k^T is stored twice with the other head-half zeroed, so attention
        # scores can contract over the full 128 partitions (K=64 matmuls
        # de-rate the whole PE stream on this part)
        kTe_sb = [slot(f"kTe{c}", T_, BF16) for c in range(KD)]
        kTo_sb = [slot(f"kTo{c}", T_, BF16) for c in range(KD)]
        for c in range(KD):
            nc.vector.memset(kTe_sb[c][P // 2:P, :], 0.0)
            nc.vector.memset(kTo_sb[c][0:P // 2, :], 0.0)
        # V' fp8, key-chunk-paired: tile tr2 holds token chunks 2tr2, 2tr2+1
        Vp8_sb = [slot(f"Vp{c}", 2 * VROW, FP8) for c in range(TCH // 2)]

        # ---- QKV projections (fp8 DoubleRow) ----
        HB = P // 2
        for nm in ("q", "k"):
            has_b = cfg["has_bq"] if nm == "q" else cfg["has_bk"]
            bias = (bq_sb if nm == "q" else bk_sb) if has_b else None
            for m in range(KD):
                for t2 in range(NT):
                    pt = ps.tile([P, MM_N], F32, tag="ps", name="ps_qk")
                    for c2 in range(KD2):
                        nc.tensor.matmul(
                            pt[:, :NTW],
                            lhsT=pair(w_sb[nm][c2],
                                      slice(m * P, (m + 1) * P)),
                            rhs=pair(xT8_sb[c2],
                                     slice(t2 * MM_N, t2 * MM_N + NTW)),
                            start=(c2 == 0), stop=(c2 == KD2 - 1),
                            perf_mode=DR)
                    sl = slice(t2 * MM_N, t2 * MM_N + NTW)
                    if nm == "q":
                        if has_b:
                            nc.vector.tensor_scalar_add(
                                out=qT_sb[m][:, sl],
                                in0=pt[:, :NTW], scalar1=bias[:, m:m + 1])
                        else:
                            nc.vector.tensor_copy(out=qT_sb[m][:, sl],
                                                  in_=pt[:, :NTW])
                    else:
                        if has_b:
                            nc.vector.tensor_scalar_add(
                                out=kTe_sb[m][0:HB, sl],
                                in0=pt[0:HB, :NTW],
                                scalar1=bias[0:HB, m:m + 1])
                            nc.vector.tensor_scalar_add(
                                out=kTo_sb[m][HB:P, sl],
                                in0=pt[HB:P, :NTW],
                                scalar1=bias[HB:P, m:m + 1])
                        else:
                            nc.vector.tensor_copy(out=kTe_sb[m][0:HB, sl],
                                                  in_=pt[0:HB, :NTW])
                            nc.vector.tensor_copy(out=kTo_sb[m][HB:P, sl],
                                                  in_=pt[HB:P, :NTW])
        # V token-major, scattered into V' layout (ones col per head);
        # evict applies the 2^-7 weight-scale compensation.
        for tr in range(TCH):
            vdst = Vp8_sb[tr // 2][:, (tr % 2) * VROW:(tr % 2 + 1) * VROW]
            for n2 in range(ND):
                pt = ps.tile([P, MM_N], F32, tag="ps", name="ps_v")
                for c2 in range(KD2):
                    nc.tensor.matmul(
                        pt[:, :NDW],
                        lhsT=pair(xT8_sb[c2],
                                  slice(tr * P, (tr + 1) * P)),
                        rhs=pair(w_sb["v"][c2],
                                 slice(n2 * MM_N, n2 * MM_N + NDW)),
                        start=(c2 == 0), stop=(c2 == KD2 - 1),
                        perf_mode=DR)
                hpn = NDW // HD_  # heads per N tile
                dst = vdst.rearrange("p (h c) -> p h c", c=VW)
                src = pt[:, :NDW].rearrange("p (h c) -> p h c", c=HD_)
                if cfg["has_bv"]:
                    tmp = work.tile([P, NDW], F32, tag="vtmp", name="vtmp")
                    nc.scalar.mul(out=tmp, in_=pt[:, :NDW], mul=1.0 / WS)
                    nc.vector.tensor_add(
                        out=dst[:, n2 * hpn:(n2 + 1) * hpn, 0:HD_],
                        in0=tmp.rearrange("p (h c) -> p h c", c=HD_),
                        in1=bv_bc[:, n2 * MM_N:n2 * MM_N + NDW].rearrange(
                            "p (h c) -> p h c", c=HD_))
                else:
                    nc.scalar.mul(
                        out=dst[:, n2 * hpn:(n2 + 1) * hpn, 0:HD_],
                        in_=src, mul=1.0 / WS)
            nc.vector.memset(
                vdst.rearrange("p (h c) -> p h c", c=VW)[:, :, HD_:VW], 1.0)

        skip = cfg.get("skip", set())
        # ---- attention -> o^T feature-major (fp8) ----
        wo_sb = []
        for c2 in range(KD2):
            t = slot(f"xT8{c2}", 2 * D_, FP8)   # reuse xT8 slots
            paired_dma(t, wo_d, c2, D_)
            wo_sb.append(t)
        oT8_sb = [slot(f"wq{c2}", 2 * T_, FP8)  # reuse wq slots
                  for c2 in range(KD2)]

        if "attn" in skip:
            for c in range(KD):
                nc.vector.tensor_copy(
                    out=pair(oT8_sb[c // 2], slice(0, T_))
                    if False else oT8_sb[c // 2][:, (c % 2) * T_:
                                                 (c % 2 + 1) * T_],
                    in_=qT_sb[c])
        attn_iter = [] if "attn" in skip else [(b, h) for b in range(TB_)
                                               for h in range(H_)]
        for b, h in attn_iter:
            hc, ho = h // HPC, (h % HPC) * HD_
            # scores^T [k, q] per key chunk; exp via ScalarE with the
            # 2^-17 weight-scale compensation on the input and the mask
            # (+ -9ln2 output scale) as per-partition bias. Output fp8,
            # written into kc-paired tiles for the DoubleRow AV matmul.
            at_tiles = []
            for kc2 in range(SKC2):
                at = attnp.tile([P, 2 * S_], FP8, tag="attnT", name="attnT")
                at_tiles.append(at)
            kTm = kTe_sb if (h % HPC) == 0 else kTo_sb
            for kc in range(SKC):
                pt = psS.tile([P, MM_N], F32, tag="psS", name="ps_s")
                nc.tensor.matmul(
                    pt[:, :S_],
                    lhsT=kTm[hc][:, b * S_ + kc * P:b * S_ + (kc + 1) * P],
                    rhs=qT_sb[hc][:, b * S_:(b + 1) * S_],
                    start=True, stop=True)
                bias = (mask2_sb[:, b * SKC + kc:b * SKC + kc + 1]
                        if cfg["has_mask"] else attb_t)
                nc.scalar.activation(
                    out=at_tiles[kc // 2][:, (kc % 2) * S_:(kc % 2 + 1) * S_],
                    in_=pt[:, :S_],
                    func=mybir.ActivationFunctionType.Exp,
                    bias=bias, scale=EXP_SCALE)
            # o^T rows [HD] + denominator row, fp8 DoubleRow over kc pairs
            pv = psV.tile([P, MM_N], F32, tag=f"pv{(b * H_ + h) % 2}",
                          name="ps_v2")
            for kc2 in range(SKC2):
                nc.tensor.matmul(
                    pv[0:VW, :S_],
                    lhsT=pair(Vp8_sb[b * SKC2 + kc2],
                              slice(h * VW, (h + 1) * VW)),
                    rhs=pair(at_tiles[kc2], None),
                    start=(kc2 == 0), stop=(kc2 == SKC2 - 1),
                    perf_mode=DR)
            rs = attn2.tile([1, S_], F32, tag="rsum", name="rsum")
            nc.scalar.copy(out=rs, in_=pv[HD_:VW, :S_])
            rs_d = dramp.tile([1, S_], F32, tag="rs_d", name="rs_d")
            dma2.dma_start(out=rs_d, in_=rs)
            rb = attn2.tile([HD_, S_], F32, tag="rsbc", name="rsbc")
            dma2.dma_start(out=rb, in_=rs_d.to_broadcast([HD_, S_]))
            nc.vector.reciprocal_approx_fast(out=rb, in_=rb)
            nc.vector.tensor_mul(
                out=oT8_sb[hc // 2][ho:ho + HD_,
                                    (hc % 2) * T_ + b * S_:
                                    (hc % 2) * T_ + (b + 1) * S_],
                in0=pv[0:HD_, :S_], in1=rb)

        # ---- O proj (fp8 DR) + residual + LN1 -> x_medium + transpose ----
        # x32 is host-pre-scaled by 2^7 to match the wo weight scale; LN1 is
        # scale-invariant so downstream sees true-scale x_medium.
        xm_bf = [slot(f"qT{c}", D_, BF16) for c in range(TCH)]  # reuse qT
        xmT8_sb = [slot(f"wk{c2}", 2 * T_, FP8)                 # reuse wk
                   for c2 in range(KD2)]
        up_tags = [t for c in range(KD) for t in (f"kTe{c}", f"kTo{c}")]
        wup_sb = []
        for i in range(KD2 * UPT):
            t = slot(up_tags[i], 2 * UPW, FP8)
            c2, j = i // UPT, i % UPT
            dma.dma_start(
                out=t[:, 0:UPW],
                in_=wup_d[2 * c2 * P:(2 * c2 + 1) * P,
                          j * UPW:(j + 1) * UPW])
            dma.dma_start(
                out=t[:, UPW:2 * UPW],
                in_=wup_d[(2 * c2 + 1) * P:(2 * c2 + 2) * P,
                          j * UPW:(j + 1) * UPW])
            wup_sb.append(t)

        def wup_lhsT(c2, fm):
            i = c2 * UPT + (fm * P) // UPW
            o = (fm * P) % UPW
            return pair(wup_sb[i], slice(o, o + P))

        def layer_norm(src, dst, g_bc, b_bc):
            # src fp32 [P, D_]; dst [P, D_] (any dtype); per-partition stats
            bw = min(512, D_)
            nsub = _ceil_div(D_, bw)
            st = statp.tile([P, nsub, 6], F32, tag="bnst", name="bnst")
            for i in range(nsub):
                nc.vector.bn_stats(out=st[:, i, :],
                                   in_=src[:, i * bw:(i + 1) * bw])
            mv = statp.tile([P, 2], F32, tag="bnmv", name="bnmv")
            nc.vector.bn_aggr(out=mv, in_=st)
            rstd = statp.tile([P, 1], F32, tag="rstd", name="rstd")
            nc.scalar.activation(out=rstd, in_=mv[:, 1:2],
                                 func=mybir.ActivationFunctionType.Sqrt,
                                 bias=eps_t, scale=1.0)
            nc.vector.reciprocal(out=rstd, in_=rstd)
            if g_bc is None:
                nc.vector.tensor_scalar(
                    out=dst, in0=src, scalar1=mv[:, 0:1], scalar2=rstd,
                    op0=mybir.AluOpType.subtract, op1=mybir.AluOpType.mult)
            else:
                tmp = statp.tile([P, D_], F32, tag="lntmp", name="lntmp")
                nc.vector.tensor_scalar(
                    out=tmp, in0=src, scalar1=mv[:, 0:1], scalar2=rstd,
                    op0=mybir.AluOpType.subtract, op1=mybir.AluOpType.mult)
                nc.vector.tensor_mul(out=tmp, in0=tmp, in1=g_bc)
                nc.vector.tensor_add(out=dst, in0=tmp, in1=b_bc)

        for tr in range(TCH):
            xt = x32p.tile([P, D_], F32, tag="x32t", name="x32t")
            dma.dma_start(out=xt, in_=x32_d[tr * P:(tr + 1) * P, :])
            of = work.tile([P, D_], F32, tag="acc", name="of")
            for n2 in range(ND):
                pt = ps.tile([P, MM_N], F32, tag="ps", name="ps_o")
                for c2 in range(KD2):
                    nc.tensor.matmul(
                        pt[:, :NDW],
                        lhsT=pair(oT8_sb[c2], slice(tr * P, (tr + 1) * P)),
                        rhs=pair(wo_sb[c2],
                                 slice(n2 * MM_N, n2 * MM_N + NDW)),
                        start=(c2 == 0), stop=(c2 == KD2 - 1),
                        perf_mode=DR)
                nc.vector.tensor_add(out=of[:, n2 * MM_N:n2 * MM_N + NDW],
                                     in0=pt[:, :NDW],
                                     in1=xt[:, n2 * MM_N:n2 * MM_N + NDW])
                if cfg["has_bo"]:
                    nc.vector.tensor_add(
                        out=of[:, n2 * MM_N:n2 * MM_N + NDW],
                        in0=of[:, n2 * MM_N:n2 * MM_N + NDW],
                        in1=bo_bc[:, n2 * MM_N:n2 * MM_N + NDW])
            if "ln" in skip:
                nc.vector.tensor_copy(out=xm_bf[tr], in_=of)
            else:
                layer_norm(of, xm_bf[tr],
                           g1_bc if cfg["has_n1"] else None,
                           b1_bc if cfg["has_n1"] else None)
            # transpose x_medium chunk -> xmT8 (PE transpose via identity;
            # the fp8 cast happens in the ScalarE evict copy)
            for c in range(KD):
                if "tr" in skip:
                    nc.vector.tensor_copy(
                        out=xmT8_sb[c // 2][:, (c % 2) * T_ + tr * P:
                                            (c % 2) * T_ + (tr + 1) * P],
                        in_=xm_bf[tr][:, c * P:(c + 1) * P])
                else:
                    pt = psT.tile([P, P], BF16, tag=f"psT{c % 2}",
                                  name="ps_t")
                    nc.tensor.transpose(pt, xm_bf[tr][:, c * P:(c + 1) * P],
                                        ident)
                    nc.scalar.copy(
                        out=xmT8_sb[c // 2][:, (c % 2) * T_ + tr * P:
                                            (c % 2) * T_ + (tr + 1) * P],
                        in_=pt)

        # ---- FFN up (fp8 DR, F-major out) + Gelu (2^-7 comp) -> gT ----
        # gT must NOT share tags with wdn targets (deadlock): give it the
        # early-dead wv/Vp tags plus fresh slots.
        g_tags = ([f"g{c}" for c in range(KF - KD2 - TCH // 2)]
                  + [f"wv{c2}" for c2 in range(KD2)]
                  + [f"Vp{c}" for c in range(TCH // 2)])
        gT_sb = [slot(g_tags[c], T_, BF16) for c in range(KF)]
        for fm in range(KF):
            for t2 in range(NT):
                pt = ps.tile([P, MM_N], F32, tag="ps", name="ps_up")
                for c2 in range(KD2):
                    nc.tensor.matmul(
                        pt[:, :NTW],
                        lhsT=wup_lhsT(c2, fm),
                        rhs=pair(xmT8_sb[c2],
                                 slice(t2 * MM_N, t2 * MM_N + NTW)),
                        start=(c2 == 0), stop=(c2 == KD2 - 1),
                        perf_mode=DR)
                nc.scalar.activation(
                    out=gT_sb[fm][:, t2 * MM_N:t2 * MM_N + NTW],
                    in_=pt[:, :NTW],
                    func=mybir.ActivationFunctionType.Gelu,
                    bias=(bup_sb[:, fm:fm + 1] if cfg["has_bup"] else 0.0),
                    scale=1.0 / WS)

        # ---- FFN down (bf16) + residual + LN2 -> out ----
        # wdn reuses only tags dead by the down phase, earliest-dying first
        # so the weight DMA stream flows without long WAR stalls.
        dn_tags = ([f"wd{c}" for c in range(KF - len(up_tags) - 3 * KD2)]
                   + [f"xT8{c2}" for c2 in range(KD2)]
                   + [f"wq{c2}" for c2 in range(KD2)]
                   + up_tags
                   + [f"wk{c2}" for c2 in range(KD2)])
        wdn_sb = []
        for fc in range(KF):
            t = slot(dn_tags[fc], D_, BF16)
            dma.dma_start(out=t, in_=wdn_d[fc * P:(fc + 1) * P, :])
            wdn_sb.append(t)
        for tr in range(TCH):
            dsb = work.tile([P, D_], F32, tag="acc", name="dsb")
            for n2 in range(ND):
                pt = ps.tile([P, MM_N], F32, tag="ps", name="ps_dn")
                for fc in range(KF):
                    nc.tensor.matmul(
                        pt[:, :NDW],
                        lhsT=gT_sb[fc][:, tr * P:(tr + 1) * P],
                        rhs=wdn_sb[fc][:, n2 * MM_N:n2 * MM_N + NDW],
                        start=(fc == 0), stop=(fc == KF - 1))
                nc.vector.tensor_add(
                    out=dsb[:, n2 * MM_N:n2 * MM_N + NDW],
                    in0=pt[:, :NDW],
                    in1=xm_bf[tr][:, n2 * MM_N:n2 * MM_N + NDW])
                if cfg["has_bdn"]:
                    nc.vector.tensor_add(
                        out=dsb[:, n2 * MM_N:n2 * MM_N + NDW],
                        in0=dsb[:, n2 * MM_N:n2 * MM_N + NDW],
                        in1=bdn_bc[:, n2 * MM_N:n2 * MM_N + NDW])
            ot = outp.tile([P, D_], F32, tag="ot", name="ot")
            if "ln" in skip:
                nc.vector.tensor_copy(out=ot, in_=dsb)
            else:
                layer_norm(dsb, ot,
                           g2_bc if cfg["has_n2"] else None,
                           b2_bc if cfg["has_n2"] else None)
            dma.dma_start(out=out_d[tr * P:(tr + 1) * P, :], in_=ot)

    nc.finalize()
    return nc


_PROGRAM_CACHE = {}


def _get_program(cfg_key, cfg):
    if cfg_key not in _PROGRAM_CACHE:
        _PROGRAM_CACHE[cfg_key] = build_program(cfg)
    return _PROGRAM_CACHE[cfg_key]


def make_in_maps(inputs):
    """Host-side prep: LoRA merge, scale fold, dtype conversion, sharding."""
    f32 = np.float32
    x = np.asarray(inputs["x"], f32)
    scale = 1.0 / np.sqrt(float(inputs["head_dim"]))

    def merged(w, a, b):
        return (np.asarray(w, f32)
                + np.asarray(a, f32) @ np.asarray(b, f32))

    wq = (merged(inputs["w_q"], inputs["w_q_lora_a"], inputs["w_q_lora_b"])
          * (scale * QS)).astype(NP_FP8)
    wk = (merged(inputs["w_k"], inputs["w_k_lora_a"], inputs["w_k_lora_b"])
          * WS).astype(NP_FP8)
    wv = (merged(inputs["w_v"], inputs["w_v_lora_a"], inputs["w_v_lora_b"])
          * WS).astype(NP_FP8)
    wo = (merged(inputs["w_o"], inputs["w_o_lora_a"], inputs["w_o_lora_b"])
          * WS).astype(NP_FP8)
    wup = (merged(inputs["w_up"], inputs["w_up_lora_a"],
                  inputs["w_up_lora_b"]) * WS).astype(NP_FP8)
    wdn = merged(inputs["w_down"], inputs["w_down_lora_a"],
                 inputs["w_down_lora_b"]).astype(NP_BF16)
    mask = np.asarray(inputs["attention_mask"], f32)  # [B,1,1,S]

    common = {
        "wq": wq, "wk": wk, "wv": wv, "wo": wo, "wup": wup, "wdn": wdn,
        "bq": (np.asarray(inputs["b_q"], f32) * (scale * QS)).astype(f32),
        "bk": (np.asarray(inputs["b_k"], f32) * WS).astype(f32),
        "bup": np.asarray(inputs["b_up"], f32),
        "bv": np.asarray(inputs["b_v"], f32),
        "bo": np.asarray(inputs["b_o"], f32),
        "bdn": np.asarray(inputs["b_down"], f32),
        "g1": np.asarray(inputs["norm_weight_1"], f32),
        "b1": np.asarray(inputs["norm_bias_1"], f32),
        "g2": np.asarray(inputs["norm_weight_2"], f32),
        "b2": np.asarray(inputs["norm_bias_2"], f32),
    }
    in_maps = []
    for i in range(N_CORES):
        xc = x[i * TB:(i + 1) * TB].reshape(T, D)
        m = dict(common)
        m["xT8"] = np.ascontiguousarray(xc.T).astype(NP_FP8)
        m["x32"] = np.ascontiguousarray(xc) * WS   # LN1 scale-invariance
        m["maskT"] = np.ascontiguousarray(mask[i * TB:(i + 1) * TB, 0, 0, :])
        in_maps.append(m)
    return in_maps


def full_cfg(inputs):
    f32 = np.float32
    return {
        "D": D, "F": F, "T": T, "TB": TB, "H": H, "HD": HD,
        "has_bq": bool(np.any(np.asarray(inputs["b_q"], f32))),
        "has_bk": bool(np.any(np.asarray(inputs["b_k"], f32))),
        "has_bup": bool(np.any(np.asarray(inputs["b_up"], f32))),
        "has_mask": bool(np.any(np.asarray(inputs["attention_mask"], f32))),
        "has_bv": bool(np.any(np.asarray(inputs["b_v"], f32))),
        "has_bo": bool(np.any(np.asarray(inputs["b_o"], f32))),
        "has_bdn": bool(np.any(np.asarray(inputs["b_down"], f32))),
        "has_n1": bool(np.any(np.asarray(inputs["norm_weight_1"], f32) != 1.0)
                       or np.any(np.asarray(inputs["norm_bias_1"], f32))),
        "has_n2": bool(np.any(np.asarray(inputs["norm_weight_2"], f32) != 1.0)
                       or np.any(np.asarray(inputs["norm_bias_2"], f32))),
    }


def run_on_hw(inputs, trace=False, tmpdir=None):
    cfg = full_cfg(inputs)
    cfg_key = tuple(sorted((k, v) for k, v in cfg.items()
                           if not isinstance(v, set)))
    nc = _get_program(cfg_key, cfg)
    in_maps = make_in_maps(inputs)
    kw = {}
    if trace:
        kw = {"trace": True, "tmpdir": tmpdir}
    res = run_bass_kernel_spmd(nc, in_maps, core_ids=list(range(N_CORES)),
                               **kw)
    out = np.empty((B, S, D), np.float32)
    for i in range(N_CORES):
        out[i * TB:(i + 1) * TB] = res.results[i]["out"].reshape(TB, S, D)
    return out, res


def kernel(**inputs):
    out, _ = run_on_hw(inputs)
    return out


# revision 12
# speedup vs baseline: 1.3409x; 1.0882x over previous
"""Fused RoBERTa layer (attention + FFN, LoRA merged) on 8 Trainium2 cores.

Sharding: pure data-parallel over batch (16 batches -> 2 per core), no
collectives. LoRA merged into base weights on host; 1/sqrt(hd) folded into
w_q.

fp8 strategy (2x PE throughput via DoubleRow double-pumping):
  - QKV / AV / O-proj / FFN-up matmuls run fp8e4m3 with
    MatmulPerfMode.DoubleRow: both operands hold TWO 128-deep K-chunks side
    by side in the free dim ([128, 2, M]), contracting 256 per pass.
  - FFN-down stays bf16 (fp8 there breaks the accuracy gate).
  - Weights pre-scaled by 2^7 (2^10 for w_q) into fp8 normal range; inverse
    scales folded into exp input scale (2^-17), gelu input scale (2^-7),
    V-evict scale (2^-7), and a host 2^7 pre-scale of the bf16 residual x
    (LN1 is scale-invariant).
  - exp folds a 2^-9 output scale via its bias so unnormalized fp8
    attention weights stay in range; the ones-column denominator is the sum
    of the SAME fp8 weights so the scale cancels exactly.

Attention normalization is BATCHED: the AV matmul emits unnormalized o
rows plus a denominator row per (b,h) (V' ones-column; for odd heads the
ones column comes FIRST and the AV output is written at partition offset
63 so o rows land on partitions 64..127 -- this keeps every evict
same-partition and vector-legal). Unnormalized o is evicted to bf16 oTu;
denominators collect into a [H, T] tile; per batch one reciprocal + one
DMA out + 8 broadcast DMAs back + 8 vector muls produce fp8 oT. The
post-pass of batch b is interleaved into batch b+1's attention (or the
O-projection) so its DMA latency is hidden.

PSUM: one 8-bank pool with manual tags: mm0/mm1 (projection/FFN double
buffer), sc0..3 (scores 4-deep, reused by the LN1 transposes), pv0/pv1
(AV). Host pre-swizzles all fp8 tensors into per-tile [128, W] contiguous
layout so every weight DMA is linear in DRAM.
"""

import math
import sys

sys.path.insert(0, "/opt/trn_rl_repo")

import numpy as np
import ml_dtypes

import concourse.bacc as bacc
import concourse.bass as bass
import concourse.tile as tile
from concourse import mybir
from concourse.bass_utils import run_bass_kernel_spmd
from concourse.masks import make_identity

BF16 = mybir.dt.bfloat16
FP8 = mybir.dt.float8e4
F32 = mybir.dt.float32
NP_BF16 = np.dtype(ml_dtypes.bfloat16)
NP_FP8 = np.dtype(ml_dtypes.float8_e4m3)

B, S, D, H, HD, F = 16, 512, 1024, 16, 64, 4096
N_CORES = 8
TB = B // N_CORES
T = TB * S

MM_N = 512
P = 128

WSHIFT = 7
WS = float(2.0 ** WSHIFT)
QSHIFT = 10
QS = float(2.0 ** QSHIFT)
EXP_SCALE = float(2.0 ** (-(WSHIFT + QSHIFT)))
ATT_BIAS = -9 * math.log(2.0)


def _ceil_div(a, b):
    return (a + b - 1) // b


def build_program(cfg):
    D_, F_, T_, TB_, H_, HD_ = (cfg["D"], cfg["F"], cfg["T"], cfg["TB"],
                                cfg["H"], cfg["HD"])
    S_ = T_ // TB_
    KD = D_ // P
    KD2 = KD // 2
    KF = F_ // P
    TCH = T_ // P
    NT = _ceil_div(T_, MM_N)
    NTW = min(MM_N, T_)
    ND = _ceil_div(D_, MM_N)
    NDW = min(MM_N, D_)
    SKC = S_ // P
    SKC2 = SKC // 2
    HPC = P // HD_             # heads per 128-partition chunk (=2)
    VW = HD_ + 1               # V' per-head width (ones column)
    VROW = H_ * VW             # V' row width for one key chunk
    UPW = 1024
    UPT = F_ // UPW

    nc = bacc.Bacc("TRN2", target_bir_lowering=False, debug=False,
                   num_devices=N_CORES)

    # ---- DRAM I/O (fp8 tensors pre-swizzled on host: [ntile, 128, W]) ----
    xT8_d = nc.dram_tensor("xT8", [KD2, P, 2 * T_], FP8,
                           kind="ExternalInput")
    xr_d = nc.dram_tensor("xr", [T_, D_], BF16, kind="ExternalInput")
    wq_d = nc.dram_tensor("wq", [KD2, P, 2 * D_], FP8, kind="ExternalInput")
    wk_d = nc.dram_tensor("wk", [KD2, P, 2 * D_], FP8, kind="ExternalInput")
    wv_d = nc.dram_tensor("wv", [KD2, P, 2 * D_], FP8, kind="ExternalInput")
    wo_d = nc.dram_tensor("wo", [KD2, P, 2 * D_], FP8, kind="ExternalInput")
    wup_d = nc.dram_tensor("wup", [KD2 * UPT, P, 2 * UPW], FP8,
                           kind="ExternalInput")
    wdn_d = nc.dram_tensor("wdn", [F_, D_], BF16, kind="ExternalInput")
    bq_d = nc.dram_tensor("bq", [D_], F32, kind="ExternalInput")
    bk_d = nc.dram_tensor("bk", [D_], F32, kind="ExternalInput")
    bup_d = nc.dram_tensor("bup", [F_], F32, kind="ExternalInput")
    mask_d = nc.dram_tensor("maskT", [TB_, S_], F32, kind="ExternalInput")
    bv_d = nc.dram_tensor("bv", [D_], F32, kind="ExternalInput")
    bo_d = nc.dram_tensor("bo", [D_], F32, kind="ExternalInput")
    bdn_d = nc.dram_tensor("bdn", [D_], F32, kind="ExternalInput")
    g1_d = nc.dram_tensor("g1", [D_], F32, kind="ExternalInput")
    b1_d = nc.dram_tensor("b1", [D_], F32, kind="ExternalInput")
    g2_d = nc.dram_tensor("g2", [D_], F32, kind="ExternalInput")
    b2_d = nc.dram_tensor("b2", [D_], F32, kind="ExternalInput")
    out_d = nc.dram_tensor("out", [T_, D_], F32, kind="ExternalOutput")

    DR = mybir.MatmulPerfMode.DoubleRow

    with tile.TileContext(nc) as tc, \
         tc.tile_pool(name="consts", bufs=1) as consts, \
         tc.tile_pool(name="slab", bufs=1) as slab, \
         tc.tile_pool(name="pall", bufs=1, space="PSUM") as pall, \
         tc.tile_pool(name="work", bufs=2) as work, \
         tc.tile_pool(name="xrp", bufs=2) as xrp, \
         tc.tile_pool(name="attnp", bufs=6) as attnp, \
         tc.tile_pool(name="attn2", bufs=4) as attn2, \
         tc.tile_pool(name="rbp", bufs=1) as rbp, \
         tc.tile_pool(name="statp", bufs=4) as statp, \
         tc.tile_pool(name="outp", bufs=2) as outp, \
         tc.tile_pool(name="dramp", bufs=2, space="DRAM") as dramp:

        dma = nc.sync          # bulk loads
        dma2 = nc.gpsimd       # latency-bound small DMAs + output

        def slot(tag, width, dtype):
            return slab.tile([P, width], dtype, tag=tag, name=f"t_{tag}")

        def pair(ap_2d, i2):
            return ap_2d.rearrange("p (i w) -> p i w", i=2) if i2 is None \
                else ap_2d.rearrange("p (i w) -> p i w", i=2)[:, :, i2]

        mm_ctr = [0]

        def mm_tile():
            mm_ctr[0] ^= 1
            return pall.tile([P, MM_N], F32, tag=f"mm{mm_ctr[0]}",
                             name="mm")

        # ---- constants ----
        eps_t = consts.tile([P, 1], F32)
        nc.vector.memset(eps_t, 1e-5)
        attb_t = consts.tile([P, 1], F32)
        nc.vector.memset(attb_t, ATT_BIAS)
        zero_t = consts.tile([P, 1], F32)
        nc.vector.memset(zero_t, 0.0)
        ident = consts.tile([P, P], BF16)
        make_identity(nc, ident)
        if cfg["has_bq"]:
            bq_sb = consts.tile([P, KD], F32)
            dma.dma_start(out=bq_sb,
                          in_=bq_d.ap().rearrange("(m p) -> p m", p=P))
        if cfg["has_bk"]:
            bk_sb = consts.tile([P, KD], F32)
            dma.dma_start(out=bk_sb,
                          in_=bk_d.ap().rearrange("(m p) -> p m", p=P))
        if cfg["has_bup"]:
            bup_sb = consts.tile([P, KF], F32)
            dma.dma_start(out=bup_sb,
                          in_=bup_d.ap().rearrange("(m p) -> p m", p=P))
        if cfg["has_mask"]:
            mask_sb = consts.tile([P, TB_ * SKC], F32)
            dma.dma_start(out=mask_sb,
                          in_=mask_d.ap().rearrange("b (kc p) -> p (b kc)",
                                                    p=P))
            mask2_sb = consts.tile([P, TB_ * SKC], F32)
            nc.vector.tensor_scalar_add(out=mask2_sb, in0=mask_sb,
                                        scalar1=ATT_BIAS)

        def bcast_row(dram_vec, n):
            t = consts.tile([P, n], F32, name=f"bc_{dram_vec.name}")
            dma.dma_start(out=t,
                          in_=dram_vec.ap().unsqueeze(0).to_broadcast([P, n]))
            return t

        bv_bc = bcast_row(bv_d, D_) if cfg["has_bv"] else None
        bo_bc = bcast_row(bo_d, D_) if cfg["has_bo"] else None
        bdn_bc = bcast_row(bdn_d, D_) if cfg["has_bdn"] else None
        g1_bc = bcast_row(g1_d, D_) if cfg["has_n1"] else None
        b1_bc = bcast_row(b1_d, D_) if cfg["has_n1"] else None
        g2_bc = bcast_row(g2_d, D_) if cfg["has_n2"] else None
        b2_bc = bcast_row(b2_d, D_) if cfg["has_n2"] else None

        # ---- load x^T and QKV weights (fp8, K-paired, linear DMAs) ----
        xT8_sb = [slot(f"xT8{c2}", 2 * T_, FP8) for c2 in range(KD2)]
        w_sb = {nm: [slot(f"w{nm}{c2}", 2 * D_, FP8) for c2 in range(KD2)]
                for nm in ("q", "k", "v")}
        for c2 in range(KD2):
            dma.dma_start(out=w_sb["q"][c2], in_=wq_d[c2])
            dma.dma_start(out=xT8_sb[c2], in_=xT8_d[c2])
        for c2 in range(KD2):
            dma.dma_start(out=w_sb["k"][c2], in_=wk_d[c2])
        for c2 in range(KD2):
            dma.dma_start(out=w_sb["v"][c2], in_=wv_d[c2])

        qT_sb = [slot(f"qT{c}", T_, BF16) for c in range(KD)]
        kTe_sb = [slot(f"kTe{c}", T_, BF16) for c in range(KD)]
        kTo_sb = [slot(f"kTo{c}", T_, BF16) for c in range(KD)]
        for c in range(KD):
            nc.vector.memset(kTe_sb[c][P // 2:P, :], 0.0)
            nc.vector.memset(kTo_sb[c][0:P // 2, :], 0.0)
        Vp8_sb = [slot(f"Vp{c}", 2 * VROW, FP8) for c in range(TCH // 2)]

        # ---- QKV projections (fp8 DoubleRow) ----
        HB = P // 2
        for nm in ("q", "k"):
            has_b = cfg["has_bq"] if nm == "q" else cfg["has_bk"]
            bias = (bq_sb if nm == "q" else bk_sb) if has_b else None
            for m in range(KD):
                for t2 in range(NT):
                    pt = mm_tile()
                    for c2 in range(KD2):
                        nc.tensor.matmul(
                            pt[:, :NTW],
                            lhsT=pair(w_sb[nm][c2],
                                      slice(m * P, (m + 1) * P)),
                            rhs=pair(xT8_sb[c2],
                                     slice(t2 * MM_N, t2 * MM_N + NTW)),
                            start=(c2 == 0), stop=(c2 == KD2 - 1),
                            perf_mode=DR)
                    sl = slice(t2 * MM_N, t2 * MM_N + NTW)
                    if nm == "q":
                        if has_b:
                            nc.vector.tensor_scalar_add(
                                out=qT_sb[m][:, sl],
                                in0=pt[:, :NTW], scalar1=bias[:, m:m + 1])
                        else:
                            nc.vector.tensor_copy(out=qT_sb[m][:, sl],
                                                  in_=pt[:, :NTW])
                    else:
                        if has_b:
                            nc.vector.tensor_scalar_add(
                                out=kTe_sb[m][0:HB, sl],
                                in0=pt[0:HB, :NTW],
                                scalar1=bias[0:HB, m:m + 1])
                            nc.vector.tensor_scalar_add(
                                out=kTo_sb[m][HB:P, sl],
                                in0=pt[HB:P, :NTW],
                                scalar1=bias[HB:P, m:m + 1])
                        else:
                            nc.vector.tensor_copy(out=kTe_sb[m][0:HB, sl],
                                                  in_=pt[0:HB, :NTW])
                            nc.vector.tensor_copy(out=kTo_sb[m][HB:P, sl],
                                                  in_=pt[HB:P, :NTW])
        # V token-major into V' ([v(64), 1] per head; 2^-7 scale on evict)
        for tr in range(TCH):
            vdst = Vp8_sb[tr // 2][:, (tr % 2) * VROW:(tr % 2 + 1) * VROW]
            vd3 = vdst.rearrange("p (h c) -> p h c", c=VW)
            for n2 in range(ND):
                pt = mm_tile()
                for c2 in range(KD2):
                    nc.tensor.matmul(
                        pt[:, :NDW],
                        lhsT=pair(xT8_sb[c2], slice(tr * P, (tr + 1) * P)),
                        rhs=pair(w_sb["v"][c2],
                                 slice(n2 * MM_N, n2 * MM_N + NDW)),
                        start=(c2 == 0), stop=(c2 == KD2 - 1),
                        perf_mode=DR)
                hpn = NDW // HD_   # heads per N tile
                src = pt[:, :NDW].rearrange("p (h c) -> p h c", c=HD_)
                if cfg["has_bv"]:
                    tmp = work.tile([P, NDW], F32, tag="vtmp", name="vtmp")
                    nc.vector.tensor_add(
                        out=tmp, in0=pt[:, :NDW],
                        in1=bv_bc[:, n2 * MM_N:n2 * MM_N + NDW])
                    src = tmp.rearrange("p (h c) -> p h c", c=HD_)
                nc.vector.tensor_scalar_mul(
                    out=vd3[:, n2 * hpn:(n2 + 1) * hpn, 0:HD_], in0=src,
                    scalar1=1.0 / WS)
            nc.vector.memset(vd3[:, :, HD_:VW], 1.0)  # ones cols

        skip = cfg.get("skip", set())
        # ---- attention ----
        wo_sb = []
        for c2 in range(KD2):
            t = slot(f"xT8{c2}", 2 * D_, FP8)
            dma.dma_start(out=t, in_=wo_d[c2])
            wo_sb.append(t)
        oT8_sb = [slot(f"wq{c2}", 2 * T_, FP8) for c2 in range(KD2)]
        oTu_sb = [slot(f"wv{hc}" if hc < KD2 else f"oTu{hc - KD2}",
                       T_, BF16) for hc in range(KD)]
        den_d = dramp.tile([H_, T_], F32, tag="den_d", name="den_d")
        rb_sb = {}

        def attn_bh(b, h):
            hc, par = h // HPC, h % HPC
            at_tiles = [attnp.tile([P, 2 * S_], FP8, tag="attnT",
                                   name="attnT") for _ in range(SKC2)]
            kTm = kTe_sb if par == 0 else kTo_sb
            for kc in range(SKC):
                pt = pall.tile([P, MM_N], F32, tag=f"sc{kc}", name="ps_s")
                nc.tensor.matmul(
                    pt[:, :S_],
                    lhsT=kTm[hc][:, b * S_ + kc * P:b * S_ + (kc + 1) * P],
                    rhs=qT_sb[hc][:, b * S_:(b + 1) * S_],
                    start=True, stop=True)
                bias = (mask2_sb[:, b * SKC + kc:b * SKC + kc + 1]
                        if cfg["has_mask"] else attb_t)
                nc.scalar.activation(
                    out=at_tiles[kc // 2][:, (kc % 2) * S_:(kc % 2 + 1) * S_],
                    in_=pt[:, :S_],
                    func=mybir.ActivationFunctionType.Exp,
                    bias=bias, scale=EXP_SCALE)
            pv = pall.tile([P, MM_N], F32, tag=f"pv{h % 2}", name="ps_v2")
            for kc2 in range(SKC2):
                nc.tensor.matmul(
                    pv[0:VW, :S_],
                    lhsT=pair(Vp8_sb[b * SKC2 + kc2],
                              slice(h * VW, (h + 1) * VW)),
                    rhs=pair(at_tiles[kc2], None),
                    start=(kc2 == 0), stop=(kc2 == SKC2 - 1),
                    perf_mode=DR)
            ho = par * HD_
            nc.vector.tensor_copy(
                out=oTu_sb[hc][ho:ho + HD_, b * S_:(b + 1) * S_],
                in_=pv[0:HD_, :S_])
            rs = attn2.tile([1, S_], F32, tag="rs", name="rs")
            nc.scalar.copy(out=rs, in_=pv[HD_:VW, :S_])
            dma2.dma_start(out=den_d[h:h + 1, b * S_:(b + 1) * S_], in_=rs)

        def post_a(b):
            # broadcast denominators back + reciprocal, per feature chunk
            sl = slice(b * S_, (b + 1) * S_)
            for hc in range(KD):
                rb = rbp.tile([P, S_], F32, tag=f"rb{hc}", name="rb")
                rb_sb[hc] = rb
                for h2 in range(HPC):
                    dma2.dma_start(
                        out=rb[h2 * HD_:(h2 + 1) * HD_, :],
                        in_=den_d[2 * hc + h2:2 * hc + h2 + 1, sl]
                        .to_broadcast([HD_, S_]))
                nc.vector.reciprocal_approx_fast(out=rb, in_=rb)

        def post_b(b):
            # normalize: oT8 = oTu * rb (same partitions, vector-legal)
            sl = slice(b * S_, (b + 1) * S_)
            for hc in range(KD):
                nc.vector.tensor_mul(
                    out=oT8_sb[hc // 2][:, (hc % 2) * T_ + b * S_:
                                        (hc % 2) * T_ + (b + 1) * S_],
                    in0=oTu_sb[hc][:, sl], in1=rb_sb[hc])

        if "attn" in skip:
            for c in range(KD):
                nc.vector.tensor_copy(
                    out=oT8_sb[c // 2][:, (c % 2) * T_:(c % 2 + 1) * T_],
                    in_=qT_sb[c])
        else:
            for h in range(H_):
                attn_bh(0, h)
            post_a(0)
            for h in range(H_):
                attn_bh(1, h)
                if h == 3:
                    post_b(0)
            post_a(1)

        # ---- O proj (fp8 DR) + residual + LN1 -> x_medium + transpose ----
        xm_bf = [slot(f"qT{c}", D_, BF16) for c in range(TCH)]
        xmT8_sb = [slot(f"wk{c2}", 2 * T_, FP8) for c2 in range(KD2)]
        up_tags = [t for c in range(KD) for t in (f"kTe{c}", f"kTo{c}")]
        wup_sb = []
        for i in range(KD2 * UPT):
            t = slot(up_tags[i], 2 * UPW, FP8)
            dma.dma_start(out=t, in_=wup_d[i])
            wup_sb.append(t)

        def wup_lhsT(c2, fm):
            i = c2 * UPT + (fm * P) // UPW
            o = (fm * P) % UPW
            return pair(wup_sb[i], slice(o, o + P))

        def layer_norm(src, dst, g_bc, b_bc):
            bw = min(512, D_)
            nsub = _ceil_div(D_, bw)
            st = statp.tile([P, nsub, 6], F32, tag="bnst", name="bnst")
            for i in range(nsub):
                nc.vector.bn_stats(out=st[:, i, :],
                                   in_=src[:, i * bw:(i + 1) * bw])
            mv = statp.tile([P, 2], F32, tag="bnmv", name="bnmv")
            nc.vector.bn_aggr(out=mv, in_=st)
            rstd = statp.tile([P, 1], F32, tag="rstd", name="rstd")
            nc.scalar.activation(out=rstd, in_=mv[:, 1:2],
                                 func=mybir.ActivationFunctionType.Sqrt,
                                 bias=eps_t, scale=1.0)
            nc.vector.reciprocal(out=rstd, in_=rstd)
            if g_bc is None:
                nc.vector.tensor_scalar(
                    out=dst, in0=src, scalar1=mv[:, 0:1], scalar2=rstd,
                    op0=mybir.AluOpType.subtract, op1=mybir.AluOpType.mult)
            else:
                tmp = statp.tile([P, D_], F32, tag="lntmp", name="lntmp")
                nc.vector.tensor_scalar(
                    out=tmp, in0=src, scalar1=mv[:, 0:1], scalar2=rstd,
                    op0=mybir.AluOpType.subtract, op1=mybir.AluOpType.mult)
                nc.vector.tensor_mul(out=tmp, in0=tmp, in1=g_bc)
                nc.vector.tensor_add(out=dst, in0=tmp, in1=b_bc)

        for tr in range(TCH):
            xt = xrp.tile([P, D_], BF16, tag="xrt", name="xrt")
            dma.dma_start(out=xt, in_=xr_d[tr * P:(tr + 1) * P, :])
            of = work.tile([P, D_], F32, tag="acc", name="of")
            for n2 in range(ND):
                pt = mm_tile()
                for c2 in range(KD2):
                    nc.tensor.matmul(
                        pt[:, :NDW],
                        lhsT=pair(oT8_sb[c2], slice(tr * P, (tr + 1) * P)),
                        rhs=pair(wo_sb[c2],
                                 slice(n2 * MM_N, n2 * MM_N + NDW)),
                        start=(c2 == 0), stop=(c2 == KD2 - 1),
                        perf_mode=DR)
                nc.vector.tensor_add(out=of[:, n2 * MM_N:n2 * MM_N + NDW],
                                     in0=pt[:, :NDW],
                                     in1=xt[:, n2 * MM_N:n2 * MM_N + NDW])
                if cfg["has_bo"]:
                    nc.vector.tensor_add(
                        out=of[:, n2 * MM_N:n2 * MM_N + NDW],
                        in0=of[:, n2 * MM_N:n2 * MM_N + NDW],
                        in1=bo_bc[:, n2 * MM_N:n2 * MM_N + NDW])
            if "ln" in skip:
                nc.vector.tensor_copy(out=xm_bf[tr], in_=of)
            else:
                layer_norm(of, xm_bf[tr],
                           g1_bc if cfg["has_n1"] else None,
                           b1_bc if cfg["has_n1"] else None)
            for c in range(KD):
                if "tr" in skip:
                    nc.vector.tensor_copy(
                        out=xmT8_sb[c // 2][:, (c % 2) * T_ + tr * P:
                                            (c % 2) * T_ + (tr + 1) * P],
                        in_=xm_bf[tr][:, c * P:(c + 1) * P])
                else:
                    pt = pall.tile([P, P], BF16, tag=f"sc{c % 4}",
                                   name="ps_t")
                    nc.tensor.transpose(pt, xm_bf[tr][:, c * P:(c + 1) * P],
                                        ident)
                    nc.vector.tensor_copy(
                        out=xmT8_sb[c // 2][:, (c % 2) * T_ + tr * P:
                                            (c % 2) * T_ + (tr + 1) * P],
                        in_=pt)
            if tr == 1 and "attn" not in skip:
                post_b(1)

        # ---- FFN up (fp8 DR) + Gelu -> gT ----
        g_tags = ([f"g{c}" for c in range(KF - KD)]
                  + [f"wv{c2}" for c2 in range(KD2)]
                  + [f"oTu{c2}" for c2 in range(KD2)])
        gT_sb = [slot(g_tags[c], T_, BF16) for c in range(KF)]
        for fm in range(KF):
            for t2 in range(NT):
                pt = mm_tile()
                for c2 in range(KD2):
                    nc.tensor.matmul(
                        pt[:, :NTW],
                        lhsT=wup_lhsT(c2, fm),
                        rhs=pair(xmT8_sb[c2],
                                 slice(t2 * MM_N, t2 * MM_N + NTW)),
                        start=(c2 == 0), stop=(c2 == KD2 - 1),
                        perf_mode=DR)
                nc.scalar.activation(
                    out=gT_sb[fm][:, t2 * MM_N:t2 * MM_N + NTW],
                    in_=pt[:, :NTW],
                    func=mybir.ActivationFunctionType.Gelu,
                    bias=(bup_sb[:, fm:fm + 1] if cfg["has_bup"]
                          else zero_t),
                    scale=1.0 / WS)

        # ---- FFN down (bf16) + residual + LN2 -> out ----
        dn_tags = ([f"Vp{c}" for c in range(TCH // 2)]
                   + [f"xT8{c2}" for c2 in range(KD2)]
                   + [f"wq{c2}" for c2 in range(KD2)]
                   + up_tags
                   + [f"wk{c2}" for c2 in range(KD2)])
        wdn_sb = []
        for fc in range(KF):
            t = slot(dn_tags[fc], D_, BF16)
            dma.dma_start(out=t, in_=wdn_d[fc * P:(fc + 1) * P, :])
            wdn_sb.append(t)
        for tr in range(TCH):
            dsb = work.tile([P, D_], F32, tag="acc", name="dsb")
            for n2 in range(ND):
                pt = mm_tile()
                for fc in range(KF):
                    nc.tensor.matmul(
                        pt[:, :NDW],
                        lhsT=gT_sb[fc][:, tr * P:(tr + 1) * P],
                        rhs=wdn_sb[fc][:, n2 * MM_N:n2 * MM_N + NDW],
                        start=(fc == 0), stop=(fc == KF - 1))
                nc.vector.tensor_add(
                    out=dsb[:, n2 * MM_N:n2 * MM_N + NDW],
                    in0=pt[:, :NDW],
                    in1=xm_bf[tr][:, n2 * MM_N:n2 * MM_N + NDW])
                if cfg["has_bdn"]:
                    nc.vector.tensor_add(
                        out=dsb[:, n2 * MM_N:n2 * MM_N + NDW],
                        in0=dsb[:, n2 * MM_N:n2 * MM_N + NDW],
                        in1=bdn_bc[:, n2 * MM_N:n2 * MM_N + NDW])
            ot = outp.tile([P, D_], F32, tag="ot", name="ot")
            if "ln" in skip:
                nc.vector.tensor_copy(out=ot, in_=dsb)
            else:
                layer_norm(dsb, ot,
                           g2_bc if cfg["has_n2"] else None,
                           b2_bc if cfg["has_n2"] else None)
            dma2.dma_start(out=out_d[tr * P:(tr + 1) * P, :], in_=ot)

    nc.finalize()
    return nc


_PROGRAM_CACHE = {}


def _get_program(cfg_key, cfg):
    if cfg_key not in _PROGRAM_CACHE:
        _PROGRAM_CACHE[cfg_key] = build_program(cfg)
    return _PROGRAM_CACHE[cfg_key]


def _swz(w, npairs, width):
    """[rows, cols] -> [npairs, 128, 2*cols] K-paired contiguous."""
    return np.ascontiguousarray(
        w.reshape(npairs, 2, P, width).transpose(0, 2, 1, 3)
        .reshape(npairs, P, 2 * width))


def make_in_maps(inputs):
    f32 = np.float32
    x = np.asarray(inputs["x"], f32)
    scale = 1.0 / np.sqrt(float(inputs["head_dim"]))

    def merged(w, a, b):
        return (np.asarray(w, f32)
                + np.asarray(a, f32) @ np.asarray(b, f32))

    KD2 = D // P // 2
    wq = _swz((merged(inputs["w_q"], inputs["w_q_lora_a"],
                      inputs["w_q_lora_b"]) * (scale * QS)).astype(NP_FP8),
              KD2, D)
    wk = _swz((merged(inputs["w_k"], inputs["w_k_lora_a"],
                      inputs["w_k_lora_b"]) * WS).astype(NP_FP8), KD2, D)
    wv = _swz((merged(inputs["w_v"], inputs["w_v_lora_a"],
                      inputs["w_v_lora_b"]) * WS).astype(NP_FP8), KD2, D)
    wo = _swz((merged(inputs["w_o"], inputs["w_o_lora_a"],
                      inputs["w_o_lora_b"]) * WS).astype(NP_FP8), KD2, D)
    wup8 = (merged(inputs["w_up"], inputs["w_up_lora_a"],
                   inputs["w_up_lora_b"]) * WS).astype(NP_FP8)
    UPW = 1024
    UPT = F // UPW
    wup = np.ascontiguousarray(
        wup8.reshape(KD2, 2, P, UPT, UPW).transpose(0, 3, 2, 1, 4)
        .reshape(KD2 * UPT, P, 2 * UPW))
    wdn = merged(inputs["w_down"], inputs["w_down_lora_a"],
                 inputs["w_down_lora_b"]).astype(NP_BF16)
    mask = np.asarray(inputs["attention_mask"], f32)

    common = {
        "wq": wq, "wk": wk, "wv": wv, "wo": wo, "wup": wup, "wdn": wdn,
        "bq": (np.asarray(inputs["b_q"], f32) * (scale * QS)).astype(f32),
        "bk": (np.asarray(inputs["b_k"], f32) * WS).astype(f32),
        "bup": np.asarray(inputs["b_up"], f32),
        "bv": np.asarray(inputs["b_v"], f32),
        "bo": np.asarray(inputs["b_o"], f32),
        "bdn": np.asarray(inputs["b_down"], f32),
        "g1": np.asarray(inputs["norm_weight_1"], f32),
        "b1": np.asarray(inputs["norm_bias_1"], f32),
        "g2": np.asarray(inputs["norm_weight_2"], f32),
        "b2": np.asarray(inputs["norm_bias_2"], f32),
    }
    in_maps = []
    for i in range(N_CORES):
        xc = x[i * TB:(i + 1) * TB].reshape(T, D)
        m = dict(common)
        m["xT8"] = _swz(np.ascontiguousarray(xc.T).astype(NP_FP8), KD2, T)
        m["xr"] = (np.ascontiguousarray(xc) * WS).astype(NP_BF16)
        m["maskT"] = np.ascontiguousarray(mask[i * TB:(i + 1) * TB, 0, 0, :])
        in_maps.append(m)
    return in_maps


def full_cfg(inputs):
    f32 = np.float32
    return {
        "D": D, "F": F, "T": T, "TB": TB, "H": H, "HD": HD,
        "has_bq": bool(np.any(np.asarray(inputs["b_q"], f32))),
        "has_bk": bool(np.any(np.asarray(inputs["b_k"], f32))),
        "has_bup": bool(np.any(np.asarray(inputs["b_up"], f32))),
        "has_mask": bool(np.any(np.asarray(inputs["attention_mask"], f32))),
        "has_bv": bool(np.any(np.asarray(inputs["b_v"], f32))),
        "has_bo": bool(np.any(np.asarray(inputs["b_o"], f32))),
        "has_bdn": bool(np.any(np.asarray(inputs["b_down"], f32))),
        "has_n1": bool(np.any(np.asarray(inputs["norm_weight_1"], f32) != 1.0)
                       or np.any(np.asarray(inputs["norm_bias_1"], f32))),
        "has_n2": bool(np.any(np.asarray(inputs["norm_weight_2"], f32) != 1.0)
                       or np.any(np.asarray(inputs["norm_bias_2"], f32))),
    }


def run_on_hw(inputs, trace=False, tmpdir=None):
    cfg = full_cfg(inputs)
    cfg_key = tuple(sorted((k, v) for k, v in cfg.items()
                           if not isinstance(v, set)))
    nc = _get_program(cfg_key, cfg)
    in_maps = make_in_maps(inputs)
    kw = {}
    if trace:
        kw = {"trace": True, "tmpdir": tmpdir}
    res = run_bass_kernel_spmd(nc, in_maps, core_ids=list(range(N_CORES)),
                               **kw)
    out = np.empty((B, S, D), np.float32)
    for i in range(N_CORES):
        out[i * TB:(i + 1) * TB] = res.results[i]["out"].reshape(TB, S, D)
    return out, res


def kernel(**inputs):
    out, _ = run_on_hw(inputs)
    return out


# revision 13
# speedup vs baseline: 1.3847x; 1.0327x over previous
"""Fused RoBERTa layer (attention + FFN, LoRA merged) on 8 Trainium2 cores.

Sharding: pure data-parallel over batch (16 batches -> 2 per core), no
collectives. LoRA merged into base weights on host; 1/sqrt(hd) folded into
w_q.

fp8 strategy (2x PE throughput via DoubleRow double-pumping):
  - QKV / AV / O-proj / FFN-up matmuls run fp8e4m3 with
    MatmulPerfMode.DoubleRow: both operands hold TWO 128-deep K-chunks side
    by side in the free dim ([128, 2, M]), contracting 256 per pass.
  - FFN-down stays bf16 (fp8 there breaks the accuracy gate).
  - Weights pre-scaled by 2^7 (2^10 for w_q) into fp8 normal range; inverse
    scales folded into exp input scale (2^-17), gelu input scale (2^-7),
    V-evict scale (2^-7), and a host 2^7 pre-scale of the bf16 residual x
    (LN1 is scale-invariant).
  - exp folds a 2^-9 output scale via its bias so unnormalized fp8
    attention weights stay in range; the ones-column denominator is the sum
    of the SAME fp8 weights so the scale cancels exactly.

Attention normalization is BATCHED: the AV matmul emits unnormalized o
rows plus a denominator row per (b,h) (V' ones-column; for odd heads the
ones column comes FIRST and the AV output is written at partition offset
63 so o rows land on partitions 64..127 -- this keeps every evict
same-partition and vector-legal). Unnormalized o is evicted to bf16 oTu;
denominators collect into a [H, T] tile; per batch one reciprocal + one
DMA out + 8 broadcast DMAs back + 8 vector muls produce fp8 oT. The
post-pass of batch b is interleaved into batch b+1's attention (or the
O-projection) so its DMA latency is hidden.

PSUM: one 8-bank pool with manual tags: mm0/mm1 (projection/FFN double
buffer), sc0..3 (scores 4-deep, reused by the LN1 transposes), pv0/pv1
(AV). Host pre-swizzles all fp8 tensors into per-tile [128, W] contiguous
layout so every weight DMA is linear in DRAM.
"""

import math
import sys

sys.path.insert(0, "/opt/trn_rl_repo")

import numpy as np
import ml_dtypes

import concourse.bacc as bacc
import concourse.bass as bass
import concourse.tile as tile
from concourse import mybir
from concourse.bass_utils import run_bass_kernel_spmd
from concourse.masks import make_identity

BF16 = mybir.dt.bfloat16
FP8 = mybir.dt.float8e4
F32 = mybir.dt.float32
NP_BF16 = np.dtype(ml_dtypes.bfloat16)
NP_FP8 = np.dtype(ml_dtypes.float8_e4m3)

B, S, D, H, HD, F = 16, 512, 1024, 16, 64, 4096
N_CORES = 8
TB = B // N_CORES
T = TB * S

MM_N = 512
P = 128

WSHIFT = 7
WS = float(2.0 ** WSHIFT)
QSHIFT = 10
QS = float(2.0 ** QSHIFT)
EXP_SCALE = float(2.0 ** (-(WSHIFT + QSHIFT)))
ATT_BIAS = -9 * math.log(2.0)


def _ceil_div(a, b):
    return (a + b - 1) // b


def build_program(cfg):
    D_, F_, T_, TB_, H_, HD_ = (cfg["D"], cfg["F"], cfg["T"], cfg["TB"],
                                cfg["H"], cfg["HD"])
    S_ = T_ // TB_
    KD = D_ // P
    KD2 = KD // 2
    KF = F_ // P
    TCH = T_ // P
    NT = _ceil_div(T_, MM_N)
    NTW = min(MM_N, T_)
    ND = _ceil_div(D_, MM_N)
    NDW = min(MM_N, D_)
    SKC = S_ // P
    SKC2 = SKC // 2
    HPC = P // HD_             # heads per 128-partition chunk (=2)
    VW = HD_ + 1               # V' per-head width (ones column)
    VROW = H_ * VW             # V' row width for one key chunk
    UPW = 1024
    UPT = F_ // UPW

    nc = bacc.Bacc("TRN2", target_bir_lowering=False, debug=False,
                   num_devices=N_CORES)

    # ---- DRAM I/O (fp8 tensors pre-swizzled on host: [ntile, 128, W]) ----
    xT8_d = nc.dram_tensor("xT8", [KD2, P, 2 * T_], FP8,
                           kind="ExternalInput")
    xr_d = nc.dram_tensor("xr", [T_, D_], BF16, kind="ExternalInput")
    wq_d = nc.dram_tensor("wq", [KD2, P, 2 * D_], FP8, kind="ExternalInput")
    wk_d = nc.dram_tensor("wk", [KD2, P, 2 * D_], FP8, kind="ExternalInput")
    wv_d = nc.dram_tensor("wv", [KD2, P, 2 * D_], FP8, kind="ExternalInput")
    wo_d = nc.dram_tensor("wo", [KD2, P, 2 * D_], FP8, kind="ExternalInput")
    wup_d = nc.dram_tensor("wup", [KD2 * UPT, P, 2 * UPW], FP8,
                           kind="ExternalInput")
    wdn_d = nc.dram_tensor("wdn", [F_, D_], BF16, kind="ExternalInput")
    bq_d = nc.dram_tensor("bq", [D_], F32, kind="ExternalInput")
    bk_d = nc.dram_tensor("bk", [D_], F32, kind="ExternalInput")
    bup_d = nc.dram_tensor("bup", [F_], F32, kind="ExternalInput")
    mask_d = nc.dram_tensor("maskT", [TB_, S_], F32, kind="ExternalInput")
    bv_d = nc.dram_tensor("bv", [D_], F32, kind="ExternalInput")
    bo_d = nc.dram_tensor("bo", [D_], F32, kind="ExternalInput")
    bdn_d = nc.dram_tensor("bdn", [D_], F32, kind="ExternalInput")
    g1_d = nc.dram_tensor("g1", [D_], F32, kind="ExternalInput")
    b1_d = nc.dram_tensor("b1", [D_], F32, kind="ExternalInput")
    g2_d = nc.dram_tensor("g2", [D_], F32, kind="ExternalInput")
    b2_d = nc.dram_tensor("b2", [D_], F32, kind="ExternalInput")
    out_d = nc.dram_tensor("out", [T_, D_], F32, kind="ExternalOutput")

    DR = mybir.MatmulPerfMode.DoubleRow

    with tile.TileContext(nc) as tc, \
         tc.tile_pool(name="consts", bufs=1) as consts, \
         tc.tile_pool(name="slab", bufs=1) as slab, \
         tc.tile_pool(name="pall", bufs=1, space="PSUM") as pall, \
         tc.tile_pool(name="work", bufs=2) as work, \
         tc.tile_pool(name="xrp", bufs=2) as xrp, \
         tc.tile_pool(name="attnp", bufs=6) as attnp, \
         tc.tile_pool(name="attn2", bufs=4) as attn2, \
         tc.tile_pool(name="rbp", bufs=1) as rbp, \
         tc.tile_pool(name="statp", bufs=4) as statp, \
         tc.tile_pool(name="outp", bufs=2) as outp, \
         tc.tile_pool(name="dramp", bufs=2, space="DRAM") as dramp:

        dma = nc.sync          # bulk loads
        dma2 = nc.gpsimd       # latency-bound small DMAs + output

        def slot(tag, width, dtype):
            return slab.tile([P, width], dtype, tag=tag, name=f"t_{tag}")

        def pair(ap_2d, i2):
            return ap_2d.rearrange("p (i w) -> p i w", i=2) if i2 is None \
                else ap_2d.rearrange("p (i w) -> p i w", i=2)[:, :, i2]

        mm_ctr = [0]

        def mm_tile():
            mm_ctr[0] ^= 1
            return pall.tile([P, MM_N], F32, tag=f"mm{mm_ctr[0]}",
                             name="mm")

        # ---- constants ----
        eps_t = consts.tile([P, 1], F32)
        nc.vector.memset(eps_t, 1e-5)
        attb_t = consts.tile([P, 1], F32)
        nc.vector.memset(attb_t, ATT_BIAS)
        zero_t = consts.tile([P, 1], F32)
        nc.vector.memset(zero_t, 0.0)
        ident = consts.tile([P, P], BF16)
        make_identity(nc, ident)
        if cfg["has_bq"]:
            bq_sb = consts.tile([P, KD], F32)
            dma.dma_start(out=bq_sb,
                          in_=bq_d.ap().rearrange("(m p) -> p m", p=P))
        if cfg["has_bk"]:
            bk_sb = consts.tile([P, KD], F32)
            dma.dma_start(out=bk_sb,
                          in_=bk_d.ap().rearrange("(m p) -> p m", p=P))
        if cfg["has_bup"]:
            bup_sb = consts.tile([P, KF], F32)
            dma.dma_start(out=bup_sb,
                          in_=bup_d.ap().rearrange("(m p) -> p m", p=P))
        if cfg["has_mask"]:
            mask_sb = consts.tile([P, TB_ * SKC], F32)
            dma.dma_start(out=mask_sb,
                          in_=mask_d.ap().rearrange("b (kc p) -> p (b kc)",
                                                    p=P))
            mask2_sb = consts.tile([P, TB_ * SKC], F32)
            nc.vector.tensor_scalar_add(out=mask2_sb, in0=mask_sb,
                                        scalar1=ATT_BIAS)

        def bcast_row(dram_vec, n):
            t = consts.tile([P, n], F32, name=f"bc_{dram_vec.name}")
            dma.dma_start(out=t,
                          in_=dram_vec.ap().unsqueeze(0).to_broadcast([P, n]))
            return t

        bv_bc = bcast_row(bv_d, D_) if cfg["has_bv"] else None
        bo_bc = bcast_row(bo_d, D_) if cfg["has_bo"] else None
        bdn_bc = bcast_row(bdn_d, D_) if cfg["has_bdn"] else None
        g1_bc = bcast_row(g1_d, D_) if cfg["has_n1"] else None
        b1_bc = bcast_row(b1_d, D_) if cfg["has_n1"] else None
        g2_bc = bcast_row(g2_d, D_) if cfg["has_n2"] else None
        b2_bc = bcast_row(b2_d, D_) if cfg["has_n2"] else None

        # ---- load x^T and QKV weights (fp8, K-paired, linear DMAs) ----
        xT8_sb = [slot(f"xT8{c2}", 2 * T_, FP8) for c2 in range(KD2)]
        w_sb = {nm: [slot(f"w{nm}{c2}", 2 * D_, FP8) for c2 in range(KD2)]
                for nm in ("q", "k", "v")}
        dma3 = nc.scalar       # second DMA queue for the cold start
        for c2 in range(KD2):
            dma.dma_start(out=w_sb["q"][c2], in_=wq_d[c2])
            dma3.dma_start(out=xT8_sb[c2], in_=xT8_d[c2])
        for c2 in range(KD2):
            dma3.dma_start(out=w_sb["k"][c2], in_=wk_d[c2])
        for c2 in range(KD2):
            dma.dma_start(out=w_sb["v"][c2], in_=wv_d[c2])

        qT_sb = [slot(f"qT{c}", T_, BF16) for c in range(KD)]
        kTe_sb = [slot(f"kTe{c}", T_, BF16) for c in range(KD)]
        kTo_sb = [slot(f"kTo{c}", T_, BF16) for c in range(KD)]
        for c in range(KD):
            nc.vector.memset(kTe_sb[c][P // 2:P, :], 0.0)
            nc.vector.memset(kTo_sb[c][0:P // 2, :], 0.0)
        Vp8_sb = [slot(f"Vp{c}", 2 * VROW, FP8) for c in range(TCH // 2)]

        # ---- QKV projections (fp8 DoubleRow) ----
        HB = P // 2
        for nm in ("q", "k"):
            has_b = cfg["has_bq"] if nm == "q" else cfg["has_bk"]
            bias = (bq_sb if nm == "q" else bk_sb) if has_b else None
            for m in range(KD):
                for t2 in range(NT):
                    pt = mm_tile()
                    for c2 in range(KD2):
                        nc.tensor.matmul(
                            pt[:, :NTW],
                            lhsT=pair(w_sb[nm][c2],
                                      slice(m * P, (m + 1) * P)),
                            rhs=pair(xT8_sb[c2],
                                     slice(t2 * MM_N, t2 * MM_N + NTW)),
                            start=(c2 == 0), stop=(c2 == KD2 - 1),
                            perf_mode=DR)
                    sl = slice(t2 * MM_N, t2 * MM_N + NTW)
                    if nm == "q":
                        if has_b:
                            nc.vector.tensor_scalar_add(
                                out=qT_sb[m][:, sl],
                                in0=pt[:, :NTW], scalar1=bias[:, m:m + 1])
                        else:
                            nc.vector.tensor_copy(out=qT_sb[m][:, sl],
                                                  in_=pt[:, :NTW])
                    else:
                        if has_b:
                            nc.vector.tensor_scalar_add(
                                out=kTe_sb[m][0:HB, sl],
                                in0=pt[0:HB, :NTW],
                                scalar1=bias[0:HB, m:m + 1])
                            nc.vector.tensor_scalar_add(
                                out=kTo_sb[m][HB:P, sl],
                                in0=pt[HB:P, :NTW],
                                scalar1=bias[HB:P, m:m + 1])
                        else:
                            nc.vector.tensor_copy(out=kTe_sb[m][0:HB, sl],
                                                  in_=pt[0:HB, :NTW])
                            nc.vector.tensor_copy(out=kTo_sb[m][HB:P, sl],
                                                  in_=pt[HB:P, :NTW])
        # V token-major into V' ([v(64), 1] per head; 2^-7 scale on evict)
        for tr in range(TCH):
            vdst = Vp8_sb[tr // 2][:, (tr % 2) * VROW:(tr % 2 + 1) * VROW]
            vd3 = vdst.rearrange("p (h c) -> p h c", c=VW)
            for n2 in range(ND):
                pt = mm_tile()
                for c2 in range(KD2):
                    nc.tensor.matmul(
                        pt[:, :NDW],
                        lhsT=pair(xT8_sb[c2], slice(tr * P, (tr + 1) * P)),
                        rhs=pair(w_sb["v"][c2],
                                 slice(n2 * MM_N, n2 * MM_N + NDW)),
                        start=(c2 == 0), stop=(c2 == KD2 - 1),
                        perf_mode=DR)
                hpn = NDW // HD_   # heads per N tile
                src = pt[:, :NDW].rearrange("p (h c) -> p h c", c=HD_)
                if cfg["has_bv"]:
                    tmp = work.tile([P, NDW], F32, tag="vtmp", name="vtmp")
                    nc.vector.tensor_add(
                        out=tmp, in0=pt[:, :NDW],
                        in1=bv_bc[:, n2 * MM_N:n2 * MM_N + NDW])
                    src = tmp.rearrange("p (h c) -> p h c", c=HD_)
                nc.vector.tensor_scalar_mul(
                    out=vd3[:, n2 * hpn:(n2 + 1) * hpn, 0:HD_], in0=src,
                    scalar1=1.0 / WS)
            nc.vector.memset(vd3[:, :, HD_:VW], 1.0)  # ones cols

        skip = cfg.get("skip", set())
        # ---- attention ----
        wo_sb = []
        for c2 in range(KD2):
            t = slot(f"xT8{c2}", 2 * D_, FP8)
            dma.dma_start(out=t, in_=wo_d[c2])
            wo_sb.append(t)
        oT8_sb = [slot(f"wq{c2}", 2 * T_, FP8) for c2 in range(KD2)]
        oTu_sb = [slot(f"wv{hc}" if hc < KD2 else f"oTu{hc - KD2}",
                       T_, BF16) for hc in range(KD)]
        den_d = dramp.tile([H_, T_], F32, tag="den_d", name="den_d")
        rb_sb = {}

        def attn_scores(b, h):
            hc, par = h // HPC, h % HPC
            at_tiles = [attnp.tile([P, 2 * S_], FP8, tag="attnT",
                                   name="attnT") for _ in range(SKC2)]
            kTm = kTe_sb if par == 0 else kTo_sb
            for kc in range(SKC):
                pt = pall.tile([P, MM_N], F32, tag=f"sc{kc}", name="ps_s")
                nc.tensor.matmul(
                    pt[:, :S_],
                    lhsT=kTm[hc][:, b * S_ + kc * P:b * S_ + (kc + 1) * P],
                    rhs=qT_sb[hc][:, b * S_:(b + 1) * S_],
                    start=True, stop=True)
                bias = (mask2_sb[:, b * SKC + kc:b * SKC + kc + 1]
                        if cfg["has_mask"] else attb_t)
                nc.scalar.activation(
                    out=at_tiles[kc // 2][:, (kc % 2) * S_:(kc % 2 + 1) * S_],
                    in_=pt[:, :S_],
                    func=mybir.ActivationFunctionType.Exp,
                    bias=bias, scale=EXP_SCALE)
            return at_tiles

        def attn_av(b, h, at_tiles):
            hc, par = h // HPC, h % HPC
            pv = pall.tile([P, MM_N], F32, tag=f"pv{h % 2}", name="ps_v2")
            for kc2 in range(SKC2):
                nc.tensor.matmul(
                    pv[0:VW, :S_],
                    lhsT=pair(Vp8_sb[b * SKC2 + kc2],
                              slice(h * VW, (h + 1) * VW)),
                    rhs=pair(at_tiles[kc2], None),
                    start=(kc2 == 0), stop=(kc2 == SKC2 - 1),
                    perf_mode=DR)
            ho = par * HD_
            nc.vector.tensor_copy(
                out=oTu_sb[hc][ho:ho + HD_, b * S_:(b + 1) * S_],
                in_=pv[0:HD_, :S_])
            rs = attn2.tile([1, S_], F32, tag="rs", name="rs")
            nc.scalar.copy(out=rs, in_=pv[HD_:VW, :S_])
            dma2.dma_start(out=den_d[h:h + 1, b * S_:(b + 1) * S_], in_=rs)

        def post_a(b):
            # broadcast denominators back + reciprocal, per feature chunk
            sl = slice(b * S_, (b + 1) * S_)
            for hc in range(KD):
                rb = rbp.tile([P, S_], F32, tag=f"rb{hc}", name="rb")
                rb_sb[hc] = rb
                for h2 in range(HPC):
                    dma2.dma_start(
                        out=rb[h2 * HD_:(h2 + 1) * HD_, :],
                        in_=den_d[2 * hc + h2:2 * hc + h2 + 1, sl]
                        .to_broadcast([HD_, S_]))
                nc.vector.reciprocal_approx_fast(out=rb, in_=rb)

        def post_b(b):
            # normalize: oT8 = oTu * rb (same partitions, vector-legal)
            sl = slice(b * S_, (b + 1) * S_)
            for hc in range(KD):
                nc.vector.tensor_mul(
                    out=oT8_sb[hc // 2][:, (hc % 2) * T_ + b * S_:
                                        (hc % 2) * T_ + (b + 1) * S_],
                    in0=oTu_sb[hc][:, sl], in1=rb_sb[hc])

        if "attn" in skip:
            for c in range(KD):
                nc.vector.tensor_copy(
                    out=oT8_sb[c // 2][:, (c % 2) * T_:(c % 2 + 1) * T_],
                    in_=qT_sb[c])
        else:
            prev = None
            for b in range(TB_):
                for h in range(H_):
                    at = attn_scores(b, h)
                    if prev is not None:
                        attn_av(*prev)
                        if prev[:2] == (1, 3):
                            post_b(0)
                    prev = (b, h, at)
                if b == 0:
                    attn_av(*prev)
                    prev = None
                    post_a(0)
            attn_av(*prev)
            post_a(1)

        # ---- O proj (fp8 DR) + residual + LN1 -> x_medium + transpose ----
        xm_bf = [slot(f"qT{c}", D_, BF16) for c in range(TCH)]
        xmT8_sb = [slot(f"wk{c2}", 2 * T_, FP8) for c2 in range(KD2)]
        up_tags = [t for c in range(KD) for t in (f"kTe{c}", f"kTo{c}")]
        wup_sb = []
        for i in range(KD2 * UPT):
            t = slot(up_tags[i], 2 * UPW, FP8)
            dma.dma_start(out=t, in_=wup_d[i])
            wup_sb.append(t)

        def wup_lhsT(c2, fm):
            i = c2 * UPT + (fm * P) // UPW
            o = (fm * P) % UPW
            return pair(wup_sb[i], slice(o, o + P))

        def layer_norm(src, dst, g_bc, b_bc):
            bw = min(512, D_)
            nsub = _ceil_div(D_, bw)
            st = statp.tile([P, nsub, 6], F32, tag="bnst", name="bnst")
            for i in range(nsub):
                nc.vector.bn_stats(out=st[:, i, :],
                                   in_=src[:, i * bw:(i + 1) * bw])
            mv = statp.tile([P, 2], F32, tag="bnmv", name="bnmv")
            nc.vector.bn_aggr(out=mv, in_=st)
            rstd = statp.tile([P, 1], F32, tag="rstd", name="rstd")
            nc.scalar.activation(out=rstd, in_=mv[:, 1:2],
                                 func=mybir.ActivationFunctionType.Sqrt,
                                 bias=eps_t, scale=1.0)
            nc.vector.reciprocal(out=rstd, in_=rstd)
            if g_bc is None:
                nc.vector.tensor_scalar(
                    out=dst, in0=src, scalar1=mv[:, 0:1], scalar2=rstd,
                    op0=mybir.AluOpType.subtract, op1=mybir.AluOpType.mult)
            else:
                tmp = statp.tile([P, D_], F32, tag="lntmp", name="lntmp")
                nc.vector.tensor_scalar(
                    out=tmp, in0=src, scalar1=mv[:, 0:1], scalar2=rstd,
                    op0=mybir.AluOpType.subtract, op1=mybir.AluOpType.mult)
                nc.vector.tensor_mul(out=tmp, in0=tmp, in1=g_bc)
                nc.vector.tensor_add(out=dst, in0=tmp, in1=b_bc)

        def transpose_tr(tr):
            for c in range(KD):
                if "tr" in skip:
                    nc.vector.tensor_copy(
                        out=xmT8_sb[c // 2][:, (c % 2) * T_ + tr * P:
                                            (c % 2) * T_ + (tr + 1) * P],
                        in_=xm_bf[tr][:, c * P:(c + 1) * P])
                else:
                    pt = pall.tile([P, P], BF16, tag=f"sc{c % 4}",
                                   name="ps_t")
                    nc.tensor.transpose(pt, xm_bf[tr][:, c * P:(c + 1) * P],
                                        ident)
                    nc.vector.tensor_copy(
                        out=xmT8_sb[c // 2][:, (c % 2) * T_ + tr * P:
                                            (c % 2) * T_ + (tr + 1) * P],
                        in_=pt)

        for tr in range(TCH):
            xt = xrp.tile([P, D_], BF16, tag="xrt", name="xrt")
            dma.dma_start(out=xt, in_=xr_d[tr * P:(tr + 1) * P, :])
            of = work.tile([P, D_], F32, tag="acc", name="of")
            for n2 in range(ND):
                pt = mm_tile()
                for c2 in range(KD2):
                    nc.tensor.matmul(
                        pt[:, :NDW],
                        lhsT=pair(oT8_sb[c2], slice(tr * P, (tr + 1) * P)),
                        rhs=pair(wo_sb[c2],
                                 slice(n2 * MM_N, n2 * MM_N + NDW)),
                        start=(c2 == 0), stop=(c2 == KD2 - 1),
                        perf_mode=DR)
                nc.vector.tensor_add(out=of[:, n2 * MM_N:n2 * MM_N + NDW],
                                     in0=pt[:, :NDW],
                                     in1=xt[:, n2 * MM_N:n2 * MM_N + NDW])
                if cfg["has_bo"]:
                    nc.vector.tensor_add(
                        out=of[:, n2 * MM_N:n2 * MM_N + NDW],
                        in0=of[:, n2 * MM_N:n2 * MM_N + NDW],
                        in1=bo_bc[:, n2 * MM_N:n2 * MM_N + NDW])
            if "ln" in skip:
                nc.vector.tensor_copy(out=xm_bf[tr], in_=of)
            else:
                layer_norm(of, xm_bf[tr],
                           g1_bc if cfg["has_n1"] else None,
                           b1_bc if cfg["has_n1"] else None)
            if tr > 0:
                transpose_tr(tr - 1)
            if tr == 1 and "attn" not in skip:
                post_b(1)
        transpose_tr(TCH - 1)

        # ---- FFN up (fp8 DR) + Gelu -> gT ----
        g_tags = ([f"g{c}" for c in range(KF - KD)]
                  + [f"wv{c2}" for c2 in range(KD2)]
                  + [f"oTu{c2}" for c2 in range(KD2)])
        gT_sb = [slot(g_tags[c], T_, BF16) for c in range(KF)]
        for fm in range(KF):
            for t2 in range(NT):
                pt = mm_tile()
                for c2 in range(KD2):
                    nc.tensor.matmul(
                        pt[:, :NTW],
                        lhsT=wup_lhsT(c2, fm),
                        rhs=pair(xmT8_sb[c2],
                                 slice(t2 * MM_N, t2 * MM_N + NTW)),
                        start=(c2 == 0), stop=(c2 == KD2 - 1),
                        perf_mode=DR)
                nc.scalar.activation(
                    out=gT_sb[fm][:, t2 * MM_N:t2 * MM_N + NTW],
                    in_=pt[:, :NTW],
                    func=mybir.ActivationFunctionType.Gelu,
                    bias=(bup_sb[:, fm:fm + 1] if cfg["has_bup"]
                          else zero_t),
                    scale=1.0 / WS)

        # ---- FFN down (bf16) + residual + LN2 -> out ----
        dn_tags = ([f"Vp{c}" for c in range(TCH // 2)]
                   + [f"xT8{c2}" for c2 in range(KD2)]
                   + [f"wq{c2}" for c2 in range(KD2)]
                   + up_tags
                   + [f"wk{c2}" for c2 in range(KD2)])
        wdn_sb = []
        for fc in range(KF):
            t = slot(dn_tags[fc], D_, BF16)
            dma.dma_start(out=t, in_=wdn_d[fc * P:(fc + 1) * P, :])
            wdn_sb.append(t)
        for tr in range(TCH):
            dsb = work.tile([P, D_], F32, tag="acc", name="dsb")
            for n2 in range(ND):
                pt = mm_tile()
                for fc in range(KF):
                    nc.tensor.matmul(
                        pt[:, :NDW],
                        lhsT=gT_sb[fc][:, tr * P:(tr + 1) * P],
                        rhs=wdn_sb[fc][:, n2 * MM_N:n2 * MM_N + NDW],
                        start=(fc == 0), stop=(fc == KF - 1))
                nc.vector.tensor_add(
                    out=dsb[:, n2 * MM_N:n2 * MM_N + NDW],
                    in0=pt[:, :NDW],
                    in1=xm_bf[tr][:, n2 * MM_N:n2 * MM_N + NDW])
                if cfg["has_bdn"]:
                    nc.vector.tensor_add(
                        out=dsb[:, n2 * MM_N:n2 * MM_N + NDW],
                        in0=dsb[:, n2 * MM_N:n2 * MM_N + NDW],
                        in1=bdn_bc[:, n2 * MM_N:n2 * MM_N + NDW])
            ot = outp.tile([P, D_], F32, tag="ot", name="ot")
            if "ln" in skip:
                nc.vector.tensor_copy(out=ot, in_=dsb)
            else:
                layer_norm(dsb, ot,
                           g2_bc if cfg["has_n2"] else None,
                           b2_bc if cfg["has_n2"] else None)
            dma.dma_start(out=out_d[tr * P:(tr + 1) * P, :], in_=ot)

    nc.finalize()
    return nc


_PROGRAM_CACHE = {}


def _get_program(cfg_key, cfg):
    if cfg_key not in _PROGRAM_CACHE:
        _PROGRAM_CACHE[cfg_key] = build_program(cfg)
    return _PROGRAM_CACHE[cfg_key]


def _swz(w, npairs, width):
    """[rows, cols] -> [npairs, 128, 2*cols] K-paired contiguous."""
    return np.ascontiguousarray(
        w.reshape(npairs, 2, P, width).transpose(0, 2, 1, 3)
        .reshape(npairs, P, 2 * width))


def make_in_maps(inputs):
    f32 = np.float32
    x = np.asarray(inputs["x"], f32)
    scale = 1.0 / np.sqrt(float(inputs["head_dim"]))

    def merged(w, a, b):
        return (np.asarray(w, f32)
                + np.asarray(a, f32) @ np.asarray(b, f32))

    KD2 = D // P // 2
    wq = _swz((merged(inputs["w_q"], inputs["w_q_lora_a"],
                      inputs["w_q_lora_b"]) * (scale * QS)).astype(NP_FP8),
              KD2, D)
    wk = _swz((merged(inputs["w_k"], inputs["w_k_lora_a"],
                      inputs["w_k_lora_b"]) * WS).astype(NP_FP8), KD2, D)
    wv = _swz((merged(inputs["w_v"], inputs["w_v_lora_a"],
                      inputs["w_v_lora_b"]) * WS).astype(NP_FP8), KD2, D)
    wo = _swz((merged(inputs["w_o"], inputs["w_o_lora_a"],
                      inputs["w_o_lora_b"]) * WS).astype(NP_FP8), KD2, D)
    wup8 = (merged(inputs["w_up"], inputs["w_up_lora_a"],
                   inputs["w_up_lora_b"]) * WS).astype(NP_FP8)
    UPW = 1024
    UPT = F // UPW
    wup = np.ascontiguousarray(
        wup8.reshape(KD2, 2, P, UPT, UPW).transpose(0, 3, 2, 1, 4)
        .reshape(KD2 * UPT, P, 2 * UPW))
    wdn = merged(inputs["w_down"], inputs["w_down_lora_a"],
                 inputs["w_down_lora_b"]).astype(NP_BF16)
    mask = np.asarray(inputs["attention_mask"], f32)

    common = {
        "wq": wq, "wk": wk, "wv": wv, "wo": wo, "wup": wup, "wdn": wdn,
        "bq": (np.asarray(inputs["b_q"], f32) * (scale * QS)).astype(f32),
        "bk": (np.asarray(inputs["b_k"], f32) * WS).astype(f32),
        "bup": np.asarray(inputs["b_up"], f32),
        "bv": np.asarray(inputs["b_v"], f32),
        "bo": np.asarray(inputs["b_o"], f32),
        "bdn": np.asarray(inputs["b_down"], f32),
        "g1": np.asarray(inputs["norm_weight_1"], f32),
        "b1": np.asarray(inputs["norm_bias_1"], f32),
        "g2": np.asarray(inputs["norm_weight_2"], f32),
        "b2": np.asarray(inputs["norm_bias_2"], f32),
    }
    in_maps = []
    for i in range(N_CORES):
        xc = x[i * TB:(i + 1) * TB].reshape(T, D)
        m = dict(common)
        m["xT8"] = _swz(np.ascontiguousarray(xc.T).astype(NP_FP8), KD2, T)
        m["xr"] = (np.ascontiguousarray(xc) * WS).astype(NP_BF16)
        m["maskT"] = np.ascontiguousarray(mask[i * TB:(i + 1) * TB, 0, 0, :])
        in_maps.append(m)
    return in_maps


def full_cfg(inputs):
    f32 = np.float32
    return {
        "D": D, "F": F, "T": T, "TB": TB, "H": H, "HD": HD,
        "has_bq": bool(np.any(np.asarray(inputs["b_q"], f32))),
        "has_bk": bool(np.any(np.asarray(inputs["b_k"], f32))),
        "has_bup": bool(np.any(np.asarray(inputs["b_up"], f32))),
        "has_mask": bool(np.any(np.asarray(inputs["attention_mask"], f32))),
        "has_bv": bool(np.any(np.asarray(inputs["b_v"], f32))),
        "has_bo": bool(np.any(np.asarray(inputs["b_o"], f32))),
        "has_bdn": bool(np.any(np.asarray(inputs["b_down"], f32))),
        "has_n1": bool(np.any(np.asarray(inputs["norm_weight_1"], f32) != 1.0)
                       or np.any(np.asarray(inputs["norm_bias_1"], f32))),
        "has_n2": bool(np.any(np.asarray(inputs["norm_weight_2"], f32) != 1.0)
                       or np.any(np.asarray(inputs["norm_bias_2"], f32))),
    }


def run_on_hw(inputs, trace=False, tmpdir=None):
    cfg = full_cfg(inputs)
    cfg_key = tuple(sorted((k, v) for k, v in cfg.items()
                           if not isinstance(v, set)))
    nc = _get_program(cfg_key, cfg)
    in_maps = make_in_maps(inputs)
    kw = {}
    if trace:
        kw = {"trace": True, "tmpdir": tmpdir}
    res = run_bass_kernel_spmd(nc, in_maps, core_ids=list(range(N_CORES)),
                               **kw)
    out = np.empty((B, S, D), np.float32)
    for i in range(N_CORES):
        out[i * TB:(i + 1) * TB] = res.results[i]["out"].reshape(TB, S, D)
    return out, res


def kernel(**inputs):
    out, _ = run_on_hw(inputs)
    return out


# revision 17
# speedup vs baseline: 1.4403x; 1.0402x over previous
"""Fused RoBERTa layer (attention + FFN, LoRA merged) on 8 Trainium2 cores.

Sharding: pure data-parallel over batch (16 batches -> 2 per core), no
collectives. LoRA merged into base weights on host; 1/sqrt(hd) folded into
w_q.

fp8 strategy (2x PE throughput via DoubleRow double-pumping):
  - QKV / AV / O-proj / FFN-up matmuls run fp8e4m3 with
    MatmulPerfMode.DoubleRow: both operands hold TWO 128-deep K-chunks side
    by side in the free dim ([128, 2, M]), contracting 256 per pass.
  - FFN-down stays bf16 (fp8 there breaks the accuracy gate).
  - Weights pre-scaled by 2^7 (2^10 for w_q) into fp8 normal range; inverse
    scales folded into exp input scale (2^-17), gelu input scale (2^-7),
    V-evict scale (2^-7), and a host 2^7 pre-scale of the bf16 residual x
    (LN1 is scale-invariant).
  - exp folds a 2^-9 output scale via its bias so unnormalized fp8
    attention weights stay in range; the ones-column denominator is the sum
    of the SAME fp8 weights so the scale cancels exactly.

Attention normalization is BATCHED: the AV matmul emits unnormalized o
rows plus a denominator row per (b,h) (V' ones-column; for odd heads the
ones column comes FIRST and the AV output is written at partition offset
63 so o rows land on partitions 64..127 -- this keeps every evict
same-partition and vector-legal). Unnormalized o is evicted to bf16 oTu;
denominators collect into a [H, T] tile; per batch one reciprocal + one
DMA out + 8 broadcast DMAs back + 8 vector muls produce fp8 oT. The
post-pass of batch b is interleaved into batch b+1's attention (or the
O-projection) so its DMA latency is hidden.

PSUM: one 8-bank pool with manual tags: mm0/mm1 (projection/FFN double
buffer), sc0..3 (scores 4-deep, reused by the LN1 transposes), pv0/pv1
(AV). Host pre-swizzles all fp8 tensors into per-tile [128, W] contiguous
layout so every weight DMA is linear in DRAM.
"""

import math
import sys

sys.path.insert(0, "/opt/trn_rl_repo")

import numpy as np
import ml_dtypes

import concourse.bacc as bacc
import concourse.bass as bass
import concourse.tile as tile
from concourse import mybir
from concourse.bass_utils import run_bass_kernel_spmd
from concourse.masks import make_identity

BF16 = mybir.dt.bfloat16
FP8 = mybir.dt.float8e4
F32 = mybir.dt.float32
NP_BF16 = np.dtype(ml_dtypes.bfloat16)
NP_FP8 = np.dtype(ml_dtypes.float8_e4m3)

B, S, D, H, HD, F = 16, 512, 1024, 16, 64, 4096
N_CORES = 8
TB = B // N_CORES
T = TB * S

MM_N = 512
P = 128

WSHIFT = 7
WS = float(2.0 ** WSHIFT)
QSHIFT = 10
QS = float(2.0 ** QSHIFT)
EXP_SCALE = float(2.0 ** (-(WSHIFT + QSHIFT)))
ATT_BIAS = -9 * math.log(2.0)


def _ceil_div(a, b):
    return (a + b - 1) // b


def build_program(cfg):
    D_, F_, T_, TB_, H_, HD_ = (cfg["D"], cfg["F"], cfg["T"], cfg["TB"],
                                cfg["H"], cfg["HD"])
    S_ = T_ // TB_
    KD = D_ // P
    KD2 = KD // 2
    KF = F_ // P
    TCH = T_ // P
    NT = _ceil_div(T_, MM_N)
    NTW = min(MM_N, T_)
    ND = _ceil_div(D_, MM_N)
    NDW = min(MM_N, D_)
    SKC = S_ // P
    SKC2 = SKC // 2
    HPC = P // HD_             # heads per 128-partition chunk (=2)
    VW = HD_ + 1               # V' per-head width (ones column)
    VROW = H_ * VW             # V' row width for one key chunk
    UPW = 1024
    UPT = F_ // UPW

    nc = bacc.Bacc("TRN2", target_bir_lowering=False, debug=False,
                   num_devices=N_CORES)

    # ---- DRAM I/O (fp8 tensors pre-swizzled on host: [ntile, 128, W]) ----
    xT8_d = nc.dram_tensor("xT8", [KD2, P, 2 * T_], FP8,
                           kind="ExternalInput")
    xr_d = nc.dram_tensor("xr", [T_, D_], BF16, kind="ExternalInput")
    wq_d = nc.dram_tensor("wq", [KD2, P, 2 * D_], FP8, kind="ExternalInput")
    wk_d = nc.dram_tensor("wk", [KD2, P, 2 * D_], FP8, kind="ExternalInput")
    wv_d = nc.dram_tensor("wv", [KD2, P, 2 * D_], FP8, kind="ExternalInput")
    wo_d = nc.dram_tensor("wo", [KD2, P, 2 * D_], FP8, kind="ExternalInput")
    wup_d = nc.dram_tensor("wup", [KD2 * UPT, P, 2 * UPW], FP8,
                           kind="ExternalInput")
    wdn_d = nc.dram_tensor("wdn", [F_, D_], BF16, kind="ExternalInput")
    bq_d = nc.dram_tensor("bq", [D_], F32, kind="ExternalInput")
    bk_d = nc.dram_tensor("bk", [D_], F32, kind="ExternalInput")
    bup_d = nc.dram_tensor("bup", [F_], F32, kind="ExternalInput")
    mask_d = nc.dram_tensor("maskT", [TB_, S_], F32, kind="ExternalInput")
    bv_d = nc.dram_tensor("bv", [D_], F32, kind="ExternalInput")
    bo_d = nc.dram_tensor("bo", [D_], F32, kind="ExternalInput")
    bdn_d = nc.dram_tensor("bdn", [D_], F32, kind="ExternalInput")
    g1_d = nc.dram_tensor("g1", [D_], F32, kind="ExternalInput")
    b1_d = nc.dram_tensor("b1", [D_], F32, kind="ExternalInput")
    g2_d = nc.dram_tensor("g2", [D_], F32, kind="ExternalInput")
    b2_d = nc.dram_tensor("b2", [D_], F32, kind="ExternalInput")
    out_d = nc.dram_tensor("out", [T_, D_], F32, kind="ExternalOutput")

    DR = mybir.MatmulPerfMode.DoubleRow

    with tile.TileContext(nc) as tc, \
         tc.tile_pool(name="consts", bufs=1) as consts, \
         tc.tile_pool(name="slab", bufs=1) as slab, \
         tc.tile_pool(name="pall", bufs=1, space="PSUM") as pall, \
         tc.tile_pool(name="work", bufs=2) as work, \
         tc.tile_pool(name="xrp", bufs=2) as xrp, \
         tc.tile_pool(name="attnp", bufs=6) as attnp, \
         tc.tile_pool(name="attn2", bufs=4) as attn2, \
         tc.tile_pool(name="rbp", bufs=1) as rbp, \
         tc.tile_pool(name="statp", bufs=4) as statp, \
         tc.tile_pool(name="outp", bufs=2) as outp, \
         tc.tile_pool(name="dramp", bufs=2, space="DRAM") as dramp:

        dma = nc.sync          # bulk loads
        dma2 = nc.gpsimd       # latency-bound small DMAs + output

        def slot(tag, width, dtype):
            return slab.tile([P, width], dtype, tag=tag, name=f"t_{tag}")

        def pair(ap_2d, i2):
            return ap_2d.rearrange("p (i w) -> p i w", i=2) if i2 is None \
                else ap_2d.rearrange("p (i w) -> p i w", i=2)[:, :, i2]

        mm_ctr = [0]

        def mm_tile():
            mm_ctr[0] ^= 1
            return pall.tile([P, MM_N], F32, tag=f"mm{mm_ctr[0]}",
                             name="mm")

        # ---- constants ----
        eps_t = consts.tile([P, 1], F32)
        nc.vector.memset(eps_t, 1e-5)
        attb_t = consts.tile([P, 1], F32)
        nc.vector.memset(attb_t, ATT_BIAS)
        zero_t = consts.tile([P, 1], F32)
        nc.vector.memset(zero_t, 0.0)
        ident = consts.tile([P, P], BF16)
        make_identity(nc, ident)
        if cfg["has_bq"]:
            bq_sb = consts.tile([P, KD], F32)
            dma.dma_start(out=bq_sb,
                          in_=bq_d.ap().rearrange("(m p) -> p m", p=P))
        if cfg["has_bk"]:
            bk_sb = consts.tile([P, KD], F32)
            dma.dma_start(out=bk_sb,
                          in_=bk_d.ap().rearrange("(m p) -> p m", p=P))
        if cfg["has_bup"]:
            bup_sb = consts.tile([P, KF], F32)
            dma.dma_start(out=bup_sb,
                          in_=bup_d.ap().rearrange("(m p) -> p m", p=P))
        if cfg["has_mask"]:
            mask_sb = consts.tile([P, TB_ * SKC], F32)
            dma.dma_start(out=mask_sb,
                          in_=mask_d.ap().rearrange("b (kc p) -> p (b kc)",
                                                    p=P))
            mask2_sb = consts.tile([P, TB_ * SKC], F32)
            nc.vector.tensor_scalar_add(out=mask2_sb, in0=mask_sb,
                                        scalar1=ATT_BIAS)

        def bcast_row(dram_vec, n):
            t = consts.tile([P, n], F32, name=f"bc_{dram_vec.name}")
            dma.dma_start(out=t,
                          in_=dram_vec.ap().unsqueeze(0).to_broadcast([P, n]))
            return t

        bv_bc = bcast_row(bv_d, D_) if cfg["has_bv"] else None
        bo_bc = bcast_row(bo_d, D_) if cfg["has_bo"] else None
        bdn_bc = bcast_row(bdn_d, D_) if cfg["has_bdn"] else None
        g1_bc = bcast_row(g1_d, D_) if cfg["has_n1"] else None
        b1_bc = bcast_row(b1_d, D_) if cfg["has_n1"] else None
        g2_bc = bcast_row(g2_d, D_) if cfg["has_n2"] else None
        b2_bc = bcast_row(b2_d, D_) if cfg["has_n2"] else None

        # ---- load x^T and QKV weights (fp8, K-paired, linear DMAs) ----
        xT8_sb = [slot(f"xT8{c2}", 2 * T_, FP8) for c2 in range(KD2)]
        w_sb = {nm: [slot(f"w{nm}{c2}", 2 * D_, FP8) for c2 in range(KD2)]
                for nm in ("q", "k", "v")}
        dma3 = nc.scalar       # second DMA queue for the cold start
        for c2 in range(KD2):
            dma.dma_start(out=w_sb["q"][c2], in_=wq_d[c2])
            dma3.dma_start(out=xT8_sb[c2], in_=xT8_d[c2])
        for c2 in range(KD2):
            dma3.dma_start(out=w_sb["k"][c2], in_=wk_d[c2])
        for c2 in range(KD2):
            dma.dma_start(out=w_sb["v"][c2], in_=wv_d[c2])

        qT_sb = [slot(f"qT{c}", T_, BF16) for c in range(KD)]
        kTe_sb = [slot(f"kTe{c}", T_, BF16) for c in range(KD)]
        kTo_sb = [slot(f"kTo{c}", T_, BF16) for c in range(KD)]
        for c in range(KD):
            nc.gpsimd.memset(kTe_sb[c][P // 2:P, :], 0.0)
            nc.gpsimd.memset(kTo_sb[c][0:P // 2, :], 0.0)
        Vp8_sb = [slot(f"Vp{c}", 2 * VROW, FP8) for c in range(TCH // 2)]

        # ---- QKV projections (fp8 DoubleRow) ----
        HB = P // 2
        for nm in ("q", "k"):
            has_b = cfg["has_bq"] if nm == "q" else cfg["has_bk"]
            bias = (bq_sb if nm == "q" else bk_sb) if has_b else None
            for m in range(KD):
                for t2 in range(NT):
                    pt = mm_tile()
                    for c2 in range(KD2):
                        nc.tensor.matmul(
                            pt[:, :NTW],
                            lhsT=pair(w_sb[nm][c2],
                                      slice(m * P, (m + 1) * P)),
                            rhs=pair(xT8_sb[c2],
                                     slice(t2 * MM_N, t2 * MM_N + NTW)),
                            start=(c2 == 0), stop=(c2 == KD2 - 1),
                            perf_mode=DR)
                    sl = slice(t2 * MM_N, t2 * MM_N + NTW)
                    if nm == "q":
                        if has_b:
                            nc.vector.tensor_scalar_add(
                                out=qT_sb[m][:, sl],
                                in0=pt[:, :NTW], scalar1=bias[:, m:m + 1])
                        else:
                            nc.vector.tensor_copy(out=qT_sb[m][:, sl],
                                                  in_=pt[:, :NTW])
                    else:
                        if has_b:
                            nc.vector.tensor_scalar_add(
                                out=kTe_sb[m][0:HB, sl],
                                in0=pt[0:HB, :NTW],
                                scalar1=bias[0:HB, m:m + 1])
                            nc.vector.tensor_scalar_add(
                                out=kTo_sb[m][HB:P, sl],
                                in0=pt[HB:P, :NTW],
                                scalar1=bias[HB:P, m:m + 1])
                        else:
                            nc.vector.tensor_copy(out=kTe_sb[m][0:HB, sl],
                                                  in_=pt[0:HB, :NTW])
                            nc.vector.tensor_copy(out=kTo_sb[m][HB:P, sl],
                                                  in_=pt[HB:P, :NTW])
        # V token-major into V' ([v(64), 1] per head; 2^-7 scale on evict)
        def v_proj_tr(tr):
            vdst = Vp8_sb[tr // 2][:, (tr % 2) * VROW:(tr % 2 + 1) * VROW]
            vd3 = vdst.rearrange("p (h c) -> p h c", c=VW)
            for n2 in range(ND):
                pt = mm_tile()
                for c2 in range(KD2):
                    nc.tensor.matmul(
                        pt[:, :NDW],
                        lhsT=pair(xT8_sb[c2], slice(tr * P, (tr + 1) * P)),
                        rhs=pair(w_sb["v"][c2],
                                 slice(n2 * MM_N, n2 * MM_N + NDW)),
                        start=(c2 == 0), stop=(c2 == KD2 - 1),
                        perf_mode=DR)
                hpn = NDW // HD_   # heads per N tile
                src = pt[:, :NDW].rearrange("p (h c) -> p h c", c=HD_)
                if cfg["has_bv"]:
                    tmp = work.tile([P, NDW], F32, tag="vtmp", name="vtmp")
                    nc.vector.tensor_add(
                        out=tmp, in0=pt[:, :NDW],
                        in1=bv_bc[:, n2 * MM_N:n2 * MM_N + NDW])
                    src = tmp.rearrange("p (h c) -> p h c", c=HD_)
                nc.vector.tensor_scalar_mul(
                    out=vd3[:, n2 * hpn:(n2 + 1) * hpn, 0:HD_], in0=src,
                    scalar1=1.0 / WS)
            nc.vector.memset(vd3[:, :, HD_:VW], 1.0)  # ones cols

        for tr in range(TCH // 2):   # b0 chunks now; rest inside attention
            v_proj_tr(tr)

        skip = cfg.get("skip", set())
        # ---- attention (+ interleaved V-proj b1-chunks / O-proj b0-chunks,
        #      which keep the PE fed while the ScalarE runs the exps) ----
        wo_sb = []

        def load_wo():
            # xT8 tags are dead only once every V-proj chunk has run
            for c2 in range(KD2):
                t = slot(f"xT8{c2}", 2 * D_, FP8)
                dma.dma_start(out=t, in_=wo_d[c2])
                wo_sb.append(t)
        oT8_sb = [slot(f"wq{c2}", 2 * T_, FP8) for c2 in range(KD2)]
        oTu_sb = [slot(f"oTu{hc}", T_, BF16) for hc in range(KD)]
        den_d = dramp.tile([H_, T_], F32, tag="den_d", name="den_d")
        rb_sb = {}

        def attn_scores(b, h):
            hc, par = h // HPC, h % HPC
            at_tiles = [attnp.tile([P, 2 * S_], FP8, tag="attnT",
                                   name="attnT") for _ in range(SKC2)]
            kTm = kTe_sb if par == 0 else kTo_sb
            for kc in range(SKC):
                pt = pall.tile([P, MM_N], F32, tag=f"sc{kc}", name="ps_s")
                nc.tensor.matmul(
                    pt[:, :S_],
                    lhsT=kTm[hc][:, b * S_ + kc * P:b * S_ + (kc + 1) * P],
                    rhs=qT_sb[hc][:, b * S_:(b + 1) * S_],
                    start=True, stop=True)
                bias = (mask2_sb[:, b * SKC + kc:b * SKC + kc + 1]
                        if cfg["has_mask"] else attb_t)
                nc.scalar.activation(
                    out=at_tiles[kc // 2][:, (kc % 2) * S_:(kc % 2 + 1) * S_],
                    in_=pt[:, :S_],
                    func=mybir.ActivationFunctionType.Exp,
                    bias=bias, scale=EXP_SCALE)
            return at_tiles

        def attn_av(b, h, at_tiles):
            hc, par = h // HPC, h % HPC
            pv = pall.tile([P, MM_N], F32, tag=f"pv{h % 2}", name="ps_v2")
            for kc2 in range(SKC2):
                nc.tensor.matmul(
                    pv[0:VW, :S_],
                    lhsT=pair(Vp8_sb[b * SKC2 + kc2],
                              slice(h * VW, (h + 1) * VW)),
                    rhs=pair(at_tiles[kc2], None),
                    start=(kc2 == 0), stop=(kc2 == SKC2 - 1),
                    perf_mode=DR)
            ho = par * HD_
            nc.vector.tensor_copy(
                out=oTu_sb[hc][ho:ho + HD_, b * S_:(b + 1) * S_],
                in_=pv[0:HD_, :S_])
            rs = attn2.tile([1, S_], F32, tag="rs", name="rs")
            nc.scalar.copy(out=rs, in_=pv[HD_:VW, :S_])
            dma2.dma_start(out=den_d[h:h + 1, b * S_:(b + 1) * S_], in_=rs)

        def post_a(b):
            # broadcast denominators back + reciprocal, per feature chunk
            sl = slice(b * S_, (b + 1) * S_)
            for hc in range(KD):
                rb = rbp.tile([P, S_], F32, tag=f"rb{hc}", name="rb")
                rb_sb[hc] = rb
                for h2 in range(HPC):
                    dma2.dma_start(
                        out=rb[h2 * HD_:(h2 + 1) * HD_, :],
                        in_=den_d[2 * hc + h2:2 * hc + h2 + 1, sl]
                        .to_broadcast([HD_, S_]))
                nc.vector.reciprocal_approx_fast(out=rb, in_=rb)

        def post_b(b):
            # normalize: oT8 = oTu * rb (same partitions, vector-legal)
            sl = slice(b * S_, (b + 1) * S_)
            for hc in range(KD):
                nc.vector.tensor_mul(
                    out=oT8_sb[hc // 2][:, (hc % 2) * T_ + b * S_:
                                        (hc % 2) * T_ + (b + 1) * S_],
                    in0=oTu_sb[hc][:, sl], in1=rb_sb[hc])

        # O-proj machinery (defined early so tr 0..3 interleave into b1)
        xm_bf = {}
        xmT8_sb = [slot(f"wk{c2}", 2 * T_, FP8) for c2 in range(KD2)]

        def layer_norm(src, dst, g_bc, b_bc):
            bw = min(512, D_)
            nsub = _ceil_div(D_, bw)
            st = statp.tile([P, nsub, 6], F32, tag="bnst", name="bnst")
            for i in range(nsub):
                nc.vector.bn_stats(out=st[:, i, :],
                                   in_=src[:, i * bw:(i + 1) * bw])
            mv = statp.tile([P, 2], F32, tag="bnmv", name="bnmv")
            nc.vector.bn_aggr(out=mv, in_=st)
            rstd = statp.tile([P, 1], F32, tag="rstd", name="rstd")
            nc.scalar.activation(out=rstd, in_=mv[:, 1:2],
                                 func=mybir.ActivationFunctionType.Sqrt,
                                 bias=eps_t, scale=1.0)
            nc.vector.reciprocal(out=rstd, in_=rstd)
            if g_bc is None:
                nc.vector.tensor_scalar(
                    out=dst, in0=src, scalar1=mv[:, 0:1], scalar2=rstd,
                    op0=mybir.AluOpType.subtract, op1=mybir.AluOpType.mult)
            else:
                tmp = statp.tile([P, D_], F32, tag="lntmp", name="lntmp")
                nc.vector.tensor_scalar(
                    out=tmp, in0=src, scalar1=mv[:, 0:1], scalar2=rstd,
                    op0=mybir.AluOpType.subtract, op1=mybir.AluOpType.mult)
                nc.vector.tensor_mul(out=tmp, in0=tmp, in1=g_bc)
                nc.vector.tensor_add(out=dst, in0=tmp, in1=b_bc)

        def o_proj_tr(tr):
            xt = xrp.tile([P, D_], BF16, tag="xrt", name="xrt")
            dma.dma_start(out=xt, in_=xr_d[tr * P:(tr + 1) * P, :])
            of = work.tile([P, D_], F32, tag="acc", name="of")
            for n2 in range(ND):
                pt = mm_tile()
                for c2 in range(KD2):
                    nc.tensor.matmul(
                        pt[:, :NDW],
                        lhsT=pair(oT8_sb[c2], slice(tr * P, (tr + 1) * P)),
                        rhs=pair(wo_sb[c2],
                                 slice(n2 * MM_N, n2 * MM_N + NDW)),
                        start=(c2 == 0), stop=(c2 == KD2 - 1),
                        perf_mode=DR)
                nc.vector.tensor_add(out=of[:, n2 * MM_N:n2 * MM_N + NDW],
                                     in0=pt[:, :NDW],
                                     in1=xt[:, n2 * MM_N:n2 * MM_N + NDW])
                if cfg["has_bo"]:
                    nc.vector.tensor_add(
                        out=of[:, n2 * MM_N:n2 * MM_N + NDW],
                        in0=of[:, n2 * MM_N:n2 * MM_N + NDW],
                        in1=bo_bc[:, n2 * MM_N:n2 * MM_N + NDW])
            xm = slot(f"qT{tr}", D_, BF16)   # reuse qT slot (scores done)
            xm_bf[tr] = xm
            if "ln" in skip:
                nc.vector.tensor_copy(out=xm, in_=of)
            else:
                layer_norm(of, xm,
                           g1_bc if cfg["has_n1"] else None,
                           b1_bc if cfg["has_n1"] else None)

        def transpose_tr(tr):
            for c in range(KD):
                if "tr" in skip:
                    nc.vector.tensor_copy(
                        out=xmT8_sb[c // 2][:, (c % 2) * T_ + tr * P:
                                            (c % 2) * T_ + (tr + 1) * P],
                        in_=xm_bf[tr][:, c * P:(c + 1) * P])
                else:
                    pt = pall.tile([P, P], BF16, tag=f"sc{c % 4}",
                                   name="ps_t")
                    nc.tensor.transpose(pt, xm_bf[tr][:, c * P:(c + 1) * P],
                                        ident)
                    nc.vector.tensor_copy(
                        out=xmT8_sb[c // 2][:, (c % 2) * T_ + tr * P:
                                            (c % 2) * T_ + (tr + 1) * P],
                        in_=pt)

        if "attn" in skip:
            for c in range(KD):
                nc.vector.tensor_copy(
                    out=oT8_sb[c // 2][:, (c % 2) * T_:(c % 2 + 1) * T_],
                    in_=qT_sb[c])
            for tr in range(TCH // 2, TCH):
                v_proj_tr(tr)
            load_wo()
            for tr in range(TCH):
                o_proj_tr(tr)
                if tr > 0:
                    transpose_tr(tr - 1)
            transpose_tr(TCH - 1)
        else:
            fill_b0 = {2: TCH // 2, 5: TCH // 2 + 1,
                       8: TCH // 2 + 2, 11: TCH // 2 + 3}
            fill_b1 = {5: 0, 8: 1, 11: 2, 14: 3}
            prev = None
            for b in range(TB_):
                for h in range(H_):
                    at = attn_scores(b, h)
                    if prev is not None:
                        attn_av(*prev)
                        if prev[:2] == (1, 3):
                            post_b(0)
                    prev = (b, h, at)
                    if b == 0 and h in fill_b0:
                        v_proj_tr(fill_b0[h])
                    if b == 1 and h in fill_b1:
                        o_proj_tr(fill_b1[h])
                if b == 0:
                    attn_av(*prev)
                    prev = None
                    post_a(0)
                    load_wo()
            attn_av(*prev)
            post_a(1)
            for i in range(TCH // 2):
                transpose_tr(i)
            post_b(1)
            for tr in range(TCH // 2, TCH):
                o_proj_tr(tr)
                if tr > TCH // 2:
                    transpose_tr(tr - 1)
            transpose_tr(TCH - 1)

        # ---- FFN up weights (after all kTe/kTo readers; tag reuse) ----
        up_tags = [t for c in range(KD) for t in (f"kTe{c}", f"kTo{c}")]
        wup_sb = []
        for i in range(KD2 * UPT):
            t = slot(up_tags[i], 2 * UPW, FP8)
            dma.dma_start(out=t, in_=wup_d[i])
            wup_sb.append(t)

        def wup_lhsT(c2, fm):
            i = c2 * UPT + (fm * P) // UPW
            o = (fm * P) % UPW
            return pair(wup_sb[i], slice(o, o + P))

        # ---- FFN up (fp8 DR) + Gelu -> gT ----
        g_tags = ([f"g{c}" for c in range(KF - KD - KD2)]
                  + [f"wv{c2}" for c2 in range(KD2)]
                  + [f"oTu{hc}" for hc in range(KD)])
        gT_sb = [slot(g_tags[c], T_, BF16) for c in range(KF)]
        for fm in range(KF):
            for t2 in range(NT):
                pt = mm_tile()
                for c2 in range(KD2):
                    nc.tensor.matmul(
                        pt[:, :NTW],
                        lhsT=wup_lhsT(c2, fm),
                        rhs=pair(xmT8_sb[c2],
                                 slice(t2 * MM_N, t2 * MM_N + NTW)),
                        start=(c2 == 0), stop=(c2 == KD2 - 1),
                        perf_mode=DR)
                nc.scalar.activation(
                    out=gT_sb[fm][:, t2 * MM_N:t2 * MM_N + NTW],
                    in_=pt[:, :NTW],
                    func=mybir.ActivationFunctionType.Gelu,
                    bias=(bup_sb[:, fm:fm + 1] if cfg["has_bup"]
                          else zero_t),
                    scale=1.0 / WS)

        # ---- FFN down (bf16) + residual + LN2 -> out ----
        dn_tags = ([f"Vp{c}" for c in range(TCH // 2)]
                   + [f"xT8{c2}" for c2 in range(KD2)]
                   + [f"wq{c2}" for c2 in range(KD2)]
                   + up_tags
                   + [f"wk{c2}" for c2 in range(KD2)])
        wdn_sb = []
        for fc in range(KF):
            t = slot(dn_tags[fc], D_, BF16)
            dma.dma_start(out=t, in_=wdn_d[fc * P:(fc + 1) * P, :])
            wdn_sb.append(t)
        for tr in range(TCH):
            dsb = work.tile([P, D_], F32, tag="acc", name="dsb")
            for n2 in range(ND):
                pt = mm_tile()
                for fc in range(KF):
                    nc.tensor.matmul(
                        pt[:, :NDW],
                        lhsT=gT_sb[fc][:, tr * P:(tr + 1) * P],
                        rhs=wdn_sb[fc][:, n2 * MM_N:n2 * MM_N + NDW],
                        start=(fc == 0), stop=(fc == KF - 1))
                nc.vector.tensor_add(
                    out=dsb[:, n2 * MM_N:n2 * MM_N + NDW],
                    in0=pt[:, :NDW],
                    in1=xm_bf[tr][:, n2 * MM_N:n2 * MM_N + NDW])
                if cfg["has_bdn"]:
                    nc.vector.tensor_add(
                        out=dsb[:, n2 * MM_N:n2 * MM_N + NDW],
                        in0=dsb[:, n2 * MM_N:n2 * MM_N + NDW],
                        in1=bdn_bc[:, n2 * MM_N:n2 * MM_N + NDW])
            ot = outp.tile([P, D_], F32, tag="ot", name="ot")
            if "ln" in skip:
                nc.vector.tensor_copy(out=ot, in_=dsb)
            else:
                layer_norm(dsb, ot,
                           g2_bc if cfg["has_n2"] else None,
                           b2_bc if cfg["has_n2"] else None)
            dma.dma_start(out=out_d[tr * P:(tr + 1) * P, :], in_=ot)

    nc.finalize()
    return nc


_PROGRAM_CACHE = {}


def _get_program(cfg_key, cfg):
    if cfg_key not in _PROGRAM_CACHE:
        _PROGRAM_CACHE[cfg_key] = build_program(cfg)
    return _PROGRAM_CACHE[cfg_key]


def _swz(w, npairs, width):
    """[rows, cols] -> [npairs, 128, 2*cols] K-paired contiguous."""
    return np.ascontiguousarray(
        w.reshape(npairs, 2, P, width).transpose(0, 2, 1, 3)
        .reshape(npairs, P, 2 * width))


def make_in_maps(inputs):
    f32 = np.float32
    x = np.asarray(inputs["x"], f32)
    scale = 1.0 / np.sqrt(float(inputs["head_dim"]))

    def merged(w, a, b):
        return (np.asarray(w, f32)
                + np.asarray(a, f32) @ np.asarray(b, f32))

    KD2 = D // P // 2
    wq = _swz((merged(inputs["w_q"], inputs["w_q_lora_a"],
                      inputs["w_q_lora_b"]) * (scale * QS)).astype(NP_FP8),
              KD2, D)
    wk = _swz((merged(inputs["w_k"], inputs["w_k_lora_a"],
                      inputs["w_k_lora_b"]) * WS).astype(NP_FP8), KD2, D)
    wv = _swz((merged(inputs["w_v"], inputs["w_v_lora_a"],
                      inputs["w_v_lora_b"]) * WS).astype(NP_FP8), KD2, D)
    wo = _swz((merged(inputs["w_o"], inputs["w_o_lora_a"],
                      inputs["w_o_lora_b"]) * WS).astype(NP_FP8), KD2, D)
    wup8 = (merged(inputs["w_up"], inputs["w_up_lora_a"],
                   inputs["w_up_lora_b"]) * WS).astype(NP_FP8)
    UPW = 1024
    UPT = F // UPW
    wup = np.ascontiguousarray(
        wup8.reshape(KD2, 2, P, UPT, UPW).transpose(0, 3, 2, 1, 4)
        .reshape(KD2 * UPT, P, 2 * UPW))
    wdn = merged(inputs["w_down"], inputs["w_down_lora_a"],
                 inputs["w_down_lora_b"]).astype(NP_BF16)
    mask = np.asarray(inputs["attention_mask"], f32)

    common = {
        "wq": wq, "wk": wk, "wv": wv, "wo": wo, "wup": wup, "wdn": wdn,
        "bq": (np.asarray(inputs["b_q"], f32) * (scale * QS)).astype(f32),
        "bk": (np.asarray(inputs["b_k"], f32) * WS).astype(f32),
        "bup": np.asarray(inputs["b_up"], f32),
        "bv": np.asarray(inputs["b_v"], f32),
        "bo": np.asarray(inputs["b_o"], f32),
        "bdn": np.asarray(inputs["b_down"], f32),
        "g1": np.asarray(inputs["norm_weight_1"], f32),
        "b1": np.asarray(inputs["norm_bias_1"], f32),
        "g2": np.asarray(inputs["norm_weight_2"], f32),
        "b2": np.asarray(inputs["norm_bias_2"], f32),
    }
    in_maps = []
    for i in range(N_CORES):
        xc = x[i * TB:(i + 1) * TB].reshape(T, D)
        m = dict(common)
        m["xT8"] = _swz(np.ascontiguousarray(xc.T).astype(NP_FP8), KD2, T)
        m["xr"] = (np.ascontiguousarray(xc) * WS).astype(NP_BF16)
        m["maskT"] = np.ascontiguousarray(mask[i * TB:(i + 1) * TB, 0, 0, :])
        in_maps.append(m)
    return in_maps


def full_cfg(inputs):
    f32 = np.float32
    return {
        "D": D, "F": F, "T": T, "TB": TB, "H": H, "HD": HD,
        "has_bq": bool(np.any(np.asarray(inputs["b_q"], f32))),
        "has_bk": bool(np.any(np.asarray(inputs["b_k"], f32))),
        "has_bup": bool(np.any(np.asarray(inputs["b_up"], f32))),
        "has_mask": bool(np.any(np.asarray(inputs["attention_mask"], f32))),
        "has_bv": bool(np.any(np.asarray(inputs["b_v"], f32))),
        "has_bo": bool(np.any(np.asarray(inputs["b_o"], f32))),
        "has_bdn": bool(np.any(np.asarray(inputs["b_down"], f32))),
        "has_n1": bool(np.any(np.asarray(inputs["norm_weight_1"], f32) != 1.0)
                       or np.any(np.asarray(inputs["norm_bias_1"], f32))),
        "has_n2": bool(np.any(np.asarray(inputs["norm_weight_2"], f32) != 1.0)
                       or np.any(np.asarray(inputs["norm_bias_2"], f32))),
    }


def run_on_hw(inputs, trace=False, tmpdir=None):
    cfg = full_cfg(inputs)
    cfg_key = tuple(sorted((k, v) for k, v in cfg.items()
                           if not isinstance(v, set)))
    nc = _get_program(cfg_key, cfg)
    in_maps = make_in_maps(inputs)
    kw = {}
    if trace:
        kw = {"trace": True, "tmpdir": tmpdir}
    res = run_bass_kernel_spmd(nc, in_maps, core_ids=list(range(N_CORES)),
                               **kw)
    out = np.empty((B, S, D), np.float32)
    for i in range(N_CORES):
        out[i * TB:(i + 1) * TB] = res.results[i]["out"].reshape(TB, S, D)
    return out, res


def kernel(**inputs):
    out, _ = run_on_hw(inputs)
    return out


# revision 19
# speedup vs baseline: 1.5110x; 1.0490x over previous
"""Fused RoBERTa layer (attention + FFN, LoRA merged) on 8 Trainium2 cores.

Sharding: pure data-parallel over batch (16 batches -> 2 per core), no
collectives. LoRA merged into base weights on host; 1/sqrt(hd) folded into
w_q.

fp8 strategy (2x PE throughput via DoubleRow double-pumping):
  - QKV / AV / O-proj / FFN-up matmuls run fp8e4m3 with
    MatmulPerfMode.DoubleRow: both operands hold TWO 128-deep K-chunks side
    by side in the free dim ([128, 2, M]), contracting 256 per pass.
  - FFN-down stays bf16 (fp8 there breaks the accuracy gate).
  - Weights pre-scaled by 2^7 (2^10 for w_q) into fp8 normal range; inverse
    scales folded into exp input scale (2^-17), gelu input scale (2^-7),
    V-evict scale (2^-7), and a host 2^7 pre-scale of the bf16 residual x
    (LN1 is scale-invariant).
  - exp folds a 2^-9 output scale via its bias so unnormalized fp8
    attention weights stay in range; the ones-column denominator is the sum
    of the SAME fp8 weights so the scale cancels exactly.

Attention normalization is BATCHED: the AV matmul emits unnormalized o
rows plus a denominator row per (b,h) (V' ones-column; for odd heads the
ones column comes FIRST and the AV output is written at partition offset
63 so o rows land on partitions 64..127 -- this keeps every evict
same-partition and vector-legal). Unnormalized o is evicted to bf16 oTu;
denominators collect into a [H, T] tile; per batch one reciprocal + one
DMA out + 8 broadcast DMAs back + 8 vector muls produce fp8 oT. The
post-pass of batch b is interleaved into batch b+1's attention (or the
O-projection) so its DMA latency is hidden.

PSUM: one 8-bank pool with manual tags: mm0/mm1 (projection/FFN double
buffer), sc0..3 (scores 4-deep, reused by the LN1 transposes), pv0/pv1
(AV). Host pre-swizzles all fp8 tensors into per-tile [128, W] contiguous
layout so every weight DMA is linear in DRAM.
"""

import math
import sys

sys.path.insert(0, "/opt/trn_rl_repo")

import numpy as np
import ml_dtypes

import concourse.bacc as bacc
import concourse.bass as bass
import concourse.tile as tile
from concourse import mybir
from concourse.bass_utils import run_bass_kernel_spmd
from concourse.masks import make_identity

BF16 = mybir.dt.bfloat16
FP8 = mybir.dt.float8e4
F32 = mybir.dt.float32
NP_BF16 = np.dtype(ml_dtypes.bfloat16)
NP_FP8 = np.dtype(ml_dtypes.float8_e4m3)

B, S, D, H, HD, F = 16, 512, 1024, 16, 64, 4096
N_CORES = 8
TB = B // N_CORES
T = TB * S

MM_N = 512
P = 128

WSHIFT = 7
WS = float(2.0 ** WSHIFT)
QSHIFT = 10
QS = float(2.0 ** QSHIFT)
EXP_SCALE = float(2.0 ** (-(WSHIFT + QSHIFT)))
ATT_BIAS = -9 * math.log(2.0)


def _ceil_div(a, b):
    return (a + b - 1) // b


def build_program(cfg):
    D_, F_, T_, TB_, H_, HD_ = (cfg["D"], cfg["F"], cfg["T"], cfg["TB"],
                                cfg["H"], cfg["HD"])
    S_ = T_ // TB_
    KD = D_ // P
    KD2 = KD // 2
    KF = F_ // P
    TCH = T_ // P
    NT = _ceil_div(T_, MM_N)
    NTW = min(MM_N, T_)
    ND = _ceil_div(D_, MM_N)
    NDW = min(MM_N, D_)
    SKC = S_ // P
    SKC2 = SKC // 2
    HPC = P // HD_             # heads per 128-partition chunk (=2)
    VW = HD_ + 1               # V' per-head width (ones column)
    VROW = H_ * VW             # V' row width for one key chunk
    UPW = 1024
    UPT = F_ // UPW

    nc = bacc.Bacc("TRN2", target_bir_lowering=False, debug=False,
                   num_devices=N_CORES)

    # ---- DRAM I/O (fp8 tensors pre-swizzled on host: [ntile, 128, W]) ----
    xT8_d = nc.dram_tensor("xT8", [KD2, P, 2 * T_], FP8,
                           kind="ExternalInput")
    xr_d = nc.dram_tensor("xr", [T_, D_], BF16, kind="ExternalInput")
    wq_d = nc.dram_tensor("wq", [KD2, P, 2 * D_], FP8, kind="ExternalInput")
    wk_d = nc.dram_tensor("wk", [KD2, P, 2 * D_], FP8, kind="ExternalInput")
    wv_d = nc.dram_tensor("wv", [KD2, P, 2 * D_], FP8, kind="ExternalInput")
    wo_d = nc.dram_tensor("wo", [KD2, P, 2 * D_], FP8, kind="ExternalInput")
    wup_d = nc.dram_tensor("wup", [KD2 * UPT, P, 2 * UPW], FP8,
                           kind="ExternalInput")
    wdn_d = nc.dram_tensor("wdn", [F_, D_], BF16, kind="ExternalInput")
    bq_d = nc.dram_tensor("bq", [D_], F32, kind="ExternalInput")
    bk_d = nc.dram_tensor("bk", [D_], F32, kind="ExternalInput")
    bup_d = nc.dram_tensor("bup", [F_], F32, kind="ExternalInput")
    mask_d = nc.dram_tensor("maskT", [TB_, S_], F32, kind="ExternalInput")
    bv_d = nc.dram_tensor("bv", [D_], F32, kind="ExternalInput")
    bo_d = nc.dram_tensor("bo", [D_], F32, kind="ExternalInput")
    bdn_d = nc.dram_tensor("bdn", [D_], F32, kind="ExternalInput")
    g1_d = nc.dram_tensor("g1", [D_], F32, kind="ExternalInput")
    b1_d = nc.dram_tensor("b1", [D_], F32, kind="ExternalInput")
    g2_d = nc.dram_tensor("g2", [D_], F32, kind="ExternalInput")
    b2_d = nc.dram_tensor("b2", [D_], F32, kind="ExternalInput")
    out_d = nc.dram_tensor("out", [T_, D_], F32, kind="ExternalOutput")

    DR = mybir.MatmulPerfMode.DoubleRow

    with tile.TileContext(nc) as tc, \
         tc.tile_pool(name="consts", bufs=1) as consts, \
         tc.tile_pool(name="slab", bufs=1) as slab, \
         tc.tile_pool(name="pall", bufs=1, space="PSUM") as pall, \
         tc.tile_pool(name="work", bufs=2) as work, \
         tc.tile_pool(name="xrp", bufs=2) as xrp, \
         tc.tile_pool(name="attnp", bufs=6) as attnp, \
         tc.tile_pool(name="attn2", bufs=4) as attn2, \
         tc.tile_pool(name="rbp", bufs=1) as rbp, \
         tc.tile_pool(name="statp", bufs=4) as statp, \
         tc.tile_pool(name="outp", bufs=2) as outp, \
         tc.tile_pool(name="dramp", bufs=2, space="DRAM") as dramp:

        dma = nc.sync          # bulk loads
        dma2 = nc.gpsimd       # latency-bound small DMAs + output

        def slot(tag, width, dtype):
            return slab.tile([P, width], dtype, tag=tag, name=f"t_{tag}")

        def pair(ap_2d, i2):
            return ap_2d.rearrange("p (i w) -> p i w", i=2) if i2 is None \
                else ap_2d.rearrange("p (i w) -> p i w", i=2)[:, :, i2]

        mm_ctr = [0]

        def mm_tile():
            mm_ctr[0] ^= 1
            return pall.tile([P, MM_N], F32, tag=f"mm{mm_ctr[0]}",
                             name="mm")

        # ---- constants ----
        eps_t = consts.tile([P, 1], F32)
        nc.vector.memset(eps_t, 1e-5)
        attb_t = consts.tile([P, 1], F32)
        nc.vector.memset(attb_t, ATT_BIAS)
        zero_t = consts.tile([P, 1], F32)
        nc.vector.memset(zero_t, 0.0)
        ident = consts.tile([P, P], BF16)
        make_identity(nc, ident)
        if cfg["has_bq"]:
            bq_sb = consts.tile([P, KD], F32)
            dma.dma_start(out=bq_sb,
                          in_=bq_d.ap().rearrange("(m p) -> p m", p=P))
        if cfg["has_bk"]:
            bk_sb = consts.tile([P, KD], F32)
            dma.dma_start(out=bk_sb,
                          in_=bk_d.ap().rearrange("(m p) -> p m", p=P))
        if cfg["has_bup"]:
            bup_sb = consts.tile([P, KF], F32)
            dma.dma_start(out=bup_sb,
                          in_=bup_d.ap().rearrange("(m p) -> p m", p=P))
        if cfg["has_mask"]:
            mask_sb = consts.tile([P, TB_ * SKC], F32)
            dma.dma_start(out=mask_sb,
                          in_=mask_d.ap().rearrange("b (kc p) -> p (b kc)",
                                                    p=P))
            mask2_sb = consts.tile([P, TB_ * SKC], F32)
            nc.vector.tensor_scalar_add(out=mask2_sb, in0=mask_sb,
                                        scalar1=ATT_BIAS)

        def bcast_row(dram_vec, n):
            t = consts.tile([P, n], F32, name=f"bc_{dram_vec.name}")
            dma.dma_start(out=t,
                          in_=dram_vec.ap().unsqueeze(0).to_broadcast([P, n]))
            return t

        bv_bc = bcast_row(bv_d, D_) if cfg["has_bv"] else None
        bo_bc = bcast_row(bo_d, D_) if cfg["has_bo"] else None
        bdn_bc = bcast_row(bdn_d, D_) if cfg["has_bdn"] else None
        g1_bc = bcast_row(g1_d, D_) if cfg["has_n1"] else None
        b1_bc = bcast_row(b1_d, D_) if cfg["has_n1"] else None
        g2_bc = bcast_row(g2_d, D_) if cfg["has_n2"] else None
        b2_bc = bcast_row(b2_d, D_) if cfg["has_n2"] else None

        # ---- load x^T and QKV weights (fp8, K-paired, linear DMAs) ----
        xT8_sb = [slot(f"xT8{c2}", 2 * T_, FP8) for c2 in range(KD2)]
        w_sb = {nm: [slot(f"w{nm}{c2}", 2 * D_, FP8) for c2 in range(KD2)]
                for nm in ("q", "k", "v")}
        dma3 = nc.scalar       # second DMA queue for the cold start
        for c2 in range(KD2):
            dma.dma_start(out=w_sb["q"][c2], in_=wq_d[c2])
            dma3.dma_start(out=xT8_sb[c2], in_=xT8_d[c2])
        for c2 in range(KD2):
            dma3.dma_start(out=w_sb["k"][c2], in_=wk_d[c2])
        for c2 in range(KD2):
            dma.dma_start(out=w_sb["v"][c2], in_=wv_d[c2])

        qT_sb = [slot(f"qT{c}", T_, BF16) for c in range(KD)]
        kTe_sb = [slot(f"kTe{c}", T_, BF16) for c in range(KD)]
        kTo_sb = [slot(f"kTo{c}", T_, BF16) for c in range(KD)]
        for c in range(KD):
            nc.gpsimd.memset(kTe_sb[c][P // 2:P, :], 0.0)
            nc.gpsimd.memset(kTo_sb[c][0:P // 2, :], 0.0)
        Vp8_sb = [slot(f"Vp{c}", 2 * VROW, FP8) for c in range(TCH // 2)]

        # ---- QKV projections (fp8 DoubleRow) ----
        # t2=1 (second batch) halves are deferred into the attention phase
        # as PE filler while the ScalarE runs softmax exps.
        HB = P // 2

        def qk_proj(nm, m, t2):
            has_b = cfg["has_bq"] if nm == "q" else cfg["has_bk"]
            bias = (bq_sb if nm == "q" else bk_sb) if has_b else None
            if True:
                if True:
                    pt = mm_tile()
                    for c2 in range(KD2):
                        nc.tensor.matmul(
                            pt[:, :NTW],
                            lhsT=pair(w_sb[nm][c2],
                                      slice(m * P, (m + 1) * P)),
                            rhs=pair(xT8_sb[c2],
                                     slice(t2 * MM_N, t2 * MM_N + NTW)),
                            start=(c2 == 0), stop=(c2 == KD2 - 1),
                            perf_mode=DR)
                    sl = slice(t2 * MM_N, t2 * MM_N + NTW)
                    if nm == "q":
                        if has_b:
                            nc.vector.tensor_scalar_add(
                                out=qT_sb[m][:, sl],
                                in0=pt[:, :NTW], scalar1=bias[:, m:m + 1])
                        else:
                            nc.vector.tensor_copy(out=qT_sb[m][:, sl],
                                                  in_=pt[:, :NTW])
                    else:
                        if has_b:
                            nc.vector.tensor_scalar_add(
                                out=kTe_sb[m][0:HB, sl],
                                in0=pt[0:HB, :NTW],
                                scalar1=bias[0:HB, m:m + 1])
                            nc.vector.tensor_scalar_add(
                                out=kTo_sb[m][HB:P, sl],
                                in0=pt[HB:P, :NTW],
                                scalar1=bias[HB:P, m:m + 1])
                        else:
                            nc.vector.tensor_copy(out=kTe_sb[m][0:HB, sl],
                                                  in_=pt[0:HB, :NTW])
                            nc.vector.tensor_copy(out=kTo_sb[m][HB:P, sl],
                                                  in_=pt[HB:P, :NTW])

        for nm in ("q", "k"):
            for m in range(KD):
                qk_proj(nm, m, 0)
        # V token-major into V' ([v(64), 1] per head; 2^-7 scale on evict)
        def v_proj_tr(tr):
            vdst = Vp8_sb[tr // 2][:, (tr % 2) * VROW:(tr % 2 + 1) * VROW]
            vd3 = vdst.rearrange("p (h c) -> p h c", c=VW)
            for n2 in range(ND):
                pt = mm_tile()
                for c2 in range(KD2):
                    nc.tensor.matmul(
                        pt[:, :NDW],
                        lhsT=pair(xT8_sb[c2], slice(tr * P, (tr + 1) * P)),
                        rhs=pair(w_sb["v"][c2],
                                 slice(n2 * MM_N, n2 * MM_N + NDW)),
                        start=(c2 == 0), stop=(c2 == KD2 - 1),
                        perf_mode=DR)
                hpn = NDW // HD_   # heads per N tile
                src = pt[:, :NDW].rearrange("p (h c) -> p h c", c=HD_)
                if cfg["has_bv"]:
                    tmp = work.tile([P, NDW], F32, tag="vtmp", name="vtmp")
                    nc.vector.tensor_add(
                        out=tmp, in0=pt[:, :NDW],
                        in1=bv_bc[:, n2 * MM_N:n2 * MM_N + NDW])
                    src = tmp.rearrange("p (h c) -> p h c", c=HD_)
                nc.vector.tensor_scalar_mul(
                    out=vd3[:, n2 * hpn:(n2 + 1) * hpn, 0:HD_], in0=src,
                    scalar1=1.0 / WS)
            nc.vector.memset(vd3[:, :, HD_:VW], 1.0)  # ones cols

        for tr in range(TCH // 2):   # b0 chunks now; rest inside attention
            v_proj_tr(tr)

        skip = cfg.get("skip", set())
        # ---- attention (+ interleaved V-proj b1-chunks / O-proj b0-chunks,
        #      which keep the PE fed while the ScalarE runs the exps) ----
        wo_sb = []

        def load_wo():
            # xT8 tags are dead only once every V-proj chunk has run
            for c2 in range(KD2):
                t = slot(f"xT8{c2}", 2 * D_, FP8)
                dma.dma_start(out=t, in_=wo_d[c2])
                wo_sb.append(t)
        oT8_sb = [slot(f"wq{c2}", 2 * T_, FP8) for c2 in range(KD2)]
        oTu_sb = [slot(f"oTu{hc}", T_, BF16) for hc in range(KD)]
        den_d = dramp.tile([H_, T_], F32, tag="den_d", name="den_d")
        rb_sb = {}

        def attn_scores(b, h):
            hc, par = h // HPC, h % HPC
            at_tiles = [attnp.tile([P, 2 * S_], FP8, tag="attnT",
                                   name="attnT") for _ in range(SKC2)]
            kTm = kTe_sb if par == 0 else kTo_sb
            for kc in range(SKC):
                pt = pall.tile([P, MM_N], F32, tag=f"sc{kc}", name="ps_s")
                nc.tensor.matmul(
                    pt[:, :S_],
                    lhsT=kTm[hc][:, b * S_ + kc * P:b * S_ + (kc + 1) * P],
                    rhs=qT_sb[hc][:, b * S_:(b + 1) * S_],
                    start=True, stop=True)
                bias = (mask2_sb[:, b * SKC + kc:b * SKC + kc + 1]
                        if cfg["has_mask"] else attb_t)
                nc.scalar.activation(
                    out=at_tiles[kc // 2][:, (kc % 2) * S_:(kc % 2 + 1) * S_],
                    in_=pt[:, :S_],
                    func=mybir.ActivationFunctionType.Exp,
                    bias=bias, scale=EXP_SCALE)
            return at_tiles

        def attn_av(b, h, at_tiles):
            hc, par = h // HPC, h % HPC
            pv = pall.tile([P, MM_N], F32, tag=f"pv{h % 2}", name="ps_v2")
            for kc2 in range(SKC2):
                nc.tensor.matmul(
                    pv[0:VW, :S_],
                    lhsT=pair(Vp8_sb[b * SKC2 + kc2],
                              slice(h * VW, (h + 1) * VW)),
                    rhs=pair(at_tiles[kc2], None),
                    start=(kc2 == 0), stop=(kc2 == SKC2 - 1),
                    perf_mode=DR)
            ho = par * HD_
            nc.vector.tensor_copy(
                out=oTu_sb[hc][ho:ho + HD_, b * S_:(b + 1) * S_],
                in_=pv[0:HD_, :S_])
            rs = attn2.tile([1, S_], F32, tag="rs", name="rs")
            nc.scalar.copy(out=rs, in_=pv[HD_:VW, :S_])
            dma2.dma_start(out=den_d[h:h + 1, b * S_:(b + 1) * S_], in_=rs)

        def rb_load(b, hc):
            # broadcast this chunk's denominators back + reciprocal
            sl = slice(b * S_, (b + 1) * S_)
            rb = rbp.tile([P, S_], F32, tag=f"rb{hc}", name="rb")
            rb_sb[hc] = rb
            for h2 in range(HPC):
                dma2.dma_start(
                    out=rb[h2 * HD_:(h2 + 1) * HD_, :],
                    in_=den_d[2 * hc + h2:2 * hc + h2 + 1, sl]
                    .to_broadcast([HD_, S_]))
            nc.vector.reciprocal_approx_fast(out=rb, in_=rb)

        def post_b(b):
            # normalize: oT8 = oTu * rb (same partitions, vector-legal)
            sl = slice(b * S_, (b + 1) * S_)
            for hc in range(KD):
                nc.vector.tensor_mul(
                    out=oT8_sb[hc // 2][:, (hc % 2) * T_ + b * S_:
                                        (hc % 2) * T_ + (b + 1) * S_],
                    in0=oTu_sb[hc][:, sl], in1=rb_sb[hc])

        # O-proj machinery (defined early so tr 0..3 interleave into b1)
        xm_bf = {}
        xmT8_sb = [slot(f"wk{c2}", 2 * T_, FP8) for c2 in range(KD2)]

        def layer_norm(src, dst, g_bc, b_bc):
            bw = min(512, D_)
            nsub = _ceil_div(D_, bw)
            st = statp.tile([P, nsub, 6], F32, tag="bnst", name="bnst")
            for i in range(nsub):
                nc.vector.bn_stats(out=st[:, i, :],
                                   in_=src[:, i * bw:(i + 1) * bw])
            mv = statp.tile([P, 2], F32, tag="bnmv", name="bnmv")
            nc.vector.bn_aggr(out=mv, in_=st)
            rstd = statp.tile([P, 1], F32, tag="rstd", name="rstd")
            nc.scalar.activation(out=rstd, in_=mv[:, 1:2],
                                 func=mybir.ActivationFunctionType.Sqrt,
                                 bias=eps_t, scale=1.0)
            nc.vector.reciprocal(out=rstd, in_=rstd)
            if g_bc is None:
                nc.vector.tensor_scalar(
                    out=dst, in0=src, scalar1=mv[:, 0:1], scalar2=rstd,
                    op0=mybir.AluOpType.subtract, op1=mybir.AluOpType.mult)
            else:
                tmp = statp.tile([P, D_], F32, tag="lntmp", name="lntmp")
                nc.vector.tensor_scalar(
                    out=tmp, in0=src, scalar1=mv[:, 0:1], scalar2=rstd,
                    op0=mybir.AluOpType.subtract, op1=mybir.AluOpType.mult)
                nc.vector.tensor_mul(out=tmp, in0=tmp, in1=g_bc)
                nc.vector.tensor_add(out=dst, in0=tmp, in1=b_bc)

        def o_proj_tr(tr):
            xt = xrp.tile([P, D_], BF16, tag="xrt", name="xrt")
            dma.dma_start(out=xt, in_=xr_d[tr * P:(tr + 1) * P, :])
            of = work.tile([P, D_], F32, tag="acc", name="of")
            for n2 in range(ND):
                pt = mm_tile()
                for c2 in range(KD2):
                    nc.tensor.matmul(
                        pt[:, :NDW],
                        lhsT=pair(oT8_sb[c2], slice(tr * P, (tr + 1) * P)),
                        rhs=pair(wo_sb[c2],
                                 slice(n2 * MM_N, n2 * MM_N + NDW)),
                        start=(c2 == 0), stop=(c2 == KD2 - 1),
                        perf_mode=DR)
                nc.vector.tensor_add(out=of[:, n2 * MM_N:n2 * MM_N + NDW],
                                     in0=pt[:, :NDW],
                                     in1=xt[:, n2 * MM_N:n2 * MM_N + NDW])
                if cfg["has_bo"]:
                    nc.vector.tensor_add(
                        out=of[:, n2 * MM_N:n2 * MM_N + NDW],
                        in0=of[:, n2 * MM_N:n2 * MM_N + NDW],
                        in1=bo_bc[:, n2 * MM_N:n2 * MM_N + NDW])
            xm = slot(f"qT{tr}", D_, BF16)   # reuse qT slot (scores done)
            xm_bf[tr] = xm
            if "ln" in skip:
                nc.vector.tensor_copy(out=xm, in_=of)
            else:
                layer_norm(of, xm,
                           g1_bc if cfg["has_n1"] else None,
                           b1_bc if cfg["has_n1"] else None)

        def transpose_tr(tr):
            for c in range(KD):
                if "tr" in skip:
                    nc.vector.tensor_copy(
                        out=xmT8_sb[c // 2][:, (c % 2) * T_ + tr * P:
                                            (c % 2) * T_ + (tr + 1) * P],
                        in_=xm_bf[tr][:, c * P:(c + 1) * P])
                else:
                    pt = pall.tile([P, P], BF16, tag=f"sc{c % 4}",
                                   name="ps_t")
                    nc.tensor.transpose(pt, xm_bf[tr][:, c * P:(c + 1) * P],
                                        ident)
                    nc.vector.tensor_copy(
                        out=xmT8_sb[c // 2][:, (c % 2) * T_ + tr * P:
                                            (c % 2) * T_ + (tr + 1) * P],
                        in_=pt)

        if "attn" in skip:
            for c in range(KD):
                nc.vector.tensor_copy(
                    out=oT8_sb[c // 2][:, (c % 2) * T_:(c % 2 + 1) * T_],
                    in_=qT_sb[c])
            for tr in range(TCH // 2, TCH):
                v_proj_tr(tr)
            for m in range(KD):
                qk_proj("q", m, 1)
                qk_proj("k", m, 1)
            load_wo()
            for tr in range(TCH):
                o_proj_tr(tr)
                if tr > 0:
                    transpose_tr(tr - 1)
            transpose_tr(TCH - 1)
        else:
            # PE fillers: b0 gets the b1-token V chunks + first deferred
            # Q/K halves; b1 gets the remaining deferred Q/K halves (their
            # evicts are vector-only, so the exp stream never stalls).
            fill_b0 = {2: ("v", TCH // 2), 5: ("v", TCH // 2 + 1),
                       8: ("v", TCH // 2 + 2), 11: ("v", TCH // 2 + 3),
                       13: ("qk", 0), 14: ("qk", 1)}
            fill_b1 = {0: ("qk", 2), 2: ("qk", 3), 4: ("qk", 4),
                       6: ("qk", 5), 8: ("qk", 6), 10: ("qk", 7)}
            prev = None
            for b in range(TB_):
                fills = fill_b0 if b == 0 else fill_b1
                for h in range(H_):
                    at = attn_scores(b, h)
                    if prev is not None:
                        attn_av(*prev)
                        if prev[1] % 2 == 1:
                            rb_load(prev[0], prev[1] // 2)
                        if prev[:2] == (1, 3):
                            post_b(0)
                    prev = (b, h, at)
                    if h in fills:
                        kind, arg = fills[h]
                        if kind == "v":
                            v_proj_tr(arg)
                        else:
                            qk_proj("q", arg, 1)
                            qk_proj("k", arg, 1)
                    if b == 1 and h == 10:
                        load_wo()
                if b == 0:
                    attn_av(*prev)
                    rb_load(0, KD - 1)
                    prev = None
            attn_av(*prev)
            rb_load(1, KD - 1)
            # O-proj for b0 token chunks (oT8 b0 half ready via post_b(0));
            # LN1 sqrts now run after the last exp -- no act-table churn.
            for tr in range(TCH // 2):
                o_proj_tr(tr)
                if tr > 0:
                    transpose_tr(tr - 1)
            post_b(1)
            transpose_tr(TCH // 2 - 1)
            for tr in range(TCH // 2, TCH):
                o_proj_tr(tr)
                transpose_tr(tr - 1)
            transpose_tr(TCH - 1)

        # ---- FFN up weights (after all kTe/kTo readers; tag reuse) ----
        up_tags = [t for c in range(KD) for t in (f"kTe{c}", f"kTo{c}")]
        wup_sb = []
        for i in range(KD2 * UPT):
            t = slot(up_tags[i], 2 * UPW, FP8)
            dma.dma_start(out=t, in_=wup_d[i])
            wup_sb.append(t)

        def wup_lhsT(c2, fm):
            i = c2 * UPT + (fm * P) // UPW
            o = (fm * P) % UPW
            return pair(wup_sb[i], slice(o, o + P))

        # ---- FFN up (fp8 DR) + Gelu -> gT ----
        g_tags = ([f"g{c}" for c in range(KF - KD - KD2)]
                  + [f"wv{c2}" for c2 in range(KD2)]
                  + [f"oTu{hc}" for hc in range(KD)])
        gT_sb = [slot(g_tags[c], T_, BF16) for c in range(KF)]
        for fm in range(KF):
            for t2 in range(NT):
                pt = mm_tile()
                for c2 in range(KD2):
                    nc.tensor.matmul(
                        pt[:, :NTW],
                        lhsT=wup_lhsT(c2, fm),
                        rhs=pair(xmT8_sb[c2],
                                 slice(t2 * MM_N, t2 * MM_N + NTW)),
                        start=(c2 == 0), stop=(c2 == KD2 - 1),
                        perf_mode=DR)
                nc.scalar.activation(
                    out=gT_sb[fm][:, t2 * MM_N:t2 * MM_N + NTW],
                    in_=pt[:, :NTW],
                    func=mybir.ActivationFunctionType.Gelu,
                    bias=(bup_sb[:, fm:fm + 1] if cfg["has_bup"]
                          else zero_t),
                    scale=1.0 / WS)

        # ---- FFN down (bf16) + residual + LN2 -> out ----
        dn_tags = ([f"Vp{c}" for c in range(TCH // 2)]
                   + [f"xT8{c2}" for c2 in range(KD2)]
                   + [f"wq{c2}" for c2 in range(KD2)]
                   + up_tags
                   + [f"wk{c2}" for c2 in range(KD2)])
        wdn_sb = []
        for fc in range(KF):
            t = slot(dn_tags[fc], D_, BF16)
            dma.dma_start(out=t, in_=wdn_d[fc * P:(fc + 1) * P, :])
            wdn_sb.append(t)
        for tr in range(TCH):
            dsb = work.tile([P, D_], F32, tag="acc", name="dsb")
            for n2 in range(ND):
                pt = mm_tile()
                for fc in range(KF):
                    nc.tensor.matmul(
                        pt[:, :NDW],
                        lhsT=gT_sb[fc][:, tr * P:(tr + 1) * P],
                        rhs=wdn_sb[fc][:, n2 * MM_N:n2 * MM_N + NDW],
                        start=(fc == 0), stop=(fc == KF - 1))
                nc.vector.tensor_add(
                    out=dsb[:, n2 * MM_N:n2 * MM_N + NDW],
                    in0=pt[:, :NDW],
                    in1=xm_bf[tr][:, n2 * MM_N:n2 * MM_N + NDW])
                if cfg["has_bdn"]:
                    nc.vector.tensor_add(
                        out=dsb[:, n2 * MM_N:n2 * MM_N + NDW],
                        in0=dsb[:, n2 * MM_N:n2 * MM_N + NDW],
                        in1=bdn_bc[:, n2 * MM_N:n2 * MM_N + NDW])
            ot = outp.tile([P, D_], F32, tag="ot", name="ot")
            if "ln" in skip:
                nc.vector.tensor_copy(out=ot, in_=dsb)
            else:
                layer_norm(dsb, ot,
                           g2_bc if cfg["has_n2"] else None,
                           b2_bc if cfg["has_n2"] else None)
            hw = D_ // 2
            dma.dma_start(out=out_d[tr * P:(tr + 1) * P, 0:hw],
                          in_=ot[:, 0:hw])
            dma2.dma_start(out=out_d[tr * P:(tr + 1) * P, hw:D_],
                           in_=ot[:, hw:D_])

    nc.finalize()
    return nc


_PROGRAM_CACHE = {}


def _get_program(cfg_key, cfg):
    if cfg_key not in _PROGRAM_CACHE:
        _PROGRAM_CACHE[cfg_key] = build_program(cfg)
    return _PROGRAM_CACHE[cfg_key]


def _swz(w, npairs, width):
    """[rows, cols] -> [npairs, 128, 2*cols] K-paired contiguous."""
    return np.ascontiguousarray(
        w.reshape(npairs, 2, P, width).transpose(0, 2, 1, 3)
        .reshape(npairs, P, 2 * width))


def make_in_maps(inputs):
    f32 = np.float32
    x = np.asarray(inputs["x"], f32)
    scale = 1.0 / np.sqrt(float(inputs["head_dim"]))

    def merged(w, a, b):
        return (np.asarray(w, f32)
                + np.asarray(a, f32) @ np.asarray(b, f32))

    KD2 = D // P // 2
    wq = _swz((merged(inputs["w_q"], inputs["w_q_lora_a"],
                      inputs["w_q_lora_b"]) * (scale * QS)).astype(NP_FP8),
              KD2, D)
    wk = _swz((merged(inputs["w_k"], inputs["w_k_lora_a"],
                      inputs["w_k_lora_b"]) * WS).astype(NP_FP8), KD2, D)
    wv = _swz((merged(inputs["w_v"], inputs["w_v_lora_a"],
                      inputs["w_v_lora_b"]) * WS).astype(NP_FP8), KD2, D)
    wo = _swz((merged(inputs["w_o"], inputs["w_o_lora_a"],
                      inputs["w_o_lora_b"]) * WS).astype(NP_FP8), KD2, D)
    wup8 = (merged(inputs["w_up"], inputs["w_up_lora_a"],
                   inputs["w_up_lora_b"]) * WS).astype(NP_FP8)
    UPW = 1024
    UPT = F // UPW
    wup = np.ascontiguousarray(
        wup8.reshape(KD2, 2, P, UPT, UPW).transpose(0, 3, 2, 1, 4)
        .reshape(KD2 * UPT, P, 2 * UPW))
    wdn = merged(inputs["w_down"], inputs["w_down_lora_a"],
                 inputs["w_down_lora_b"]).astype(NP_BF16)
    mask = np.asarray(inputs["attention_mask"], f32)

    common = {
        "wq": wq, "wk": wk, "wv": wv, "wo": wo, "wup": wup, "wdn": wdn,
        "bq": (np.asarray(inputs["b_q"], f32) * (scale * QS)).astype(f32),
        "bk": (np.asarray(inputs["b_k"], f32) * WS).astype(f32),
        "bup": np.asarray(inputs["b_up"], f32),
        "bv": np.asarray(inputs["b_v"], f32),
        "bo": np.asarray(inputs["b_o"], f32),
        "bdn": np.asarray(inputs["b_down"], f32),
        "g1": np.asarray(inputs["norm_weight_1"], f32),
        "b1": np.asarray(inputs["norm_bias_1"], f32),
        "g2": np.asarray(inputs["norm_weight_2"], f32),
        "b2": np.asarray(inputs["norm_bias_2"], f32),
    }
    in_maps = []
    for i in range(N_CORES):
        xc = x[i * TB:(i + 1) * TB].reshape(T, D)
        m = dict(common)
        m["xT8"] = _swz(np.ascontiguousarray(xc.T).astype(NP_FP8), KD2, T)
        m["xr"] = (np.ascontiguousarray(xc) * WS).astype(NP_BF16)
        m["maskT"] = np.ascontiguousarray(mask[i * TB:(i + 1) * TB, 0, 0, :])
        in_maps.append(m)
    return in_maps


def full_cfg(inputs):
    f32 = np.float32
    return {
        "D": D, "F": F, "T": T, "TB": TB, "H": H, "HD": HD,
        "has_bq": bool(np.any(np.asarray(inputs["b_q"], f32))),
        "has_bk": bool(np.any(np.asarray(inputs["b_k"], f32))),
        "has_bup": bool(np.any(np.asarray(inputs["b_up"], f32))),
        "has_mask": bool(np.any(np.asarray(inputs["attention_mask"], f32))),
        "has_bv": bool(np.any(np.asarray(inputs["b_v"], f32))),
        "has_bo": bool(np.any(np.asarray(inputs["b_o"], f32))),
        "has_bdn": bool(np.any(np.asarray(inputs["b_down"], f32))),
        "has_n1": bool(np.any(np.asarray(inputs["norm_weight_1"], f32) != 1.0)
                       or np.any(np.asarray(inputs["norm_bias_1"], f32))),
        "has_n2": bool(np.any(np.asarray(inputs["norm_weight_2"], f32) != 1.0)
                       or np.any(np.asarray(inputs["norm_bias_2"], f32))),
    }


def run_on_hw(inputs, trace=False, tmpdir=None):
    cfg = full_cfg(inputs)
    cfg_key = tuple(sorted((k, v) for k, v in cfg.items()
                           if not isinstance(v, set)))
    nc = _get_program(cfg_key, cfg)
    in_maps = make_in_maps(inputs)
    kw = {}
    if trace:
        kw = {"trace": True, "tmpdir": tmpdir}
    res = run_bass_kernel_spmd(nc, in_maps, core_ids=list(range(N_CORES)),
                               **kw)
    out = np.empty((B, S, D), np.float32)
    for i in range(N_CORES):
        out[i * TB:(i + 1) * TB] = res.results[i]["out"].reshape(TB, S, D)
    return out, res


def kernel(**inputs):
    out, _ = run_on_hw(inputs)
    return out


# revision 21
# speedup vs baseline: 1.5315x; 1.0136x over previous
"""Fused RoBERTa layer (attention + FFN, LoRA merged) on 8 Trainium2 cores.

Sharding: pure data-parallel over batch (16 batches -> 2 per core), no
collectives. LoRA merged into base weights on host; 1/sqrt(hd) folded into
w_q.

fp8 strategy (2x PE throughput via DoubleRow double-pumping):
  - QKV / AV / O-proj / FFN-up matmuls run fp8e4m3 with
    MatmulPerfMode.DoubleRow: both operands hold TWO 128-deep K-chunks side
    by side in the free dim ([128, 2, M]), contracting 256 per pass.
  - FFN-down stays bf16 (fp8 there breaks the accuracy gate).
  - Weights pre-scaled by 2^7 (2^10 for w_q) into fp8 normal range; inverse
    scales folded into exp input scale (2^-17), gelu input scale (2^-7),
    V-evict scale (2^-7), and a host 2^7 pre-scale of the bf16 residual x
    (LN1 is scale-invariant).
  - exp folds a 2^-9 output scale via its bias so unnormalized fp8
    attention weights stay in range; the ones-column denominator is the sum
    of the SAME fp8 weights so the scale cancels exactly.

Attention normalization is BATCHED: the AV matmul emits unnormalized o
rows plus a denominator row per (b,h) (V' ones-column; for odd heads the
ones column comes FIRST and the AV output is written at partition offset
63 so o rows land on partitions 64..127 -- this keeps every evict
same-partition and vector-legal). Unnormalized o is evicted to bf16 oTu;
denominators collect into a [H, T] tile; per batch one reciprocal + one
DMA out + 8 broadcast DMAs back + 8 vector muls produce fp8 oT. The
post-pass of batch b is interleaved into batch b+1's attention (or the
O-projection) so its DMA latency is hidden.

PSUM: one 8-bank pool with manual tags: mm0/mm1 (projection/FFN double
buffer), sc0..3 (scores 4-deep, reused by the LN1 transposes), pv0/pv1
(AV). Host pre-swizzles all fp8 tensors into per-tile [128, W] contiguous
layout so every weight DMA is linear in DRAM.
"""

import math
import sys

sys.path.insert(0, "/opt/trn_rl_repo")

import numpy as np
import ml_dtypes

import concourse.bacc as bacc
import concourse.bass as bass
import concourse.tile as tile
from concourse import mybir
from concourse.bass_utils import run_bass_kernel_spmd
from concourse.masks import make_identity

BF16 = mybir.dt.bfloat16
FP8 = mybir.dt.float8e4
F32 = mybir.dt.float32
NP_BF16 = np.dtype(ml_dtypes.bfloat16)
NP_FP8 = np.dtype(ml_dtypes.float8_e4m3)

B, S, D, H, HD, F = 16, 512, 1024, 16, 64, 4096
N_CORES = 8
TB = B // N_CORES
T = TB * S

MM_N = 512
P = 128

WSHIFT = 7
WS = float(2.0 ** WSHIFT)
QSHIFT = 10
QS = float(2.0 ** QSHIFT)
EXP_SCALE = float(2.0 ** (-(WSHIFT + QSHIFT)))
ATT_BIAS = -9 * math.log(2.0)


def _ceil_div(a, b):
    return (a + b - 1) // b


def build_program(cfg):
    D_, F_, T_, TB_, H_, HD_ = (cfg["D"], cfg["F"], cfg["T"], cfg["TB"],
                                cfg["H"], cfg["HD"])
    S_ = T_ // TB_
    KD = D_ // P
    KD2 = KD // 2
    KF = F_ // P
    TCH = T_ // P
    NT = _ceil_div(T_, MM_N)
    NTW = min(MM_N, T_)
    ND = _ceil_div(D_, MM_N)
    NDW = min(MM_N, D_)
    SKC = S_ // P
    SKC2 = SKC // 2
    HPC = P // HD_             # heads per 128-partition chunk (=2)
    VW = HD_ + 1               # V' per-head width (ones column)
    VROW = H_ * VW             # V' row width for one key chunk
    UPW = 1024
    UPT = F_ // UPW

    nc = bacc.Bacc("TRN2", target_bir_lowering=False, debug=False,
                   num_devices=N_CORES)

    # ---- DRAM I/O (fp8 tensors pre-swizzled on host: [ntile, 128, W]) ----
    xT8_d = nc.dram_tensor("xT8", [KD2, P, 2 * T_], FP8,
                           kind="ExternalInput")
    xr_d = nc.dram_tensor("xr", [T_, D_], BF16, kind="ExternalInput")
    wq_d = nc.dram_tensor("wq", [KD2, P, 2 * D_], FP8, kind="ExternalInput")
    wk_d = nc.dram_tensor("wk", [KD2, P, 2 * D_], FP8, kind="ExternalInput")
    wv_d = nc.dram_tensor("wv", [KD2, P, 2 * D_], FP8, kind="ExternalInput")
    wo_d = nc.dram_tensor("wo", [KD2, P, 2 * D_], FP8, kind="ExternalInput")
    wup_d = nc.dram_tensor("wup", [KD2 * UPT, P, 2 * UPW], FP8,
                           kind="ExternalInput")
    wdn_d = nc.dram_tensor("wdn", [F_, D_], BF16, kind="ExternalInput")
    bq_d = nc.dram_tensor("bq", [D_], F32, kind="ExternalInput")
    bk_d = nc.dram_tensor("bk", [D_], F32, kind="ExternalInput")
    bup_d = nc.dram_tensor("bup", [F_], F32, kind="ExternalInput")
    mask_d = nc.dram_tensor("maskT", [TB_, S_], F32, kind="ExternalInput")
    bv_d = nc.dram_tensor("bv", [D_], F32, kind="ExternalInput")
    bo_d = nc.dram_tensor("bo", [D_], F32, kind="ExternalInput")
    bdn_d = nc.dram_tensor("bdn", [D_], F32, kind="ExternalInput")
    g1_d = nc.dram_tensor("g1", [D_], F32, kind="ExternalInput")
    b1_d = nc.dram_tensor("b1", [D_], F32, kind="ExternalInput")
    g2_d = nc.dram_tensor("g2", [D_], F32, kind="ExternalInput")
    b2_d = nc.dram_tensor("b2", [D_], F32, kind="ExternalInput")
    out_d = nc.dram_tensor("out", [T_, D_], F32, kind="ExternalOutput")

    DR = mybir.MatmulPerfMode.DoubleRow

    with tile.TileContext(nc) as tc, \
         tc.tile_pool(name="consts", bufs=1) as consts, \
         tc.tile_pool(name="slab", bufs=1) as slab, \
         tc.tile_pool(name="pall", bufs=1, space="PSUM") as pall, \
         tc.tile_pool(name="work", bufs=2) as work, \
         tc.tile_pool(name="xrp", bufs=2) as xrp, \
         tc.tile_pool(name="attnp", bufs=6) as attnp, \
         tc.tile_pool(name="attn2", bufs=4) as attn2, \
         tc.tile_pool(name="rbp", bufs=1) as rbp, \
         tc.tile_pool(name="statp", bufs=4) as statp, \
         tc.tile_pool(name="outp", bufs=2) as outp, \
         tc.tile_pool(name="dramp", bufs=2, space="DRAM") as dramp:

        dma = nc.sync          # bulk loads
        dma2 = nc.gpsimd       # latency-bound small DMAs + output

        def slot(tag, width, dtype):
            return slab.tile([P, width], dtype, tag=tag, name=f"t_{tag}")

        def pair(ap_2d, i2):
            return ap_2d.rearrange("p (i w) -> p i w", i=2) if i2 is None \
                else ap_2d.rearrange("p (i w) -> p i w", i=2)[:, :, i2]

        mm_ctr = [0]

        def mm_tile():
            mm_ctr[0] ^= 1
            return pall.tile([P, MM_N], F32, tag=f"mm{mm_ctr[0]}",
                             name="mm")

        # ---- constants ----
        eps_t = consts.tile([P, 1], F32)
        nc.vector.memset(eps_t, 1e-5)
        attb_t = consts.tile([P, 1], F32)
        nc.vector.memset(attb_t, ATT_BIAS)
        zero_t = consts.tile([P, 1], F32)
        nc.vector.memset(zero_t, 0.0)
        ident = consts.tile([P, P], BF16)
        make_identity(nc, ident)
        if cfg["has_bq"]:
            bq_sb = consts.tile([P, KD], F32)
            dma.dma_start(out=bq_sb,
                          in_=bq_d.ap().rearrange("(m p) -> p m", p=P))
        if cfg["has_bk"]:
            bk_sb = consts.tile([P, KD], F32)
            dma.dma_start(out=bk_sb,
                          in_=bk_d.ap().rearrange("(m p) -> p m", p=P))
        if cfg["has_bup"]:
            bup_sb = consts.tile([P, KF], F32)
            dma.dma_start(out=bup_sb,
                          in_=bup_d.ap().rearrange("(m p) -> p m", p=P))
        if cfg["has_mask"]:
            mask_sb = consts.tile([P, TB_ * SKC], F32)
            dma.dma_start(out=mask_sb,
                          in_=mask_d.ap().rearrange("b (kc p) -> p (b kc)",
                                                    p=P))
            mask2_sb = consts.tile([P, TB_ * SKC], F32)
            nc.vector.tensor_scalar_add(out=mask2_sb, in0=mask_sb,
                                        scalar1=ATT_BIAS)

        def bcast_row(dram_vec, n):
            t = consts.tile([P, n], F32, name=f"bc_{dram_vec.name}")
            dma.dma_start(out=t,
                          in_=dram_vec.ap().unsqueeze(0).to_broadcast([P, n]))
            return t

        bv_bc = bcast_row(bv_d, D_) if cfg["has_bv"] else None
        bo_bc = bcast_row(bo_d, D_) if cfg["has_bo"] else None
        bdn_bc = bcast_row(bdn_d, D_) if cfg["has_bdn"] else None
        g1_bc = bcast_row(g1_d, D_) if cfg["has_n1"] else None
        b1_bc = bcast_row(b1_d, D_) if cfg["has_n1"] else None
        g2_bc = bcast_row(g2_d, D_) if cfg["has_n2"] else None
        b2_bc = bcast_row(b2_d, D_) if cfg["has_n2"] else None

        # ---- load x^T and QKV weights (fp8, K-paired, linear DMAs) ----
        xT8_sb = [slot(f"xT8{c2}", 2 * T_, FP8) for c2 in range(KD2)]
        w_sb = {nm: [slot(f"w{nm}{c2}", 2 * D_, FP8) for c2 in range(KD2)]
                for nm in ("q", "k", "v")}
        dma3 = nc.scalar       # second DMA queue for the cold start
        for c2 in range(KD2):
            dma.dma_start(out=w_sb["q"][c2], in_=wq_d[c2])
            dma3.dma_start(out=xT8_sb[c2], in_=xT8_d[c2])
        for c2 in range(KD2):
            dma3.dma_start(out=w_sb["k"][c2], in_=wk_d[c2])
        for c2 in range(KD2):
            dma.dma_start(out=w_sb["v"][c2], in_=wv_d[c2])

        qT_sb = [slot(f"qT{c}", T_, BF16) for c in range(KD)]
        kTe_sb = [slot(f"kTe{c}", T_, BF16) for c in range(KD)]
        kTo_sb = [slot(f"kTo{c}", T_, BF16) for c in range(KD)]
        for c in range(KD):
            nc.gpsimd.memset(kTe_sb[c][P // 2:P, :], 0.0)
            nc.gpsimd.memset(kTo_sb[c][0:P // 2, :], 0.0)
        Vp8_sb = [slot(f"Vp{c}", 2 * VROW, FP8) for c in range(TCH // 2)]

        # ---- QKV projections (fp8 DoubleRow) ----
        # t2=1 (second batch) halves are deferred into the attention phase
        # as PE filler while the ScalarE runs softmax exps.
        HB = P // 2

        def qk_proj(nm, m, t2):
            has_b = cfg["has_bq"] if nm == "q" else cfg["has_bk"]
            bias = (bq_sb if nm == "q" else bk_sb) if has_b else None
            if True:
                if True:
                    pt = mm_tile()
                    for c2 in range(KD2):
                        nc.tensor.matmul(
                            pt[:, :NTW],
                            lhsT=pair(w_sb[nm][c2],
                                      slice(m * P, (m + 1) * P)),
                            rhs=pair(xT8_sb[c2],
                                     slice(t2 * MM_N, t2 * MM_N + NTW)),
                            start=(c2 == 0), stop=(c2 == KD2 - 1),
                            perf_mode=DR)
                    sl = slice(t2 * MM_N, t2 * MM_N + NTW)
                    if nm == "q":
                        if has_b:
                            nc.vector.tensor_scalar_add(
                                out=qT_sb[m][:, sl],
                                in0=pt[:, :NTW], scalar1=bias[:, m:m + 1])
                        else:
                            nc.vector.tensor_copy(out=qT_sb[m][:, sl],
                                                  in_=pt[:, :NTW])
                    else:
                        if has_b:
                            nc.vector.tensor_scalar_add(
                                out=kTe_sb[m][0:HB, sl],
                                in0=pt[0:HB, :NTW],
                                scalar1=bias[0:HB, m:m + 1])
                            nc.vector.tensor_scalar_add(
                                out=kTo_sb[m][HB:P, sl],
                                in0=pt[HB:P, :NTW],
                                scalar1=bias[HB:P, m:m + 1])
                        else:
                            nc.vector.tensor_copy(out=kTe_sb[m][0:HB, sl],
                                                  in_=pt[0:HB, :NTW])
                            nc.vector.tensor_copy(out=kTo_sb[m][HB:P, sl],
                                                  in_=pt[HB:P, :NTW])

        for nm in ("q", "k"):
            for m in range(KD):
                qk_proj(nm, m, 0)
        # V token-major into V' ([v(64), 1] per head; 2^-7 scale on evict)
        def v_proj_tr(tr):
            vdst = Vp8_sb[tr // 2][:, (tr % 2) * VROW:(tr % 2 + 1) * VROW]
            vd3 = vdst.rearrange("p (h c) -> p h c", c=VW)
            for n2 in range(ND):
                pt = mm_tile()
                for c2 in range(KD2):
                    nc.tensor.matmul(
                        pt[:, :NDW],
                        lhsT=pair(xT8_sb[c2], slice(tr * P, (tr + 1) * P)),
                        rhs=pair(w_sb["v"][c2],
                                 slice(n2 * MM_N, n2 * MM_N + NDW)),
                        start=(c2 == 0), stop=(c2 == KD2 - 1),
                        perf_mode=DR)
                hpn = NDW // HD_   # heads per N tile
                src = pt[:, :NDW].rearrange("p (h c) -> p h c", c=HD_)
                if cfg["has_bv"]:
                    tmp = work.tile([P, NDW], F32, tag="vtmp", name="vtmp")
                    nc.vector.tensor_add(
                        out=tmp, in0=pt[:, :NDW],
                        in1=bv_bc[:, n2 * MM_N:n2 * MM_N + NDW])
                    src = tmp.rearrange("p (h c) -> p h c", c=HD_)
                nc.vector.tensor_scalar_mul(
                    out=vd3[:, n2 * hpn:(n2 + 1) * hpn, 0:HD_], in0=src,
                    scalar1=1.0 / WS)
            nc.vector.memset(vd3[:, :, HD_:VW], 1.0)  # ones cols

        for tr in range(TCH // 2):   # b0 chunks now; rest inside attention
            v_proj_tr(tr)

        skip = cfg.get("skip", set())
        # ---- attention (+ interleaved V-proj b1-chunks / O-proj b0-chunks,
        #      which keep the PE fed while the ScalarE runs the exps) ----
        wo_sb = []

        def load_wo():
            # xT8 tags are dead only once every V-proj chunk has run
            for c2 in range(KD2):
                t = slot(f"xT8{c2}", 2 * D_, FP8)
                dma.dma_start(out=t, in_=wo_d[c2])
                wo_sb.append(t)
        oT8_sb = [slot(f"wq{c2}", 2 * T_, FP8) for c2 in range(KD2)]
        oTu_sb = [slot(f"oTu{hc}", T_, BF16) for hc in range(KD)]
        den_d = dramp.tile([H_, T_], F32, tag="den_d", name="den_d")
        rb_sb = {}

        def attn_scores(b, h):
            hc, par = h // HPC, h % HPC
            at_tiles = [attnp.tile([P, 2 * S_], FP8, tag="attnT",
                                   name="attnT") for _ in range(SKC2)]
            kTm = kTe_sb if par == 0 else kTo_sb
            for kc in range(SKC):
                pt = pall.tile([P, MM_N], F32, tag=f"sc{kc}", name="ps_s")
                nc.tensor.matmul(
                    pt[:, :S_],
                    lhsT=kTm[hc][:, b * S_ + kc * P:b * S_ + (kc + 1) * P],
                    rhs=qT_sb[hc][:, b * S_:(b + 1) * S_],
                    start=True, stop=True)
                bias = (mask2_sb[:, b * SKC + kc:b * SKC + kc + 1]
                        if cfg["has_mask"] else attb_t)
                nc.scalar.activation(
                    out=at_tiles[kc // 2][:, (kc % 2) * S_:(kc % 2 + 1) * S_],
                    in_=pt[:, :S_],
                    func=mybir.ActivationFunctionType.Exp,
                    bias=bias, scale=EXP_SCALE)
            return at_tiles

        def attn_av(b, h, at_tiles):
            hc, par = h // HPC, h % HPC
            pv = pall.tile([P, MM_N], F32, tag=f"pv{h % 2}", name="ps_v2")
            for kc2 in range(SKC2):
                nc.tensor.matmul(
                    pv[0:VW, :S_],
                    lhsT=pair(Vp8_sb[b * SKC2 + kc2],
                              slice(h * VW, (h + 1) * VW)),
                    rhs=pair(at_tiles[kc2], None),
                    start=(kc2 == 0), stop=(kc2 == SKC2 - 1),
                    perf_mode=DR)
            ho = par * HD_
            nc.vector.tensor_copy(
                out=oTu_sb[hc][ho:ho + HD_, b * S_:(b + 1) * S_],
                in_=pv[0:HD_, :S_])
            rs = attn2.tile([1, S_], F32, tag="rs", name="rs")
            nc.vector.tensor_copy(out=rs, in_=pv[HD_:VW, :S_])
            dma2.dma_start(out=den_d[h:h + 1, b * S_:(b + 1) * S_], in_=rs)

        def rb_load(b, hc):
            # broadcast this chunk's denominators back + reciprocal
            sl = slice(b * S_, (b + 1) * S_)
            rb = rbp.tile([P, S_], F32, tag=f"rb{hc}", name="rb")
            rb_sb[hc] = rb
            for h2 in range(HPC):
                dma2.dma_start(
                    out=rb[h2 * HD_:(h2 + 1) * HD_, :],
                    in_=den_d[2 * hc + h2:2 * hc + h2 + 1, sl]
                    .to_broadcast([HD_, S_]))
            nc.vector.reciprocal_approx_fast(out=rb, in_=rb)

        def post_b(b):
            # normalize: oT8 = oTu * rb (same partitions, vector-legal)
            sl = slice(b * S_, (b + 1) * S_)
            for hc in range(KD):
                nc.vector.tensor_mul(
                    out=oT8_sb[hc // 2][:, (hc % 2) * T_ + b * S_:
                                        (hc % 2) * T_ + (b + 1) * S_],
                    in0=oTu_sb[hc][:, sl], in1=rb_sb[hc])

        # O-proj machinery (defined early so tr 0..3 interleave into b1)
        xm_bf = {}
        xmT8_sb = [slot(f"wk{c2}", 2 * T_, FP8) for c2 in range(KD2)]

        def layer_norm(src, dst, g_bc, b_bc):
            bw = min(512, D_)
            nsub = _ceil_div(D_, bw)
            st = statp.tile([P, nsub, 6], F32, tag="bnst", name="bnst")
            for i in range(nsub):
                nc.vector.bn_stats(out=st[:, i, :],
                                   in_=src[:, i * bw:(i + 1) * bw])
            mv = statp.tile([P, 2], F32, tag="bnmv", name="bnmv")
            nc.vector.bn_aggr(out=mv, in_=st)
            rstd = statp.tile([P, 1], F32, tag="rstd", name="rstd")
            nc.scalar.activation(out=rstd, in_=mv[:, 1:2],
                                 func=mybir.ActivationFunctionType.Sqrt,
                                 bias=eps_t, scale=1.0)
            nc.vector.reciprocal(out=rstd, in_=rstd)
            if g_bc is None:
                nc.vector.tensor_scalar(
                    out=dst, in0=src, scalar1=mv[:, 0:1], scalar2=rstd,
                    op0=mybir.AluOpType.subtract, op1=mybir.AluOpType.mult)
            else:
                tmp = statp.tile([P, D_], F32, tag="lntmp", name="lntmp")
                nc.vector.tensor_scalar(
                    out=tmp, in0=src, scalar1=mv[:, 0:1], scalar2=rstd,
                    op0=mybir.AluOpType.subtract, op1=mybir.AluOpType.mult)
                nc.vector.tensor_mul(out=tmp, in0=tmp, in1=g_bc)
                nc.vector.tensor_add(out=dst, in0=tmp, in1=b_bc)

        def o_proj_tr(tr):
            xt = xrp.tile([P, D_], BF16, tag="xrt", name="xrt")
            dma.dma_start(out=xt, in_=xr_d[tr * P:(tr + 1) * P, :])
            of = work.tile([P, D_], F32, tag="acc", name="of")
            for n2 in range(ND):
                pt = mm_tile()
                for c2 in range(KD2):
                    nc.tensor.matmul(
                        pt[:, :NDW],
                        lhsT=pair(oT8_sb[c2], slice(tr * P, (tr + 1) * P)),
                        rhs=pair(wo_sb[c2],
                                 slice(n2 * MM_N, n2 * MM_N + NDW)),
                        start=(c2 == 0), stop=(c2 == KD2 - 1),
                        perf_mode=DR)
                nc.vector.tensor_add(out=of[:, n2 * MM_N:n2 * MM_N + NDW],
                                     in0=pt[:, :NDW],
                                     in1=xt[:, n2 * MM_N:n2 * MM_N + NDW])
                if cfg["has_bo"]:
                    nc.vector.tensor_add(
                        out=of[:, n2 * MM_N:n2 * MM_N + NDW],
                        in0=of[:, n2 * MM_N:n2 * MM_N + NDW],
                        in1=bo_bc[:, n2 * MM_N:n2 * MM_N + NDW])
            xm = slot(f"qT{tr}", D_, BF16)   # reuse qT slot (scores done)
            xm_bf[tr] = xm
            if "ln" in skip:
                nc.vector.tensor_copy(out=xm, in_=of)
            else:
                layer_norm(of, xm,
                           g1_bc if cfg["has_n1"] else None,
                           b1_bc if cfg["has_n1"] else None)

        def transpose_tr(tr):
            for c in range(KD):
                if "tr" in skip:
                    nc.vector.tensor_copy(
                        out=xmT8_sb[c // 2][:, (c % 2) * T_ + tr * P:
                                            (c % 2) * T_ + (tr + 1) * P],
                        in_=xm_bf[tr][:, c * P:(c + 1) * P])
                else:
                    pt = pall.tile([P, P], BF16, tag=f"sc{c % 4}",
                                   name="ps_t")
                    nc.tensor.transpose(pt, xm_bf[tr][:, c * P:(c + 1) * P],
                                        ident)
                    nc.vector.tensor_copy(
                        out=xmT8_sb[c // 2][:, (c % 2) * T_ + tr * P:
                                            (c % 2) * T_ + (tr + 1) * P],
                        in_=pt)

        if "attn" in skip:
            for c in range(KD):
                nc.vector.tensor_copy(
                    out=oT8_sb[c // 2][:, (c % 2) * T_:(c % 2 + 1) * T_],
                    in_=qT_sb[c])
            for tr in range(TCH // 2, TCH):
                v_proj_tr(tr)
            for m in range(KD):
                qk_proj("q", m, 1)
                qk_proj("k", m, 1)
            load_wo()
            for tr in range(TCH):
                o_proj_tr(tr)
                if tr > 0:
                    transpose_tr(tr - 1)
            transpose_tr(TCH - 1)
        else:
            # PE fillers: b0 gets the b1-token V chunks + first deferred
            # Q/K halves; b1 gets the remaining deferred Q/K halves (their
            # evicts are vector-only, so the exp stream never stalls).
            fill_b0 = {2: ("v", TCH // 2), 5: ("v", TCH // 2 + 1),
                       8: ("v", TCH // 2 + 2), 11: ("v", TCH // 2 + 3),
                       13: ("qk", 0), 14: ("qk", 1)}
            fill_b1 = {0: ("qk", 2), 2: ("qk", 3), 4: ("qk", 4),
                       6: ("qk", 5), 8: ("qk", 6), 10: ("qk", 7)}
            prev = None
            for b in range(TB_):
                fills = fill_b0 if b == 0 else fill_b1
                for h in range(H_):
                    at = attn_scores(b, h)
                    if prev is not None:
                        attn_av(*prev)
                        if prev[1] % 2 == 1:
                            rb_load(prev[0], prev[1] // 2)
                        if prev[:2] == (1, 3):
                            post_b(0)
                    prev = (b, h, at)
                    if h in fills:
                        kind, arg = fills[h]
                        if kind == "v":
                            v_proj_tr(arg)
                        else:
                            qk_proj("q", arg, 1)
                            qk_proj("k", arg, 1)
                    if b == 1 and h == 10:
                        load_wo()
                if b == 0:
                    attn_av(*prev)
                    rb_load(0, KD - 1)
                    prev = None
            attn_av(*prev)
            rb_load(1, KD - 1)
            # O-proj for b0 token chunks (oT8 b0 half ready via post_b(0));
            # LN1 sqrts now run after the last exp -- no act-table churn.
            for tr in range(TCH // 2):
                o_proj_tr(tr)
                if tr > 0:
                    transpose_tr(tr - 1)
            post_b(1)
            transpose_tr(TCH // 2 - 1)
            for tr in range(TCH // 2, TCH):
                o_proj_tr(tr)
                transpose_tr(tr - 1)
            transpose_tr(TCH - 1)

        # ---- FFN up weights (after all kTe/kTo readers; tag reuse) ----
        up_tags = [t for c in range(KD) for t in (f"kTe{c}", f"kTo{c}")]
        wup_sb = []
        for i in range(KD2 * UPT):
            t = slot(up_tags[i], 2 * UPW, FP8)
            dma.dma_start(out=t, in_=wup_d[i])
            wup_sb.append(t)

        def wup_lhsT(c2, fm):
            i = c2 * UPT + (fm * P) // UPW
            o = (fm * P) % UPW
            return pair(wup_sb[i], slice(o, o + P))

        # ---- FFN up (fp8 DR) + Gelu -> gT ----
        g_tags = ([f"g{c}" for c in range(KF - KD - KD2)]
                  + [f"wv{c2}" for c2 in range(KD2)]
                  + [f"oTu{hc}" for hc in range(KD)])
        gT_sb = [slot(g_tags[c], T_, BF16) for c in range(KF)]
        for fm in range(KF):
            for t2 in range(NT):
                pt = mm_tile()
                for c2 in range(KD2):
                    nc.tensor.matmul(
                        pt[:, :NTW],
                        lhsT=wup_lhsT(c2, fm),
                        rhs=pair(xmT8_sb[c2],
                                 slice(t2 * MM_N, t2 * MM_N + NTW)),
                        start=(c2 == 0), stop=(c2 == KD2 - 1),
                        perf_mode=DR)
                nc.scalar.activation(
                    out=gT_sb[fm][:, t2 * MM_N:t2 * MM_N + NTW],
                    in_=pt[:, :NTW],
                    func=mybir.ActivationFunctionType.Gelu,
                    bias=(bup_sb[:, fm:fm + 1] if cfg["has_bup"]
                          else zero_t),
                    scale=1.0 / WS)

        # ---- FFN down (bf16) + residual + LN2 -> out ----
        dn_tags = ([f"Vp{c}" for c in range(TCH // 2)]
                   + [f"xT8{c2}" for c2 in range(KD2)]
                   + [f"wq{c2}" for c2 in range(KD2)]
                   + up_tags
                   + [f"wk{c2}" for c2 in range(KD2)])
        wdn_sb = []
        for fc in range(KF):
            t = slot(dn_tags[fc], D_, BF16)
            dma.dma_start(out=t, in_=wdn_d[fc * P:(fc + 1) * P, :])
            wdn_sb.append(t)
        for tr in range(TCH):
            dsb = work.tile([P, D_], F32, tag="acc", name="dsb")
            for n2 in range(ND):
                pt = mm_tile()
                for fc in range(KF):
                    nc.tensor.matmul(
                        pt[:, :NDW],
                        lhsT=gT_sb[fc][:, tr * P:(tr + 1) * P],
                        rhs=wdn_sb[fc][:, n2 * MM_N:n2 * MM_N + NDW],
                        start=(fc == 0), stop=(fc == KF - 1))
                nc.vector.tensor_add(
                    out=dsb[:, n2 * MM_N:n2 * MM_N + NDW],
                    in0=pt[:, :NDW],
                    in1=xm_bf[tr][:, n2 * MM_N:n2 * MM_N + NDW])
                if cfg["has_bdn"]:
                    nc.vector.tensor_add(
                        out=dsb[:, n2 * MM_N:n2 * MM_N + NDW],
                        in0=dsb[:, n2 * MM_N:n2 * MM_N + NDW],
                        in1=bdn_bc[:, n2 * MM_N:n2 * MM_N + NDW])
            ot = outp.tile([P, D_], F32, tag="ot", name="ot")
            if "ln" in skip:
                nc.vector.tensor_copy(out=ot, in_=dsb)
            else:
                layer_norm(dsb, ot,
                           g2_bc if cfg["has_n2"] else None,
                           b2_bc if cfg["has_n2"] else None)
            if tr < TCH - 1:
                hw = D_ // 2
                dma.dma_start(out=out_d[tr * P:(tr + 1) * P, 0:hw],
                              in_=ot[:, 0:hw])
                dma2.dma_start(out=out_d[tr * P:(tr + 1) * P, hw:D_],
                               in_=ot[:, hw:D_])
            else:
                # last chunk is latency-exposed: split across all queues
                qw = D_ // 4
                engs = (nc.sync, nc.gpsimd, nc.scalar, nc.sync)
                for qi, eng in enumerate(engs):
                    eng.dma_start(
                        out=out_d[tr * P:(tr + 1) * P,
                                  qi * qw:(qi + 1) * qw],
                        in_=ot[:, qi * qw:(qi + 1) * qw])

    nc.finalize()
    return nc


_PROGRAM_CACHE = {}


def _get_program(cfg_key, cfg):
    if cfg_key not in _PROGRAM_CACHE:
        _PROGRAM_CACHE[cfg_key] = build_program(cfg)
    return _PROGRAM_CACHE[cfg_key]


def _swz(w, npairs, width):
    """[rows, cols] -> [npairs, 128, 2*cols] K-paired contiguous."""
    return np.ascontiguousarray(
        w.reshape(npairs, 2, P, width).transpose(0, 2, 1, 3)
        .reshape(npairs, P, 2 * width))


def make_in_maps(inputs):
    f32 = np.float32
    x = np.asarray(inputs["x"], f32)
    scale = 1.0 / np.sqrt(float(inputs["head_dim"]))

    def merged(w, a, b):
        return (np.asarray(w, f32)
                + np.asarray(a, f32) @ np.asarray(b, f32))

    KD2 = D // P // 2
    wq = _swz((merged(inputs["w_q"], inputs["w_q_lora_a"],
                      inputs["w_q_lora_b"]) * (scale * QS)).astype(NP_FP8),
              KD2, D)
    wk = _swz((merged(inputs["w_k"], inputs["w_k_lora_a"],
                      inputs["w_k_lora_b"]) * WS).astype(NP_FP8), KD2, D)
    wv = _swz((merged(inputs["w_v"], inputs["w_v_lora_a"],
                      inputs["w_v_lora_b"]) * WS).astype(NP_FP8), KD2, D)
    wo = _swz((merged(inputs["w_o"], inputs["w_o_lora_a"],
                      inputs["w_o_lora_b"]) * WS).astype(NP_FP8), KD2, D)
    wup8 = (merged(inputs["w_up"], inputs["w_up_lora_a"],
                   inputs["w_up_lora_b"]) * WS).astype(NP_FP8)
    UPW = 1024
    UPT = F // UPW
    wup = np.ascontiguousarray(
        wup8.reshape(KD2, 2, P, UPT, UPW).transpose(0, 3, 2, 1, 4)
        .reshape(KD2 * UPT, P, 2 * UPW))
    wdn = merged(inputs["w_down"], inputs["w_down_lora_a"],
                 inputs["w_down_lora_b"]).astype(NP_BF16)
    mask = np.asarray(inputs["attention_mask"], f32)

    common = {
        "wq": wq, "wk": wk, "wv": wv, "wo": wo, "wup": wup, "wdn": wdn,
        "bq": (np.asarray(inputs["b_q"], f32) * (scale * QS)).astype(f32),
        "bk": (np.asarray(inputs["b_k"], f32) * WS).astype(f32),
        "bup": np.asarray(inputs["b_up"], f32),
        "bv": np.asarray(inputs["b_v"], f32),
        "bo": np.asarray(inputs["b_o"], f32),
        "bdn": np.asarray(inputs["b_down"], f32),
        "g1": np.asarray(inputs["norm_weight_1"], f32),
        "b1": np.asarray(inputs["norm_bias_1"], f32),
        "g2": np.asarray(inputs["norm_weight_2"], f32),
        "b2": np.asarray(inputs["norm_bias_2"], f32),
    }
    in_maps = []
    for i in range(N_CORES):
        xc = x[i * TB:(i + 1) * TB].reshape(T, D)
        m = dict(common)
        m["xT8"] = _swz(np.ascontiguousarray(xc.T).astype(NP_FP8), KD2, T)
        m["xr"] = (np.ascontiguousarray(xc) * WS).astype(NP_BF16)
        m["maskT"] = np.ascontiguousarray(mask[i * TB:(i + 1) * TB, 0, 0, :])
        in_maps.append(m)
    return in_maps


def full_cfg(inputs):
    f32 = np.float32
    return {
        "D": D, "F": F, "T": T, "TB": TB, "H": H, "HD": HD,
        "has_bq": bool(np.any(np.asarray(inputs["b_q"], f32))),
        "has_bk": bool(np.any(np.asarray(inputs["b_k"], f32))),
        "has_bup": bool(np.any(np.asarray(inputs["b_up"], f32))),
        "has_mask": bool(np.any(np.asarray(inputs["attention_mask"], f32))),
        "has_bv": bool(np.any(np.asarray(inputs["b_v"], f32))),
        "has_bo": bool(np.any(np.asarray(inputs["b_o"], f32))),
        "has_bdn": bool(np.any(np.asarray(inputs["b_down"], f32))),
        "has_n1": bool(np.any(np.asarray(inputs["norm_weight_1"], f32) != 1.0)
                       or np.any(np.asarray(inputs["norm_bias_1"], f32))),
        "has_n2": bool(np.any(np.asarray(inputs["norm_weight_2"], f32) != 1.0)
                       or np.any(np.asarray(inputs["norm_bias_2"], f32))),
    }


def run_on_hw(inputs, trace=False, tmpdir=None):
    cfg = full_cfg(inputs)
    cfg_key = tuple(sorted((k, v) for k, v in cfg.items()
                           if not isinstance(v, set)))
    nc = _get_program(cfg_key, cfg)
    in_maps = make_in_maps(inputs)
    kw = {}
    if trace:
        kw = {"trace": True, "tmpdir": tmpdir}
    res = run_bass_kernel_spmd(nc, in_maps, core_ids=list(range(N_CORES)),
                               **kw)
    out = np.empty((B, S, D), np.float32)
    for i in range(N_CORES):
        out[i * TB:(i + 1) * TB] = res.results[i]["out"].reshape(TB, S, D)
    return out, res


def kernel(**inputs):
    out, _ = run_on_hw(inputs)
    return out
